# revision 26
# baseline (speedup 1.0000x reference)
"""NodeAttention (GNN scatter-softmax attention) on 8 Trainium2 NeuronCores.

Strategy (v5 — on-chip KV, paired tiles):
- Host deals nodes to 8 cores round-robin by degree rank, so every core sees a
  near-identical degree profile; one static NEFF serves all cores (SPMD).
- Per core: 49 node-tiles x 128 nodes; node-tile t gets a dense padded slot
  grid [128, D] (D = max degree in the PAIR of tiles across cores; tiles are
  processed two-at-a-time so fixed per-instruction costs amortize over 2x the
  slots; adjacent degree-sorted tiles have near-identical D so pairing adds
  only ~2% padding).
- The host replicates x per SLOT in k-major order, so each KV projection
  matmul's PSUM output lands with the TARGET node on partitions. K is copied
  to SBUF as [P, slots, 64] and V as [P, 64, slots] (feature-major) straight
  from PSUM — no DRAM staging, no gather.
- Per-edge scores: bf16 QK mul in DVE 2x mode, fold-fold-reduce for the
  per-head dot; per-edge bias via block-diagonal matmuls (3 slots x 34
  ef-features on 102 partitions); exp on ACT; softmax normalization AFTER
  aggregation. Feature-major V lets the attn broadcast ride a middle dim so
  the weighted-V mul is also 2x, with a fold+reduce over slots.
- Big muls and one fold run on the otherwise-idle GpSimd (Pool) engine
  (SBUF-only ops; GPSIMD cannot touch PSUM). PSUM->SBUF copies + exp on ACT.
- LayerNorm runs chunked (4 output chunks) so the epilogue and y write-back
  overlap the main loop.
- No max-subtraction in softmax (scores are O(10); identical result).
  Padding slots masked via an extra edge-feature column (weight 1, value -75).
- temp/sqrt(d) folded into Wq; temp folded into We; be via a ones column;
  bo folded into the residual x shipped from host.
"""

import os
import json
import numpy as np
import ml_dtypes

import concourse.bass as bass
import concourse.bacc as bacc
import concourse.tile as tile
from concourse import mybir
from concourse.bass_utils import run_bass_kernel_spmd
from concourse.masks import make_identity

N, E = 50000, 800000
D_NODE, D_EDGE, H = 64, 32, 4
D_H = D_NODE // H
LN_EPS = 1e-5
NCORES = 8
P = 128
NT = 49                # node tiles per core
NPC = NT * P           # padded nodes per core = 6272
GRP = 2                # tiles processed per iteration
EF_R = 34              # 32 ef features + mask col + ones col (carries be)
EF3 = 3 * EF_R         # 102: three slots stacked on partitions
MASK_VAL = -75.0
F32 = mybir.dt.float32
BF16 = mybir.dt.bfloat16
BF_NP = ml_dtypes.bfloat16


# ---------------------------------------------------------------- host prep --
def _host_prep(node_features, edge_features, edge_index, Wq, bq, Wk, bk, Wv, bv,
               We, be, Wo, bo, ln_gamma, ln_beta, log_temp):
    x = np.ascontiguousarray(np.asarray(node_features, dtype=np.float32))
    ef = np.ascontiguousarray(np.asarray(edge_features, dtype=np.float32))
    src = np.asarray(edge_index[0], dtype=np.int64)
    tgt = np.asarray(edge_index[1], dtype=np.int64)
    temp = np.exp(np.asarray(log_temp, dtype=np.float32))

    deg = np.bincount(tgt, minlength=N)
    order = np.argsort(-deg, kind="stable")
    node_lists = []
    for c in range(NCORES):
        nl = order[c::NCORES]
        nl = np.concatenate([nl, np.full(NPC - len(nl), -1, dtype=np.int64)])
        node_lists.append(nl)

    D_t = np.zeros(NT, dtype=np.int64)
    for c in range(NCORES):
        d = np.where(node_lists[c] >= 0, deg[np.maximum(node_lists[c], 0)], 0)
        D_t = np.maximum(D_t, d.reshape(NT, P).max(axis=1))
    D_t = np.maximum(D_t, 2)
    D_t = D_t + (D_t % 2)          # even D so the aggregation fold halves cleanly
    # iterations: pairs of adjacent tiles sharing one D (last tile solo)
    iters = []                     # (t0, G, D)
    t = 0
    while t < NT:
        G = min(GRP, NT - t)
        D = int(D_t[t:t + G].max())
        D_t[t:t + G] = D
        iters.append((t, G, D))
        t += G
    assert D_t.max() <= 128, f"degree {D_t.max()} exceeds single-bank design"
    SD = int(D_t.sum())
    KC_i = [-(-(G * D) // 3) for (_, G, D) in iters]
    SKC = sum(KC_i)

    eorder = np.argsort(tgt, kind="stable")
    estart = np.zeros(N + 1, dtype=np.int64)
    np.cumsum(deg, out=estart[1:])

    qscale = (np.repeat(temp, D_H) / np.sqrt(D_H)).astype(np.float32)
    Wq_aug = (np.concatenate([np.asarray(Wq).T, np.asarray(bq)[None, :]], 0)
              * qscale[None, :]).astype(BF_NP)                           # [65,64]
    Wkv_aug = np.concatenate(
        [np.concatenate([np.asarray(Wk).T, np.asarray(Wv).T], 1),
         np.concatenate([np.asarray(bk), np.asarray(bv)])[None, :]], 0
    ).astype(BF_NP)                                                      # [65,128]
    We_augT = np.concatenate(
        [np.asarray(We).T * temp[None, :],
         np.ones((1, H), np.float32),
         (np.asarray(be) * temp)[None, :]], 0
    ).astype(np.float32)                                                 # [34,4]
    We_blk = np.zeros((EF3, 3 * H), dtype=np.float32)
    for j3 in range(3):
        We_blk[j3 * EF_R:(j3 + 1) * EF_R, j3 * H:(j3 + 1) * H] = We_augT
    We_blk = We_blk.astype(BF_NP)
    Wo_T = np.ascontiguousarray(np.asarray(Wo).T).astype(BF_NP)          # [64,64]
    gb = np.stack([np.asarray(ln_gamma), np.asarray(ln_beta)]).astype(np.float32)

    x_aug = np.concatenate(
        [x, np.ones((N, 1), np.float32)], 1).astype(BF_NP)               # [N,65]

    per_core = []
    for c in range(NCORES):
        nl = node_lists[c]
        efT = np.zeros((EF3, SKC * P), dtype=BF_NP)
        xTc = np.zeros((65, SD * P), dtype=BF_NP)
        coff = 0
        koff = 0
        for it, (t0, G, D) in enumerate(iters):
            SL = G * D
            KC = KC_i[it]
            # gather edge ids for each tile in the group: slot j = g*D + d
            nlt = nl[t0 * P:(t0 + G) * P].reshape(G, P)          # [G,P]
            degt = np.where(nlt >= 0, deg[np.maximum(nlt, 0)], 0)
            k = np.arange(D)
            valid = k[None, None, :] < degt[:, :, None]          # [G,P,D]
            pos = estart[np.maximum(nlt, 0)][:, :, None] + k[None, None, :]
            eids = eorder[np.minimum(pos, E - 1)]
            eids = np.where(valid, eids, 0)
            gsrc = np.where(valid, src[eids], -1)                # [G,P,D]
            # slot (p, j=g*D+d) lives at xTc column (coff + j)*128 + p
            cols = ((coff + np.arange(SL).reshape(G, 1, D)) * P
                    + np.arange(P)[None, :, None])               # [G,P,D]
            xv = np.where(valid.reshape(-1)[:, None],
                          x_aug[np.maximum(gsrc.reshape(-1), 0)],
                          0).astype(BF_NP)
            xTc[:, cols.reshape(-1)] = xv.T
            # edge-feature bias blocks over the group's SL slots
            blk = np.zeros((P, KC * 3, EF_R), dtype=np.float32)
            blk[:, :, D_EDGE] = MASK_VAL
            efv = np.where(valid[:, :, :, None], ef[eids], 0.0)  # [G,P,D,Ef]
            efv = efv.transpose(1, 0, 2, 3).reshape(P, SL, D_EDGE)
            vmask = valid.transpose(1, 0, 2).reshape(P, SL)
            blk[:, :SL, :D_EDGE] = efv
            blk[:, :SL, D_EDGE] = np.where(vmask, 0.0, MASK_VAL)
            blk[:, :, D_EDGE + 1] = 1.0
            # [P, KC, 3, EF_R] -> [3, EF_R, KC, P] -> [102, KC*128]
            efT[:, koff * P:(koff + KC) * P] = (
                blk.reshape(P, KC, 3, EF_R).transpose(2, 3, 1, 0)
                .reshape(EF3, KC * P).astype(BF_NP))
            coff += SL
            koff += KC
        xq = np.where(nl[:, None] >= 0, x[np.maximum(nl, 0)], 0.0).astype(np.float32)
        xqT_aug = np.concatenate([xq.T, np.ones((1, NPC), np.float32)],
                                 0).astype(BF_NP)
        xqr = (xq + np.asarray(bo, dtype=np.float32)[None, :])
        xq_g = np.ascontiguousarray(
            xqr.reshape(NT, P, D_NODE).transpose(1, 0, 2).reshape(P, NT * D_NODE))
        per_core.append({
            "efT": efT,
            "xTc": xTc,
            "xqT": np.ascontiguousarray(xqT_aug),
            "xq": xq_g,
            "wq": Wq_aug,
            "wkv": np.ascontiguousarray(Wkv_aug),
            "we": np.ascontiguousarray(We_blk),
            "wo": Wo_T,
            "gb": gb,
        })
    meta = dict(iters=iters)
    return per_core, node_lists, meta


# ------------------------------------------------------------- bass kernel --
def _build_kernel(meta, debug_mode=None):
    if debug_mode is None:
        debug_mode = os.environ.get("KERNEL_DEBUG_MODE", "")
    iters = meta["iters"]
    KC_i = [-(-(G * D) // 3) for (_, G, D) in iters]
    SD = sum(G * D for (_, G, D) in iters)
    SKC = sum(KC_i)
    # eft groups: ~7 DMAs over the run, aligned to iteration KC blocks
    NG = 7
    tgt_sz = -(-SKC // NG)
    gsz = []
    acc = 0
    for kc in KC_i:
        if acc + kc > tgt_sz and acc > 0:
            gsz.append(acc)
            acc = 0
        acc += kc
    gsz.append(acc)
    nc = bacc.Bacc(None, target_bir_lowering=False)

    # engine assignment knobs for the fungible ops ("dve" | "pool")
    ENG = dict(qk="pool", exv="pool", scf1="pool", scf2="dve",
               unnf1="dve", ln_a="pool", ln_b="dve")
    ENG.update(json.loads(os.environ.get("KERNEL_ENG", "{}")))
    CFG = dict(vsplit=0, chunked_ep=1, eftq="sync", wq_q="scalar",
               xqq="gpsimd")
    CFG.update(json.loads(os.environ.get("KERNEL_CFG", "{}")))

    def VE(key):
        return nc.gpsimd if ENG[key] == "pool" else nc.vector

    efT = nc.dram_tensor("efT", [EF3, SKC * P], BF16, kind="ExternalInput")
    xTc = nc.dram_tensor("xTc", [65, SD * P], BF16, kind="ExternalInput")
    xqT = nc.dram_tensor("xqT", [65, NPC], BF16, kind="ExternalInput")
    xq = nc.dram_tensor("xq", [P, NT * D_NODE], F32, kind="ExternalInput")
    wq = nc.dram_tensor("wq", [65, D_NODE], BF16, kind="ExternalInput")
    wkv = nc.dram_tensor("wkv", [65, 2 * D_NODE], BF16, kind="ExternalInput")
    we = nc.dram_tensor("we", [EF3, 3 * H], BF16, kind="ExternalInput")
    wo = nc.dram_tensor("wo", [D_NODE, D_NODE], BF16, kind="ExternalInput")
    gb = nc.dram_tensor("gb", [2, D_NODE], F32, kind="ExternalInput")
    y = nc.dram_tensor("y", [P, NT * D_NODE], F32, kind="ExternalOutput")

    with tile.TileContext(nc) as tc:
        with (
            tc.tile_pool(name="singles", bufs=1) as singles,
        ):
            wq_sb = singles.tile([65, D_NODE], BF16)
            getattr(nc, CFG["wq_q"]).dma_start(out=wq_sb[:], in_=wq[:])
            wkv_sb = singles.tile([65, 2 * D_NODE], BF16)
            getattr(nc, CFG["wq_q"]).dma_start(out=wkv_sb[:], in_=wkv[:])
            we_sb = singles.tile([EF3, 3 * H], BF16)
            getattr(nc, CFG["wq_q"]).dma_start(out=we_sb[:], in_=we[:])
            wo_sb = singles.tile([D_NODE, D_NODE], BF16)
            getattr(nc, CFG["wq_q"]).dma_start(out=wo_sb[:], in_=wo[:])
            gamma_sb = singles.tile([P, D_NODE], F32)
            getattr(nc, CFG["xqq"]).dma_start(
                out=gamma_sb[:],
                in_=bass.AP(tensor=gb[:].tensor, offset=0,
                            ap=[[0, P], [1, D_NODE]]))
            beta_sb = singles.tile([P, D_NODE], F32)
            getattr(nc, CFG["xqq"]).dma_start(
                out=beta_sb[:],
                in_=bass.AP(tensor=gb[:].tensor, offset=D_NODE,
                            ap=[[0, P], [1, D_NODE]]))
            xqT_sb = singles.tile([65, NPC], BF16)
            getattr(nc, CFG["wq_q"]).dma_start(out=xqT_sb[:], in_=xqT[:])
            xq_sb = singles.tile([P, NT, D_NODE], F32)
            getattr(nc, CFG["xqq"]).dma_start(out=xq_sb[:], in_=xq[:])
            ident = singles.tile([P, P], BF16)
            make_identity(nc, ident[:])
            eps_sb = singles.tile([P, 1], F32)
            nc.vector.memset(eps_sb[:], LN_EPS)
            q_all = singles.tile([P, NT, D_NODE], BF16)
            yout_sb = singles.tile([P, NT, D_NODE], F32)
            mv_sb = singles.tile([P, NT, 2], F32)
            sd_sb = singles.tile([P, NT], F32)
            rsd_sb = singles.tile([P, NT], F32)
            mursd_sb = singles.tile([P, NT], F32)

            with (
                tc.tile_pool(name="xtp", bufs=2) as xtp,
                tc.tile_pool(name="kvp", bufs=2, space="PSUM") as kvp,
                tc.tile_pool(name="ktp", bufs=2) as ktp,
                tc.tile_pool(name="vtp", bufs=2) as vtp,
                tc.tile_pool(name="eft", bufs=2) as eftp,
                tc.tile_pool(name="mid", bufs=2) as midp,
                tc.tile_pool(name="sml", bufs=2) as smlp,
                tc.tile_pool(name="pb", bufs=2, space="PSUM") as pb,
                tc.tile_pool(name="pt", bufs=1, space="PSUM") as ptp,
                tc.tile_pool(name="py", bufs=1, space="PSUM") as pyp,
            ):
                # ---- all Q' tiles upfront: 49 matmuls, 13 PSUM drains ----
                for g in range(-(-NT // 4)):
                    n4 = min(4, NT - g * 4)
                    qp = kvp.tile([P, 4, D_NODE], F32, name="qp", tag="kv")
                    for j in range(n4):
                        t = g * 4 + j
                        nc.tensor.matmul(
                            out=qp[:, j, :],
                            lhsT=xqT_sb[:, t * P:(t + 1) * P],
                            rhs=wq_sb[:], start=True, stop=True)
                    nc.scalar.copy(out=q_all[:, g * 4:g * 4 + n4, :],
                                   in_=qp[:, 0:n4, :])

                coff = 0
                koff = 0
                goff = 0
                gi = 0
                gleft = 0
                eft_sb = None
                ep_done = 0
                for it, (t0, G, D) in enumerate(iters):
                    SL = G * D
                    KC = KC_i[it]
                    # ---- per-group source features, k-major slot order ----
                    xt_sb = xtp.tile([65, SL * P], BF16, name="xt_sb", tag="xt")
                    nc.sync.dma_start(
                        out=xt_sb[:], in_=xTc[:, coff * P:(coff + SL) * P])
                    if gleft == 0:
                        skc = gsz[gi]
                        eft_sb = eftp.tile([EF3, skc, P], BF16, tag="eft",
                                           name="eft_sb")
                        getattr(nc, CFG["eftq"]).dma_start(
                            out=eft_sb[:], in_=efT[:, goff * P:(goff + skc) * P])
                        gbase = goff
                        goff += skc
                        gi += 1
                        gleft = skc
                    kbase = koff - gbase

                    # ---- KV build: target node on partitions, no DRAM ----
                    kt = ktp.tile([P, SL, D_NODE], BF16, name="kt", tag="kt")
                    vt = vtp.tile([P, D_NODE, SL], BF16, name="vt", tag="vt")
                    for c8 in range(-(-SL // 8)):
                        j0 = c8 * 8
                        kk = min(8, SL - j0)
                        pt = kvp.tile([P, 8, 2 * D_NODE], F32, name="pt",
                                      tag="kv")
                        for j in range(kk):
                            nc.tensor.matmul(
                                out=pt[:, j, :],
                                lhsT=xt_sb[:, (j0 + j) * P:(j0 + j + 1) * P],
                                rhs=wkv_sb[:], start=True, stop=True)
                        nc.scalar.copy(out=kt[:, j0:j0 + kk, :],
                                       in_=pt[:, 0:kk, 0:D_NODE])
                        vdst = vt[:, :, j0:j0 + kk]
                        vsrc = pt[:, 0:kk, D_NODE:2 * D_NODE].rearrange(
                            "p j w -> p w j")
                        if CFG["vsplit"] and (it * 7 + c8) % CFG["vsplit"] == 0:
                            nc.vector.tensor_copy(out=vdst, in_=vsrc)
                        else:
                            nc.scalar.copy(out=vdst, in_=vsrc)

                    # ---- per-edge bias: 3 slots per matmul ----
                    biasp = pb.tile([P, 3 * KC, H], F32, tag="biasp",
                                    name="biasp")
                    for k in range(KC):
                        nc.tensor.matmul(out=biasp[:, 3 * k:3 * (k + 1), :],
                                         lhsT=eft_sb[:, kbase + k, :],
                                         rhs=we_sb[:], start=True, stop=True)

                    # ---- scores: QK mul then fold-fold-reduce ----
                    qkp = midp.tile([P, SL, D_NODE], BF16, tag="qkp",
                                    name="qkp")
                    q_b = bass.AP(tensor=q_all[:].tensor,
                                  offset=q_all[:].offset + t0 * D_NODE,
                                  ap=[q_all[:].ap[0], [D_NODE, G], [0, D],
                                      [1, D_NODE]])
                    VE("qk").tensor_mul(
                        out=qkp[:].rearrange("p (g d) w -> p g d w", g=G),
                        in0=kt[:].rearrange("p (g d) w -> p g d w", g=G),
                        in1=q_b)
                    qk4 = qkp[:].rearrange("p s (h w) -> p s h w", h=H)
                    sf1 = smlp.tile([P, SL, H, 8], BF16, tag="sf1", name="sf1")
                    VE("scf1").tensor_add(out=sf1[:], in0=qk4[:, :, :, 0:8],
                                          in1=qk4[:, :, :, 8:16])
                    sf2 = smlp.tile([P, SL, H, 4], BF16, tag="sf2", name="sf2")
                    VE("scf2").tensor_add(out=sf2[:], in0=sf1[:, :, :, 0:4],
                                          in1=sf1[:, :, :, 4:8])
                    sc = smlp.tile([P, SL, H], F32, tag="sc", name="sc")
                    nc.vector.tensor_reduce(
                        out=sc[:], in_=sf2[:],
                        axis=mybir.AxisListType.X, op=mybir.AluOpType.add)
                    sc2 = smlp.tile([P, SL, H], F32, tag="sc2", name="sc2")
                    nc.vector.tensor_add(out=sc2[:], in0=sc[:],
                                         in1=biasp[:, 0:SL, :])
                    ex = smlp.tile([P, H, SL], BF16, tag="ex", name="ex")
                    nc.scalar.activation(
                        out=ex[:], in_=sc2[:].rearrange("p s h -> p h s"),
                        func=mybir.ActivationFunctionType.Exp)

                    den = smlp.tile([P, H, G], F32, tag="den", name="den")
                    nc.vector.tensor_reduce(
                        out=den[:],
                        in_=ex[:].rearrange("p h (g d) -> p h g d", g=G),
                        axis=mybir.AxisListType.X, op=mybir.AluOpType.add)
                    rden = smlp.tile([P, H, G], F32, tag="rden", name="rden")
                    nc.gpsimd.tensor_scalar_add(den[:], den[:], 1e-10)
                    nc.vector.reciprocal(out=rden[:], in_=den[:])

                    # ---- weighted V aggregation: mul, fold, reduce ----
                    exv = midp.tile([P, D_NODE, SL], BF16, tag="exv",
                                    name="exv")
                    ex_b = bass.AP(tensor=ex[:].tensor, offset=ex[:].offset,
                                   ap=[ex[:].ap[0], [SL, H], [0, D_H],
                                       [1, SL]])
                    VE("exv").tensor_mul(out=exv[:], in0=vt[:], in1=ex_b)
                    hD = D // 2
                    exv4 = exv[:].rearrange("p w (g d) -> p w g d", g=G)
                    uf1 = midp.tile([P, D_NODE, G, hD], BF16, tag="uf1",
                                    name="uf1")
                    VE("unnf1").tensor_add(out=uf1[:], in0=exv4[:, :, :, 0:hD],
                                           in1=exv4[:, :, :, hD:D])
                    unn = smlp.tile([P, D_NODE, G], F32, tag="unn", name="unn")
                    nc.vector.tensor_reduce(
                        out=unn[:], in_=uf1[:], axis=mybir.AxisListType.X,
                        op=mybir.AluOpType.add)
                    outn = smlp.tile([P, D_NODE, G], BF16, tag="outn",
                                     name="outn")
                    rden_b = bass.AP(tensor=rden[:].tensor,
                                     offset=rden[:].offset,
                                     ap=[rden[:].ap[0], [G, H], [0, D_H],
                                         [1, G]])
                    nc.gpsimd.tensor_mul(
                        out=outn[:].rearrange("p (h w) g -> p h w g", h=H),
                        in0=unn[:].rearrange("p (h w) g -> p h w g", h=H),
                        in1=rden_b)

                    # ---- projection: y = outn @ Wo.T + (x + bo) ----
                    tp = ptp.tile([D_NODE, G, P], BF16, tag="tp", name="tp")
                    for g in range(G):
                        nc.tensor.transpose(out=tp[:, g, :],
                                            in_=outn[:, :, g],
                                            identity=ident[:])
                    tps = smlp.tile([D_NODE, G, P], BF16, tag="tps",
                                    name="tps")
                    nc.vector.tensor_copy(out=tps[:], in_=tp[:])
                    yp = pyp.tile([P, G, D_NODE], F32, tag="yp", name="yp")
                    for g in range(G):
                        nc.tensor.matmul(out=yp[:, g, :], lhsT=tps[:, g, :],
                                         rhs=wo_sb[:], start=True, stop=True)
                    nc.vector.tensor_add(out=yout_sb[:, t0:t0 + G, :],
                                         in0=yp[:],
                                         in1=xq_sb[:, t0:t0 + G, :])
                    for g in range(G):
                        stats = smlp.tile([P, 6], F32, tag="stats",
                                          name="stats")
                        nc.vector.bn_stats(out=stats[:],
                                           in_=yout_sb[:, t0 + g, :])
                        nc.vector.bn_aggr(out=mv_sb[:, t0 + g, :],
                                          in_=stats[:])
                    coff += SL
                    koff += KC
                    gleft -= KC

                    # ---- chunked layernorm epilogue: overlap with loop ----
                    tend = t0 + G
                    ep_bounds = (12, 24, 36, NT) if CFG["chunked_ep"] else (NT,)
                    hit = [b for b in ep_bounds if ep_done < b <= tend]
                    if not debug_mode and hit:
                        te0, te1 = ep_done, tend
                        nch = te1 - te0
                        mvs = mv_sb[:, te0:te1, :]
                        mu = bass.AP(tensor=mvs.tensor, offset=mvs.offset,
                                     ap=[mvs.ap[0], [2, nch]])
                        var = bass.AP(tensor=mvs.tensor, offset=mvs.offset + 1,
                                      ap=[mvs.ap[0], [2, nch]])
                        # rsd = exp(-0.5*ln(var+eps)); ln+exp share one ACT
                        # function table (sqrt would force a table swap)
                        nc.scalar.activation(
                            out=sd_sb[:, te0:te1], in_=var,
                            func=mybir.ActivationFunctionType.Ln,
                            bias=eps_sb[:])
                        nc.scalar.activation(
                            out=rsd_sb[:, te0:te1], in_=sd_sb[:, te0:te1],
                            func=mybir.ActivationFunctionType.Exp,
                            scale=-0.5)
                        nc.vector.tensor_mul(out=mursd_sb[:, te0:te1], in0=mu,
                                             in1=rsd_sb[:, te0:te1])

                        def bc_t(a):   # [P, nch] -> [P, nch, 64]
                            return bass.AP(tensor=a.tensor, offset=a.offset,
                                           ap=list(a.ap) + [[0, D_NODE]])

                        def bc_f(a):   # [P, 64] -> [P, nch, 64]
                            return bass.AP(tensor=a.tensor, offset=a.offset,
                                           ap=[a.ap[0], [0, nch], a.ap[1]])

                        yv = yout_sb[:, te0:te1, :]
                        VE("ln_a").tensor_mul(out=yv, in0=yv,
                                              in1=bc_t(rsd_sb[:, te0:te1]))
                        VE("ln_b").tensor_sub(out=yv, in0=yv,
                                              in1=bc_t(mursd_sb[:, te0:te1]))
                        VE("ln_a").tensor_mul(out=yv, in0=yv,
                                              in1=bc_f(gamma_sb[:]))
                        VE("ln_b").tensor_add(out=yv, in0=yv,
                                              in1=bc_f(beta_sb[:]))
                        getattr(nc, CFG["xqq"]).dma_start(
                            out=y[:, te0 * D_NODE:te1 * D_NODE], in_=yv)
                        ep_done = te1

    nc.compile()
    return nc


# ------------------------------------------------------------------ driver --
def kernel(**inputs) -> np.ndarray:
    per_core, node_lists, meta = _host_prep(**inputs)
    nc = _build_kernel(meta)
    res = run_bass_kernel_spmd(nc, per_core, core_ids=list(range(NCORES)))
    y_full = np.zeros((N, D_NODE), dtype=np.float32)
    for c in range(NCORES):
        yc = res.results[c]["y"].reshape(P, NT, D_NODE).transpose(1, 0, 2)
        yc = yc.reshape(NPC, D_NODE)
        nl = node_lists[c]
        real = nl >= 0
        y_full[nl[real]] = yc[real]
    return y_full


# revision 27
# speedup vs baseline: 1.0393x; 1.0393x over previous
"""NodeAttention (GNN scatter-softmax attention) on 8 Trainium2 NeuronCores.

Strategy (v5 — on-chip KV, paired tiles):
- Host deals nodes to 8 cores round-robin by degree rank, so every core sees a
  near-identical degree profile; one static NEFF serves all cores (SPMD).
- Per core: 49 node-tiles x 128 nodes; node-tile t gets a dense padded slot
  grid [128, D] (D = max degree in the PAIR of tiles across cores; tiles are
  processed two-at-a-time so fixed per-instruction costs amortize over 2x the
  slots; adjacent degree-sorted tiles have near-identical D so pairing adds
  only ~2% padding).
- The host replicates x per SLOT in k-major order, so each KV projection
  matmul's PSUM output lands with the TARGET node on partitions. K is copied
  to SBUF as [P, slots, 64] and V as [P, 64, slots] (feature-major) straight
  from PSUM — no DRAM staging, no gather.
- Per-edge scores: bf16 QK mul in DVE 2x mode, fold-fold-reduce for the
  per-head dot; per-edge bias via block-diagonal matmuls (3 slots x 34
  ef-features on 102 partitions); exp on ACT; softmax normalization AFTER
  aggregation. Feature-major V lets the attn broadcast ride a middle dim so
  the weighted-V mul is also 2x, with a fold+reduce over slots.
- Big muls and one fold run on the otherwise-idle GpSimd (Pool) engine
  (SBUF-only ops; GPSIMD cannot touch PSUM). PSUM->SBUF copies + exp on ACT.
- LayerNorm runs chunked (4 output chunks) so the epilogue and y write-back
  overlap the main loop.
- No max-subtraction in softmax (scores are O(10); identical result).
  Padding slots masked via an extra edge-feature column (weight 1, value -75).
- temp/sqrt(d) folded into Wq; temp folded into We; be via a ones column;
  bo folded into the residual x shipped from host.
"""

import os
import json
import numpy as np
import ml_dtypes

import concourse.bass as bass
import concourse.bacc as bacc
import concourse.tile as tile
from concourse import mybir
from concourse.bass_utils import run_bass_kernel_spmd
from concourse.masks import make_identity

N, E = 50000, 800000
D_NODE, D_EDGE, H = 64, 32, 4
D_H = D_NODE // H
LN_EPS = 1e-5
NCORES = 8
P = 128
NT = 49                # node tiles per core
NPC = NT * P           # padded nodes per core = 6272
GRP = 2                # tiles processed per iteration
EF_R = 34              # 32 ef features + mask col + ones col (carries be)
EF3 = 3 * EF_R         # 102: three slots stacked on partitions
MASK_VAL = -75.0
F32 = mybir.dt.float32
BF16 = mybir.dt.bfloat16
BF_NP = ml_dtypes.bfloat16


# ---------------------------------------------------------------- host prep --
def _host_prep(node_features, edge_features, edge_index, Wq, bq, Wk, bk, Wv, bv,
               We, be, Wo, bo, ln_gamma, ln_beta, log_temp):
    x = np.ascontiguousarray(np.asarray(node_features, dtype=np.float32))
    ef = np.ascontiguousarray(np.asarray(edge_features, dtype=np.float32))
    src = np.asarray(edge_index[0], dtype=np.int64)
    tgt = np.asarray(edge_index[1], dtype=np.int64)
    temp = np.exp(np.asarray(log_temp, dtype=np.float32))

    deg = np.bincount(tgt, minlength=N)
    order = np.argsort(-deg, kind="stable")
    node_lists = []
    for c in range(NCORES):
        nl = order[c::NCORES]
        nl = np.concatenate([nl, np.full(NPC - len(nl), -1, dtype=np.int64)])
        node_lists.append(nl)

    D_t = np.zeros(NT, dtype=np.int64)
    for c in range(NCORES):
        d = np.where(node_lists[c] >= 0, deg[np.maximum(node_lists[c], 0)], 0)
        D_t = np.maximum(D_t, d.reshape(NT, P).max(axis=1))
    D_t = np.maximum(D_t, 2)
    D_t = D_t + (D_t % 2)          # even D so the aggregation fold halves cleanly
    # iterations: pairs of adjacent tiles sharing one D (last tile solo)
    iters = []                     # (t0, G, D)
    t = 0
    while t < NT:
        G = min(GRP, NT - t)
        D = int(D_t[t:t + G].max())
        D_t[t:t + G] = D
        iters.append((t, G, D))
        t += G
    assert D_t.max() <= 128, f"degree {D_t.max()} exceeds single-bank design"
    SD = int(D_t.sum())
    KC_i = [-(-(G * D) // 3) for (_, G, D) in iters]
    SKC = sum(KC_i)

    eorder = np.argsort(tgt, kind="stable")
    estart = np.zeros(N + 1, dtype=np.int64)
    np.cumsum(deg, out=estart[1:])

    qscale = (np.repeat(temp, D_H) / np.sqrt(D_H)).astype(np.float32)
    Wq_aug = (np.concatenate([np.asarray(Wq).T, np.asarray(bq)[None, :]], 0)
              * qscale[None, :]).astype(BF_NP)                           # [65,64]
    Wkv_aug = np.concatenate(
        [np.concatenate([np.asarray(Wk).T, np.asarray(Wv).T], 1),
         np.concatenate([np.asarray(bk), np.asarray(bv)])[None, :]], 0
    ).astype(BF_NP)                                                      # [65,128]
    We_augT = np.concatenate(
        [np.asarray(We).T * temp[None, :],
         np.ones((1, H), np.float32),
         (np.asarray(be) * temp)[None, :]], 0
    ).astype(np.float32)                                                 # [34,4]
    We_blk = np.zeros((EF3, 3 * H), dtype=np.float32)
    for j3 in range(3):
        We_blk[j3 * EF_R:(j3 + 1) * EF_R, j3 * H:(j3 + 1) * H] = We_augT
    We_blk = We_blk.astype(BF_NP)
    Wo_T = np.ascontiguousarray(np.asarray(Wo).T).astype(BF_NP)          # [64,64]
    gb = np.stack([np.asarray(ln_gamma), np.asarray(ln_beta)]).astype(np.float32)

    x_aug = np.concatenate(
        [x, np.ones((N, 1), np.float32)], 1).astype(BF_NP)               # [N,65]

    per_core = []
    for c in range(NCORES):
        nl = node_lists[c]
        efT = np.zeros((EF3, SKC * P), dtype=BF_NP)
        xTc = np.zeros((65, SD * P), dtype=BF_NP)
        coff = 0
        koff = 0
        for it, (t0, G, D) in enumerate(iters):
            SL = G * D
            KC = KC_i[it]
            # gather edge ids for each tile in the group: slot j = g*D + d
            nlt = nl[t0 * P:(t0 + G) * P].reshape(G, P)          # [G,P]
            degt = np.where(nlt >= 0, deg[np.maximum(nlt, 0)], 0)
            k = np.arange(D)
            valid = k[None, None, :] < degt[:, :, None]          # [G,P,D]
            pos = estart[np.maximum(nlt, 0)][:, :, None] + k[None, None, :]
            eids = eorder[np.minimum(pos, E - 1)]
            eids = np.where(valid, eids, 0)
            gsrc = np.where(valid, src[eids], -1)                # [G,P,D]
            # slot (p, j=g*D+d) lives at xTc column (coff + j)*128 + p
            cols = ((coff + np.arange(SL).reshape(G, 1, D)) * P
                    + np.arange(P)[None, :, None])               # [G,P,D]
            xv = np.where(valid.reshape(-1)[:, None],
                          x_aug[np.maximum(gsrc.reshape(-1), 0)],
                          0).astype(BF_NP)
            xTc[:, cols.reshape(-1)] = xv.T
            # edge-feature bias blocks over the group's SL slots
            blk = np.zeros((P, KC * 3, EF_R), dtype=np.float32)
            blk[:, :, D_EDGE] = MASK_VAL
            efv = np.where(valid[:, :, :, None], ef[eids], 0.0)  # [G,P,D,Ef]
            efv = efv.transpose(1, 0, 2, 3).reshape(P, SL, D_EDGE)
            vmask = valid.transpose(1, 0, 2).reshape(P, SL)
            blk[:, :SL, :D_EDGE] = efv
            blk[:, :SL, D_EDGE] = np.where(vmask, 0.0, MASK_VAL)
            blk[:, :, D_EDGE + 1] = 1.0
            # [P, KC, 3, EF_R] -> [3, EF_R, KC, P] -> [102, KC*128]
            efT[:, koff * P:(koff + KC) * P] = (
                blk.reshape(P, KC, 3, EF_R).transpose(2, 3, 1, 0)
                .reshape(EF3, KC * P).astype(BF_NP))
            coff += SL
            koff += KC
        xq = np.where(nl[:, None] >= 0, x[np.maximum(nl, 0)], 0.0).astype(np.float32)
        xqT_aug = np.concatenate([xq.T, np.ones((1, NPC), np.float32)],
                                 0).astype(BF_NP)
        xqr = (xq + np.asarray(bo, dtype=np.float32)[None, :])
        xq_g = np.ascontiguousarray(
            xqr.reshape(NT, P, D_NODE).transpose(1, 0, 2).reshape(P, NT * D_NODE))
        per_core.append({
            "efT": efT,
            "xTc": xTc,
            "xqT": np.ascontiguousarray(xqT_aug),
            "xq": xq_g,
            "wq": Wq_aug,
            "wkv": np.ascontiguousarray(Wkv_aug),
            "we": np.ascontiguousarray(We_blk),
            "wo": Wo_T,
            "gb": gb,
        })
    meta = dict(iters=iters)
    return per_core, node_lists, meta


# ------------------------------------------------------------- bass kernel --
def _build_kernel(meta, debug_mode=None):
    if debug_mode is None:
        debug_mode = os.environ.get("KERNEL_DEBUG_MODE", "")
    iters = meta["iters"]
    KC_i = [-(-(G * D) // 3) for (_, G, D) in iters]
    SD = sum(G * D for (_, G, D) in iters)
    SKC = sum(KC_i)
    # eft groups: ~7 DMAs over the run, aligned to iteration KC blocks
    NG = 7
    tgt_sz = -(-SKC // NG)
    gsz = []
    acc = 0
    for kc in KC_i:
        if acc + kc > tgt_sz and acc > 0:
            gsz.append(acc)
            acc = 0
        acc += kc
    gsz.append(acc)
    nc = bacc.Bacc(None, target_bir_lowering=False)

    # engine assignment knobs for the fungible ops ("dve" | "pool")
    ENG = dict(qk="pool", exv="pool", scf1="pool", scf2="dve",
               unnf1="dve", ln_a="pool", ln_b="dve")
    ENG.update(json.loads(os.environ.get("KERNEL_ENG", "{}")))
    CFG = dict(vsplit=0, chunked_ep=1, eftq="sync", wq_q="scalar",
               xqq="gpsimd")
    CFG.update(json.loads(os.environ.get("KERNEL_CFG", "{}")))

    def VE(key):
        return nc.gpsimd if ENG[key] == "pool" else nc.vector

    efT = nc.dram_tensor("efT", [EF3, SKC * P], BF16, kind="ExternalInput")
    xTc = nc.dram_tensor("xTc", [65, SD * P], BF16, kind="ExternalInput")
    xqT = nc.dram_tensor("xqT", [65, NPC], BF16, kind="ExternalInput")
    xq = nc.dram_tensor("xq", [P, NT * D_NODE], F32, kind="ExternalInput")
    wq = nc.dram_tensor("wq", [65, D_NODE], BF16, kind="ExternalInput")
    wkv = nc.dram_tensor("wkv", [65, 2 * D_NODE], BF16, kind="ExternalInput")
    we = nc.dram_tensor("we", [EF3, 3 * H], BF16, kind="ExternalInput")
    wo = nc.dram_tensor("wo", [D_NODE, D_NODE], BF16, kind="ExternalInput")
    gb = nc.dram_tensor("gb", [2, D_NODE], F32, kind="ExternalInput")
    y = nc.dram_tensor("y", [P, NT * D_NODE], F32, kind="ExternalOutput")

    with tile.TileContext(nc) as tc:
        with (
            tc.tile_pool(name="singles", bufs=1) as singles,
        ):
            wq_sb = singles.tile([65, D_NODE], BF16)
            getattr(nc, CFG["wq_q"]).dma_start(out=wq_sb[:], in_=wq[:])
            wkv_sb = singles.tile([65, 2 * D_NODE], BF16)
            getattr(nc, CFG["wq_q"]).dma_start(out=wkv_sb[:], in_=wkv[:])
            we_sb = singles.tile([EF3, 3 * H], BF16)
            getattr(nc, CFG["wq_q"]).dma_start(out=we_sb[:], in_=we[:])
            wo_sb = singles.tile([D_NODE, D_NODE], BF16)
            getattr(nc, CFG["wq_q"]).dma_start(out=wo_sb[:], in_=wo[:])
            gamma_sb = singles.tile([P, D_NODE], F32)
            getattr(nc, CFG["xqq"]).dma_start(
                out=gamma_sb[:],
                in_=bass.AP(tensor=gb[:].tensor, offset=0,
                            ap=[[0, P], [1, D_NODE]]))
            beta_sb = singles.tile([P, D_NODE], F32)
            getattr(nc, CFG["xqq"]).dma_start(
                out=beta_sb[:],
                in_=bass.AP(tensor=gb[:].tensor, offset=D_NODE,
                            ap=[[0, P], [1, D_NODE]]))
            xqT_sb = singles.tile([65, NPC], BF16)
            getattr(nc, CFG["wq_q"]).dma_start(out=xqT_sb[:], in_=xqT[:])
            xq_sb = singles.tile([P, NT, D_NODE], F32)
            getattr(nc, CFG["xqq"]).dma_start(out=xq_sb[:], in_=xq[:])
            ident = singles.tile([P, P], BF16)
            make_identity(nc, ident[:])
            eps_sb = singles.tile([P, 1], F32)
            nc.vector.memset(eps_sb[:], LN_EPS)
            q_all = singles.tile([P, NT, D_NODE], BF16)
            yout_sb = singles.tile([P, NT, D_NODE], F32)
            mv_sb = singles.tile([P, NT, 2], F32)
            sd_sb = singles.tile([P, NT], F32)
            rsd_sb = singles.tile([P, NT], F32)
            mursd_sb = singles.tile([P, NT], F32)

            with (
                tc.tile_pool(name="xtp", bufs=2) as xtp,
                tc.tile_pool(name="kvp", bufs=2, space="PSUM") as kvp,
                tc.tile_pool(name="ktp", bufs=2) as ktp,
                tc.tile_pool(name="vtp", bufs=2) as vtp,
                tc.tile_pool(name="eft", bufs=2) as eftp,
                tc.tile_pool(name="mid", bufs=2) as midp,
                tc.tile_pool(name="sml", bufs=2) as smlp,
                tc.tile_pool(name="pb", bufs=2, space="PSUM") as pb,
                tc.tile_pool(name="pt", bufs=1, space="PSUM") as ptp,
                tc.tile_pool(name="py", bufs=1, space="PSUM") as pyp,
            ):
                # pin the ln+exp+copy ACT table once; the auto-placement pass
                # would otherwise bounce between exp-only and ln tables
                nc.scalar.add_instruction(mybir.InstLoadActFuncSet(
                    name=nc.get_next_instruction_name(), ins=[], outs=[],
                    act_func_set_id=6))

                # ---- all Q' tiles upfront: 49 matmuls, 13 PSUM drains ----
                for g in range(-(-NT // 4)):
                    n4 = min(4, NT - g * 4)
                    qp = kvp.tile([P, 4, D_NODE], F32, name="qp", tag="kv")
                    for j in range(n4):
                        t = g * 4 + j
                        nc.tensor.matmul(
                            out=qp[:, j, :],
                            lhsT=xqT_sb[:, t * P:(t + 1) * P],
                            rhs=wq_sb[:], start=True, stop=True)
                    nc.scalar.copy(out=q_all[:, g * 4:g * 4 + n4, :],
                                   in_=qp[:, 0:n4, :])

                coff = 0
                koff = 0
                goff = 0
                gi = 0
                gleft = 0
                eft_sb = None
                ep_done = 0
                for it, (t0, G, D) in enumerate(iters):
                    SL = G * D
                    KC = KC_i[it]
                    # ---- per-group source features, k-major slot order ----
                    xt_sb = xtp.tile([65, SL * P], BF16, name="xt_sb", tag="xt")
                    nc.sync.dma_start(
                        out=xt_sb[:], in_=xTc[:, coff * P:(coff + SL) * P])
                    if gleft == 0:
                        skc = gsz[gi]
                        eft_sb = eftp.tile([EF3, skc, P], BF16, tag="eft",
                                           name="eft_sb")
                        getattr(nc, CFG["eftq"]).dma_start(
                            out=eft_sb[:], in_=efT[:, goff * P:(goff + skc) * P])
                        gbase = goff
                        goff += skc
                        gi += 1
                        gleft = skc
                    kbase = koff - gbase

                    # ---- KV build: target node on partitions, no DRAM ----
                    kt = ktp.tile([P, SL, D_NODE], BF16, name="kt", tag="kt")
                    vt = vtp.tile([P, D_NODE, SL], BF16, name="vt", tag="vt")
                    for c8 in range(-(-SL // 8)):
                        j0 = c8 * 8
                        kk = min(8, SL - j0)
                        pt = kvp.tile([P, 8, 2 * D_NODE], F32, name="pt",
                                      tag="kv")
                        for j in range(kk):
                            nc.tensor.matmul(
                                out=pt[:, j, :],
                                lhsT=xt_sb[:, (j0 + j) * P:(j0 + j + 1) * P],
                                rhs=wkv_sb[:], start=True, stop=True)
                        nc.scalar.copy(out=kt[:, j0:j0 + kk, :],
                                       in_=pt[:, 0:kk, 0:D_NODE])
                        vdst = vt[:, :, j0:j0 + kk]
                        vsrc = pt[:, 0:kk, D_NODE:2 * D_NODE].rearrange(
                            "p j w -> p w j")
                        if CFG["vsplit"] and (it * 7 + c8) % CFG["vsplit"] == 0:
                            nc.vector.tensor_copy(out=vdst, in_=vsrc)
                        else:
                            nc.scalar.copy(out=vdst, in_=vsrc)

                    # ---- per-edge bias: 3 slots per matmul ----
                    biasp = pb.tile([P, 3 * KC, H], F32, tag="biasp",
                                    name="biasp")
                    for k in range(KC):
                        nc.tensor.matmul(out=biasp[:, 3 * k:3 * (k + 1), :],
                                         lhsT=eft_sb[:, kbase + k, :],
                                         rhs=we_sb[:], start=True, stop=True)

                    # ---- scores: QK mul then fold-fold-reduce ----
                    qkp = midp.tile([P, SL, D_NODE], BF16, tag="qkp",
                                    name="qkp")
                    q_b = bass.AP(tensor=q_all[:].tensor,
                                  offset=q_all[:].offset + t0 * D_NODE,
                                  ap=[q_all[:].ap[0], [D_NODE, G], [0, D],
                                      [1, D_NODE]])
                    VE("qk").tensor_mul(
                        out=qkp[:].rearrange("p (g d) w -> p g d w", g=G),
                        in0=kt[:].rearrange("p (g d) w -> p g d w", g=G),
                        in1=q_b)
                    qk4 = qkp[:].rearrange("p s (h w) -> p s h w", h=H)
                    sf1 = smlp.tile([P, SL, H, 8], BF16, tag="sf1", name="sf1")
                    VE("scf1").tensor_add(out=sf1[:], in0=qk4[:, :, :, 0:8],
                                          in1=qk4[:, :, :, 8:16])
                    sf2 = smlp.tile([P, SL, H, 4], BF16, tag="sf2", name="sf2")
                    VE("scf2").tensor_add(out=sf2[:], in0=sf1[:, :, :, 0:4],
                                          in1=sf1[:, :, :, 4:8])
                    sc = smlp.tile([P, SL, H], F32, tag="sc", name="sc")
                    nc.vector.tensor_reduce(
                        out=sc[:], in_=sf2[:],
                        axis=mybir.AxisListType.X, op=mybir.AluOpType.add)
                    sc2 = smlp.tile([P, SL, H], F32, tag="sc2", name="sc2")
                    nc.vector.tensor_add(out=sc2[:], in0=sc[:],
                                         in1=biasp[:, 0:SL, :])
                    ex = smlp.tile([P, H, SL], BF16, tag="ex", name="ex")
                    nc.scalar.activation(
                        out=ex[:], in_=sc2[:].rearrange("p s h -> p h s"),
                        func=mybir.ActivationFunctionType.Exp)

                    den = smlp.tile([P, H, G], F32, tag="den", name="den")
                    nc.vector.tensor_reduce(
                        out=den[:],
                        in_=ex[:].rearrange("p h (g d) -> p h g d", g=G),
                        axis=mybir.AxisListType.X, op=mybir.AluOpType.add)
                    rden = smlp.tile([P, H, G], F32, tag="rden", name="rden")
                    nc.gpsimd.tensor_scalar_add(den[:], den[:], 1e-10)
                    nc.vector.reciprocal(out=rden[:], in_=den[:])

                    # ---- weighted V aggregation: mul, fold, reduce ----
                    exv = midp.tile([P, D_NODE, SL], BF16, tag="exv",
                                    name="exv")
                    ex_b = bass.AP(tensor=ex[:].tensor, offset=ex[:].offset,
                                   ap=[ex[:].ap[0], [SL, H], [0, D_H],
                                       [1, SL]])
                    VE("exv").tensor_mul(out=exv[:], in0=vt[:], in1=ex_b)
                    hD = D // 2
                    exv4 = exv[:].rearrange("p w (g d) -> p w g d", g=G)
                    uf1 = midp.tile([P, D_NODE, G, hD], BF16, tag="uf1",
                                    name="uf1")
                    VE("unnf1").tensor_add(out=uf1[:], in0=exv4[:, :, :, 0:hD],
                                           in1=exv4[:, :, :, hD:D])
                    unn = smlp.tile([P, D_NODE, G], F32, tag="unn", name="unn")
                    nc.vector.tensor_reduce(
                        out=unn[:], in_=uf1[:], axis=mybir.AxisListType.X,
                        op=mybir.AluOpType.add)
                    outn = smlp.tile([P, D_NODE, G], BF16, tag="outn",
                                     name="outn")
                    rden_b = bass.AP(tensor=rden[:].tensor,
                                     offset=rden[:].offset,
                                     ap=[rden[:].ap[0], [G, H], [0, D_H],
                                         [1, G]])
                    nc.gpsimd.tensor_mul(
                        out=outn[:].rearrange("p (h w) g -> p h w g", h=H),
                        in0=unn[:].rearrange("p (h w) g -> p h w g", h=H),
                        in1=rden_b)

                    # ---- projection: y = outn @ Wo.T + (x + bo) ----
                    tp = ptp.tile([D_NODE, G, P], BF16, tag="tp", name="tp")
                    for g in range(G):
                        nc.tensor.transpose(out=tp[:, g, :],
                                            in_=outn[:, :, g],
                                            identity=ident[:])
                    tps = smlp.tile([D_NODE, G, P], BF16, tag="tps",
                                    name="tps")
                    nc.vector.tensor_copy(out=tps[:], in_=tp[:])
                    yp = pyp.tile([P, G, D_NODE], F32, tag="yp", name="yp")
                    for g in range(G):
                        nc.tensor.matmul(out=yp[:, g, :], lhsT=tps[:, g, :],
                                         rhs=wo_sb[:], start=True, stop=True)
                    nc.vector.tensor_add(out=yout_sb[:, t0:t0 + G, :],
                                         in0=yp[:],
                                         in1=xq_sb[:, t0:t0 + G, :])
                    for g in range(G):
                        stats = smlp.tile([P, 6], F32, tag="stats",
                                          name="stats")
                        nc.vector.bn_stats(out=stats[:],
                                           in_=yout_sb[:, t0 + g, :])
                        nc.vector.bn_aggr(out=mv_sb[:, t0 + g, :],
                                          in_=stats[:])
                    coff += SL
                    koff += KC
                    gleft -= KC

                    # ---- chunked layernorm epilogue: overlap with loop ----
                    tend = t0 + G
                    ep_bounds = (12, 24, 36, NT) if CFG["chunked_ep"] else (NT,)
                    hit = [b for b in ep_bounds if ep_done < b <= tend]
                    if not debug_mode and hit:
                        te0, te1 = ep_done, tend
                        nch = te1 - te0
                        mvs = mv_sb[:, te0:te1, :]
                        mu = bass.AP(tensor=mvs.tensor, offset=mvs.offset,
                                     ap=[mvs.ap[0], [2, nch]])
                        var = bass.AP(tensor=mvs.tensor, offset=mvs.offset + 1,
                                      ap=[mvs.ap[0], [2, nch]])
                        # rsd = exp(-0.5*ln(var+eps)); ln+exp share one ACT
                        # function table (sqrt would force a table swap)
                        nc.scalar.activation(
                            out=sd_sb[:, te0:te1], in_=var,
                            func=mybir.ActivationFunctionType.Ln,
                            bias=eps_sb[:])
                        nc.scalar.activation(
                            out=rsd_sb[:, te0:te1], in_=sd_sb[:, te0:te1],
                            func=mybir.ActivationFunctionType.Exp,
                            scale=-0.5)
                        nc.vector.tensor_mul(out=mursd_sb[:, te0:te1], in0=mu,
                                             in1=rsd_sb[:, te0:te1])

                        def bc_t(a):   # [P, nch] -> [P, nch, 64]
                            return bass.AP(tensor=a.tensor, offset=a.offset,
                                           ap=list(a.ap) + [[0, D_NODE]])

                        def bc_f(a):   # [P, 64] -> [P, nch, 64]
                            return bass.AP(tensor=a.tensor, offset=a.offset,
                                           ap=[a.ap[0], [0, nch], a.ap[1]])

                        yv = yout_sb[:, te0:te1, :]
                        VE("ln_a").tensor_mul(out=yv, in0=yv,
                                              in1=bc_t(rsd_sb[:, te0:te1]))
                        VE("ln_b").tensor_sub(out=yv, in0=yv,
                                              in1=bc_t(mursd_sb[:, te0:te1]))
                        VE("ln_a").tensor_mul(out=yv, in0=yv,
                                              in1=bc_f(gamma_sb[:]))
                        VE("ln_b").tensor_add(out=yv, in0=yv,
                                              in1=bc_f(beta_sb[:]))
                        getattr(nc, CFG["xqq"]).dma_start(
                            out=y[:, te0 * D_NODE:te1 * D_NODE], in_=yv)
                        ep_done = te1

    nc.compile()
    return nc


# ------------------------------------------------------------------ driver --
def kernel(**inputs) -> np.ndarray:
    per_core, node_lists, meta = _host_prep(**inputs)
    nc = _build_kernel(meta)
    res = run_bass_kernel_spmd(nc, per_core, core_ids=list(range(NCORES)))
    y_full = np.zeros((N, D_NODE), dtype=np.float32)
    for c in range(NCORES):
        yc = res.results[c]["y"].reshape(P, NT, D_NODE).transpose(1, 0, 2)
        yc = yc.reshape(NPC, D_NODE)
        nl = node_lists[c]
        real = nl >= 0
        y_full[nl[real]] = yc[real]
    return y_full


# revision 31
# speedup vs baseline: 1.2927x; 1.2437x over previous
"""NodeAttention (GNN scatter-softmax attention) on 8 Trainium2 NeuronCores.

Strategy (v5 — on-chip KV, paired tiles):
- Host deals nodes to 8 cores round-robin by degree rank, so every core sees a
  near-identical degree profile; one static NEFF serves all cores (SPMD).
- Per core: 49 node-tiles x 128 nodes; node-tile t gets a dense padded slot
  grid [128, D] (D = max degree in the PAIR of tiles across cores; tiles are
  processed two-at-a-time so fixed per-instruction costs amortize over 2x the
  slots; adjacent degree-sorted tiles have near-identical D so pairing adds
  only ~2% padding).
- The host replicates x per SLOT in k-major order, so each KV projection
  matmul's PSUM output lands with the TARGET node on partitions. K is copied
  to SBUF as [P, slots, 64] and V as [P, 64, slots] (feature-major) straight
  from PSUM — no DRAM staging, no gather.
- Per-edge scores: bf16 QK mul in DVE 2x mode, fold-fold-reduce for the
  per-head dot; per-edge bias via block-diagonal matmuls (3 slots x 34
  ef-features on 102 partitions); exp on ACT; softmax normalization AFTER
  aggregation. Feature-major V lets the attn broadcast ride a middle dim so
  the weighted-V mul is also 2x, with a fold+reduce over slots.
- Big muls and one fold run on the otherwise-idle GpSimd (Pool) engine
  (SBUF-only ops; GPSIMD cannot touch PSUM). PSUM->SBUF copies + exp on ACT.
- LayerNorm runs chunked (4 output chunks) so the epilogue and y write-back
  overlap the main loop.
- No max-subtraction in softmax (scores are O(10); identical result).
  Padding slots masked via an extra edge-feature column (weight 1, value -75).
- temp/sqrt(d) folded into Wq; temp folded into We; be via a ones column;
  bo folded into the residual x shipped from host.
"""

import os
import json
import numpy as np
import ml_dtypes

import concourse.bass as bass
import concourse.bacc as bacc
import concourse.tile as tile
from concourse import mybir
from concourse.bass_utils import run_bass_kernel_spmd
from concourse.masks import make_identity

N, E = 50000, 800000
D_NODE, D_EDGE, H = 64, 32, 4
D_H = D_NODE // H
LN_EPS = 1e-5
NCORES = 8
P = 128
NT = 49                # node tiles per core
NPC = NT * P           # padded nodes per core = 6272
GRP = int(os.environ.get('KERNEL_GRP', '1'))   # tiles per iteration
EF_R = 34              # 32 ef features + mask col + ones col (carries be)
EF3 = 3 * EF_R         # 102: three slots stacked on partitions
MASK_VAL = -75.0
F32 = mybir.dt.float32
BF16 = mybir.dt.bfloat16
BF_NP = ml_dtypes.bfloat16


# ---------------------------------------------------------------- host prep --
def _host_prep(node_features, edge_features, edge_index, Wq, bq, Wk, bk, Wv, bv,
               We, be, Wo, bo, ln_gamma, ln_beta, log_temp):
    x = np.ascontiguousarray(np.asarray(node_features, dtype=np.float32))
    ef = np.ascontiguousarray(np.asarray(edge_features, dtype=np.float32))
    src = np.asarray(edge_index[0], dtype=np.int64)
    tgt = np.asarray(edge_index[1], dtype=np.int64)
    temp = np.exp(np.asarray(log_temp, dtype=np.float32))

    deg = np.bincount(tgt, minlength=N)
    order = np.argsort(-deg, kind="stable")
    node_lists = []
    for c in range(NCORES):
        nl = order[c::NCORES]
        nl = np.concatenate([nl, np.full(NPC - len(nl), -1, dtype=np.int64)])
        node_lists.append(nl)

    D_t = np.zeros(NT, dtype=np.int64)
    for c in range(NCORES):
        d = np.where(node_lists[c] >= 0, deg[np.maximum(node_lists[c], 0)], 0)
        D_t = np.maximum(D_t, d.reshape(NT, P).max(axis=1))
    D_t = np.maximum(D_t, 2)
    D_t = D_t + (D_t % 2)          # even D so the aggregation fold halves cleanly
    # iterations: pairs of adjacent tiles sharing one D (last tile solo)
    iters = []                     # (t0, G, D)
    t = 0
    while t < NT:
        G = min(GRP, NT - t)
        D = int(D_t[t:t + G].max())
        D_t[t:t + G] = D
        iters.append((t, G, D))
        t += G
    assert D_t.max() <= 128, f"degree {D_t.max()} exceeds single-bank design"
    SD = int(D_t.sum())
    KC_i = [-(-(G * D) // 3) for (_, G, D) in iters]
    SKC = sum(KC_i)

    eorder = np.argsort(tgt, kind="stable")
    estart = np.zeros(N + 1, dtype=np.int64)
    np.cumsum(deg, out=estart[1:])

    qscale = (np.repeat(temp, D_H) / np.sqrt(D_H)).astype(np.float32)
    Wq_aug = (np.concatenate([np.asarray(Wq).T, np.asarray(bq)[None, :]], 0)
              * qscale[None, :]).astype(BF_NP)                           # [65,64]
    Wkv_aug = np.concatenate(
        [np.concatenate([np.asarray(Wk).T, np.asarray(Wv).T], 1),
         np.concatenate([np.asarray(bk), np.asarray(bv)])[None, :]], 0
    ).astype(BF_NP)                                                      # [65,128]
    We_augT = np.concatenate(
        [np.asarray(We).T * temp[None, :],
         np.ones((1, H), np.float32),
         (np.asarray(be) * temp)[None, :]], 0
    ).astype(np.float32)                                                 # [34,4]
    We_blk = np.zeros((EF3, 3 * H), dtype=np.float32)
    for j3 in range(3):
        We_blk[j3 * EF_R:(j3 + 1) * EF_R, j3 * H:(j3 + 1) * H] = We_augT
    We_blk = We_blk.astype(BF_NP)
    Wo_T = np.ascontiguousarray(np.asarray(Wo).T).astype(BF_NP)          # [64,64]
    gb = np.stack([np.asarray(ln_gamma), np.asarray(ln_beta)]).astype(np.float32)

    x_aug = np.concatenate(
        [x, np.ones((N, 1), np.float32)], 1).astype(BF_NP)               # [N,65]

    per_core = []
    for c in range(NCORES):
        nl = node_lists[c]
        efT = np.zeros((EF3, SKC * P), dtype=BF_NP)
        xTc = np.zeros((65, SD * P), dtype=BF_NP)
        coff = 0
        koff = 0
        for it, (t0, G, D) in enumerate(iters):
            SL = G * D
            KC = KC_i[it]
            # gather edge ids for each tile in the group: slot j = g*D + d
            nlt = nl[t0 * P:(t0 + G) * P].reshape(G, P)          # [G,P]
            degt = np.where(nlt >= 0, deg[np.maximum(nlt, 0)], 0)
            k = np.arange(D)
            valid = k[None, None, :] < degt[:, :, None]          # [G,P,D]
            pos = estart[np.maximum(nlt, 0)][:, :, None] + k[None, None, :]
            eids = eorder[np.minimum(pos, E - 1)]
            eids = np.where(valid, eids, 0)
            gsrc = np.where(valid, src[eids], -1)                # [G,P,D]
            # slot (p, j=g*D+d) lives at xTc column (coff + j)*128 + p
            cols = ((coff + np.arange(SL).reshape(G, 1, D)) * P
                    + np.arange(P)[None, :, None])               # [G,P,D]
            xv = np.where(valid.reshape(-1)[:, None],
                          x_aug[np.maximum(gsrc.reshape(-1), 0)],
                          0).astype(BF_NP)
            xTc[:, cols.reshape(-1)] = xv.T
            # edge-feature bias blocks over the group's SL slots
            blk = np.zeros((P, KC * 3, EF_R), dtype=np.float32)
            blk[:, :, D_EDGE] = MASK_VAL
            efv = np.where(valid[:, :, :, None], ef[eids], 0.0)  # [G,P,D,Ef]
            efv = efv.transpose(1, 0, 2, 3).reshape(P, SL, D_EDGE)
            vmask = valid.transpose(1, 0, 2).reshape(P, SL)
            blk[:, :SL, :D_EDGE] = efv
            blk[:, :SL, D_EDGE] = np.where(vmask, 0.0, MASK_VAL)
            blk[:, :, D_EDGE + 1] = 1.0
            # [P, KC, 3, EF_R] -> [3, EF_R, KC, P] -> [102, KC*128]
            efT[:, koff * P:(koff + KC) * P] = (
                blk.reshape(P, KC, 3, EF_R).transpose(2, 3, 1, 0)
                .reshape(EF3, KC * P).astype(BF_NP))
            coff += SL
            koff += KC
        xq = np.where(nl[:, None] >= 0, x[np.maximum(nl, 0)], 0.0).astype(np.float32)
        xqT_aug = np.concatenate([xq.T, np.ones((1, NPC), np.float32)],
                                 0).astype(BF_NP)
        xqr = (xq + np.asarray(bo, dtype=np.float32)[None, :])
        xq_g = np.ascontiguousarray(
            xqr.reshape(NT, P, D_NODE).transpose(1, 0, 2).reshape(P, NT * D_NODE))
        per_core.append({
            "efT": efT,
            "xTc": xTc,
            "xqT": np.ascontiguousarray(xqT_aug),
            "xq": xq_g,
            "wq": Wq_aug,
            "wkv": np.ascontiguousarray(Wkv_aug),
            "we": np.ascontiguousarray(We_blk),
            "wo": Wo_T,
            "gb": gb,
        })
    meta = dict(iters=iters)
    return per_core, node_lists, meta


# ------------------------------------------------------------- bass kernel --
def _build_kernel(meta, debug_mode=None):
    if debug_mode is None:
        debug_mode = os.environ.get("KERNEL_DEBUG_MODE", "")
    iters = meta["iters"]
    KC_i = [-(-(G * D) // 3) for (_, G, D) in iters]
    SD = sum(G * D for (_, G, D) in iters)
    SKC = sum(KC_i)
    # eft groups: ~7 DMAs over the run, aligned to iteration KC blocks
    NG = 7
    tgt_sz = -(-SKC // NG)
    gsz = []
    acc = 0
    for kc in KC_i:
        if acc + kc > tgt_sz and acc > 0:
            gsz.append(acc)
            acc = 0
        acc += kc
    gsz.append(acc)
    nc = bacc.Bacc(None, target_bir_lowering=False)

    # engine assignment knobs for the fungible ops ("dve" | "pool")
    ENG = dict(qk="pool", exv="pool", scf1="dve", scf2="dve",
               unnf1="pool", ln_a="pool", ln_b="dve")
    ENG.update(json.loads(os.environ.get("KERNEL_ENG", "{}")))
    CFG = dict(vsplit=0, chunked_ep=1, eftq="sync", wq_q="scalar",
               xqq="gpsimd", xtb=3, smb=4, midb=4, ktb=4, ksplit=0, chunk=8,
               kvb=2)
    CFG.update(json.loads(os.environ.get("KERNEL_CFG", "{}")))

    def VE(key):
        return nc.gpsimd if ENG[key] == "pool" else nc.vector

    efT = nc.dram_tensor("efT", [EF3, SKC * P], BF16, kind="ExternalInput")
    xTc = nc.dram_tensor("xTc", [65, SD * P], BF16, kind="ExternalInput")
    xqT = nc.dram_tensor("xqT", [65, NPC], BF16, kind="ExternalInput")
    xq = nc.dram_tensor("xq", [P, NT * D_NODE], F32, kind="ExternalInput")
    wq = nc.dram_tensor("wq", [65, D_NODE], BF16, kind="ExternalInput")
    wkv = nc.dram_tensor("wkv", [65, 2 * D_NODE], BF16, kind="ExternalInput")
    we = nc.dram_tensor("we", [EF3, 3 * H], BF16, kind="ExternalInput")
    wo = nc.dram_tensor("wo", [D_NODE, D_NODE], BF16, kind="ExternalInput")
    gb = nc.dram_tensor("gb", [2, D_NODE], F32, kind="ExternalInput")
    y = nc.dram_tensor("y", [P, NT * D_NODE], F32, kind="ExternalOutput")

    with tile.TileContext(nc) as tc:
        with (
            tc.tile_pool(name="singles", bufs=1) as singles,
        ):
            wq_sb = singles.tile([65, D_NODE], BF16)
            getattr(nc, CFG["wq_q"]).dma_start(out=wq_sb[:], in_=wq[:])
            wkv_sb = singles.tile([65, 2 * D_NODE], BF16)
            getattr(nc, CFG["wq_q"]).dma_start(out=wkv_sb[:], in_=wkv[:])
            we_sb = singles.tile([EF3, 3 * H], BF16)
            getattr(nc, CFG["wq_q"]).dma_start(out=we_sb[:], in_=we[:])
            wo_sb = singles.tile([D_NODE, D_NODE], BF16)
            getattr(nc, CFG["wq_q"]).dma_start(out=wo_sb[:], in_=wo[:])
            gamma_sb = singles.tile([P, D_NODE], F32)
            getattr(nc, CFG["xqq"]).dma_start(
                out=gamma_sb[:],
                in_=bass.AP(tensor=gb[:].tensor, offset=0,
                            ap=[[0, P], [1, D_NODE]]))
            beta_sb = singles.tile([P, D_NODE], F32)
            getattr(nc, CFG["xqq"]).dma_start(
                out=beta_sb[:],
                in_=bass.AP(tensor=gb[:].tensor, offset=D_NODE,
                            ap=[[0, P], [1, D_NODE]]))
            xqT_sb = singles.tile([65, NPC], BF16)
            getattr(nc, CFG["wq_q"]).dma_start(out=xqT_sb[:], in_=xqT[:])
            xq_sb = singles.tile([P, NT, D_NODE], F32)
            getattr(nc, CFG["xqq"]).dma_start(out=xq_sb[:], in_=xq[:])
            ident = singles.tile([P, P], BF16)
            make_identity(nc, ident[:])
            eps_sb = singles.tile([P, 1], F32)
            nc.vector.memset(eps_sb[:], LN_EPS)
            q_all = singles.tile([P, NT, D_NODE], BF16)
            yout_sb = singles.tile([P, NT, D_NODE], F32)
            mv_sb = singles.tile([P, NT, 2], F32)
            sd_sb = singles.tile([P, NT], F32)
            rsd_sb = singles.tile([P, NT], F32)
            mursd_sb = singles.tile([P, NT], F32)

            with (
                tc.tile_pool(name="xtp", bufs=CFG["xtb"]) as xtp,
                tc.tile_pool(name="kvp", bufs=CFG["kvb"], space="PSUM") as kvp,
                tc.tile_pool(name="ktp", bufs=CFG["ktb"]) as ktp,
                tc.tile_pool(name="vtp", bufs=CFG["ktb"]) as vtp,
                tc.tile_pool(name="eft", bufs=2) as eftp,
                tc.tile_pool(name="mid", bufs=CFG["midb"]) as midp,
                tc.tile_pool(name="sml", bufs=CFG["smb"]) as smlp,
                tc.tile_pool(name="pb", bufs=2, space="PSUM") as pb,
                tc.tile_pool(name="pt", bufs=1, space="PSUM") as ptp,
                tc.tile_pool(name="py", bufs=1, space="PSUM") as pyp,
            ):
                # pin the ln+exp+copy ACT table once; the auto-placement pass
                # would otherwise bounce between exp-only and ln tables
                nc.scalar.add_instruction(mybir.InstLoadActFuncSet(
                    name=nc.get_next_instruction_name(), ins=[], outs=[],
                    act_func_set_id=6))

                # ---- all Q' tiles upfront: 49 matmuls, 13 PSUM drains ----
                for g in range(-(-NT // 4)):
                    n4 = min(4, NT - g * 4)
                    qp = pyp.tile([P, 4, D_NODE], F32, name="qp", tag="yp")
                    for j in range(n4):
                        t = g * 4 + j
                        nc.tensor.matmul(
                            out=qp[:, j, :],
                            lhsT=xqT_sb[:, t * P:(t + 1) * P],
                            rhs=wq_sb[:], start=True, stop=True)
                    nc.vector.tensor_copy(out=q_all[:, g * 4:g * 4 + n4, :],
                                          in_=qp[:, 0:n4, :])

                coff = 0
                koff = 0
                goff = 0
                gi = 0
                gleft = 0
                eft_sb = None
                ep_done = 0
                for it, (t0, G, D) in enumerate(iters):
                    SL = G * D
                    KC = KC_i[it]
                    # ---- per-group source features, k-major slot order ----
                    xt_sb = xtp.tile([65, SL * P], BF16, name="xt_sb", tag="xt")
                    nc.sync.dma_start(
                        out=xt_sb[:], in_=xTc[:, coff * P:(coff + SL) * P])
                    if gleft == 0:
                        skc = gsz[gi]
                        eft_sb = eftp.tile([EF3, skc, P], BF16, tag="eft",
                                           name="eft_sb")
                        getattr(nc, CFG["eftq"]).dma_start(
                            out=eft_sb[:], in_=efT[:, goff * P:(goff + skc) * P])
                        gbase = goff
                        goff += skc
                        gi += 1
                        gleft = skc
                    kbase = koff - gbase

                    # ---- KV build: target node on partitions, no DRAM ----
                    # PSUM-direct tiles skip the K copy: the QK mul reads K
                    # straight from PSUM on DVE
    

                    CH = CFG["chunk"]
                    kvF = vtp.tile([P, 2 * D_NODE, SL], BF16, name="kvF",
                                   tag="vt")
                    for c8 in range(-(-SL // CH)):
                        j0 = c8 * CH
                        kk = min(CH, SL - j0)
                        pt = kvp.tile([P, CH, 2 * D_NODE], F32, name="pt",
                                      tag="kv")
                        for j in range(kk):
                            nc.tensor.matmul(
                                out=pt[:, j, :],
                                lhsT=xt_sb[:, (j0 + j) * P:(j0 + j + 1) * P],
                                rhs=wkv_sb[:], start=True, stop=True)
                        if CFG["vsplit"] and (it * 7 + c8) % CFG["vsplit"] == 0:
                            nc.vector.tensor_copy(
                                out=kvF[:, :, j0:j0 + kk],
                                in_=pt[:, 0:kk, :].rearrange("p j f -> p f j"))
                        else:
                            nc.scalar.copy(
                                out=kvF[:, :, j0:j0 + kk],
                                in_=pt[:, 0:kk, :].rearrange("p j f -> p f j"))

                    # ---- per-edge bias: 3 slots per matmul ----
                    biasp = pb.tile([P, 3 * KC, H], F32, tag="biasp",
                                    name="biasp")
                    for k in range(KC):
                        nc.tensor.matmul(out=biasp[:, 3 * k:3 * (k + 1), :],
                                         lhsT=eft_sb[:, kbase + k, :],
                                         rhs=we_sb[:], start=True, stop=True)

                    # ---- scores: QK mul then a w-fold chain (slots stay
                    # innermost-packed so every fold runs in DVE 2x mode) ----
                    qkp = midp.tile([P, D_NODE, SL], BF16, tag="qkp",
                                    name="qkp")
                    q_f = bass.AP(tensor=q_all[:].tensor,
                                  offset=q_all[:].offset + t0 * D_NODE,
                                  ap=[q_all[:].ap[0], [1, D_NODE], [0, SL]])
                    VE("qk").tensor_mul(out=qkp[:],
                                        in0=kvF[:, 0:D_NODE, :], in1=q_f)
                    qk4 = qkp[:].rearrange("p (h w) s -> p h w s", h=H)
                    sf1 = smlp.tile([P, H, 8, SL], BF16, tag="sf1", name="sf1")
                    VE("scf1").tensor_add(out=sf1[:], in0=qk4[:, :, 0:8, :],
                                          in1=qk4[:, :, 8:16, :])
                    sf2 = smlp.tile([P, H, 4, SL], BF16, tag="sf2", name="sf2")
                    VE("scf2").tensor_add(out=sf2[:], in0=sf1[:, :, 0:4, :],
                                          in1=sf1[:, :, 4:8, :])
                    sf3 = smlp.tile([P, H, 2, SL], BF16, tag="sf3", name="sf3")
                    nc.vector.tensor_add(out=sf3[:], in0=sf2[:, :, 0:2, :],
                                         in1=sf2[:, :, 2:4, :])
                    sc2 = smlp.tile([P, H, SL], F32, tag="sc2", name="sc2")
                    nc.vector.tensor_add(out=sc2[:], in0=sf3[:, :, 0, :],
                                         in1=sf3[:, :, 1, :])
                    bias_h = biasp[:, 0:SL, :].rearrange("p s h -> p h s")
                    nc.vector.tensor_add(out=sc2[:], in0=sc2[:], in1=bias_h)
                    ex = smlp.tile([P, H, SL], BF16, tag="ex", name="ex")
                    nc.scalar.activation(
                        out=ex[:], in_=sc2[:],
                        func=mybir.ActivationFunctionType.Exp)

                    den = smlp.tile([P, H, G], F32, tag="den", name="den")
                    nc.vector.tensor_reduce(
                        out=den[:],
                        in_=ex[:].rearrange("p h (g d) -> p h g d", g=G),
                        axis=mybir.AxisListType.X, op=mybir.AluOpType.add)
                    rden = smlp.tile([P, H, G], F32, tag="rden", name="rden")
                    nc.gpsimd.tensor_scalar_add(den[:], den[:], 1e-10)
                    nc.vector.reciprocal(out=rden[:], in_=den[:])

                    # ---- weighted V aggregation: mul, fold, reduce ----
                    exv = midp.tile([P, D_NODE, SL], BF16, tag="exv",
                                    name="exv")
                    ex_b = bass.AP(tensor=ex[:].tensor, offset=ex[:].offset,
                                   ap=[ex[:].ap[0], [SL, H], [0, D_H],
                                       [1, SL]])
                    VE("exv").tensor_mul(out=exv[:],
                                         in0=kvF[:, D_NODE:2 * D_NODE, :],
                                         in1=ex_b)
                    hD = D // 2
                    exv4 = exv[:].rearrange("p w (g d) -> p w g d", g=G)
                    uf1 = midp.tile([P, D_NODE, G, hD], BF16, tag="uf1",
                                    name="uf1")
                    VE("unnf1").tensor_add(out=uf1[:], in0=exv4[:, :, :, 0:hD],
                                           in1=exv4[:, :, :, hD:D])
                    unn = smlp.tile([P, D_NODE, G], F32, tag="unn", name="unn")
                    nc.vector.tensor_reduce(
                        out=unn[:], in_=uf1[:], axis=mybir.AxisListType.X,
                        op=mybir.AluOpType.add)
                    outn = smlp.tile([P, D_NODE, G], BF16, tag="outn",
                                     name="outn")
                    rden_b = bass.AP(tensor=rden[:].tensor,
                                     offset=rden[:].offset,
                                     ap=[rden[:].ap[0], [G, H], [0, D_H],
                                         [1, G]])
                    nc.gpsimd.tensor_mul(
                        out=outn[:].rearrange("p (h w) g -> p h w g", h=H),
                        in0=unn[:].rearrange("p (h w) g -> p h w g", h=H),
                        in1=rden_b)

                    # ---- projection: y = outn @ Wo.T + (x + bo) ----
                    tp = ptp.tile([D_NODE, G, P], BF16, tag="tp", name="tp")
                    for g in range(G):
                        nc.tensor.transpose(out=tp[:, g, :],
                                            in_=outn[:, :, g],
                                            identity=ident[:])
                    tps = smlp.tile([D_NODE, G, P], BF16, tag="tps",
                                    name="tps")
                    nc.vector.tensor_copy(out=tps[:], in_=tp[:])
                    yp = pyp.tile([P, G, D_NODE], F32, tag="yp", name="yp")
                    for g in range(G):
                        nc.tensor.matmul(out=yp[:, g, :], lhsT=tps[:, g, :],
                                         rhs=wo_sb[:], start=True, stop=True)
                    nc.vector.tensor_add(out=yout_sb[:, t0:t0 + G, :],
                                         in0=yp[:],
                                         in1=xq_sb[:, t0:t0 + G, :])
                    for g in range(G):
                        stats = smlp.tile([P, 6], F32, tag="stats",
                                          name="stats")
                        nc.vector.bn_stats(out=stats[:],
                                           in_=yout_sb[:, t0 + g, :])
                        nc.vector.bn_aggr(out=mv_sb[:, t0 + g, :],
                                          in_=stats[:])
                    coff += SL
                    koff += KC
                    gleft -= KC

                    # ---- chunked layernorm epilogue: overlap with loop ----
                    tend = t0 + G
                    ep_bounds = (12, 24, 36, NT) if CFG["chunked_ep"] else (NT,)
                    hit = [b for b in ep_bounds if ep_done < b <= tend]
                    if not debug_mode and hit:
                        te0, te1 = ep_done, tend
                        nch = te1 - te0
                        mvs = mv_sb[:, te0:te1, :]
                        mu = bass.AP(tensor=mvs.tensor, offset=mvs.offset,
                                     ap=[mvs.ap[0], [2, nch]])
                        var = bass.AP(tensor=mvs.tensor, offset=mvs.offset + 1,
                                      ap=[mvs.ap[0], [2, nch]])
                        # rsd = exp(-0.5*ln(var+eps)); ln+exp share one ACT
                        # function table (sqrt would force a table swap)
                        nc.scalar.activation(
                            out=sd_sb[:, te0:te1], in_=var,
                            func=mybir.ActivationFunctionType.Ln,
                            bias=eps_sb[:])
                        nc.scalar.activation(
                            out=rsd_sb[:, te0:te1], in_=sd_sb[:, te0:te1],
                            func=mybir.ActivationFunctionType.Exp,
                            scale=-0.5)
                        nc.vector.tensor_mul(out=mursd_sb[:, te0:te1], in0=mu,
                                             in1=rsd_sb[:, te0:te1])

                        def bc_t(a):   # [P, nch] -> [P, nch, 64]
                            return bass.AP(tensor=a.tensor, offset=a.offset,
                                           ap=list(a.ap) + [[0, D_NODE]])

                        def bc_f(a):   # [P, 64] -> [P, nch, 64]
                            return bass.AP(tensor=a.tensor, offset=a.offset,
                                           ap=[a.ap[0], [0, nch], a.ap[1]])

                        yv = yout_sb[:, te0:te1, :]
                        VE("ln_a").tensor_mul(out=yv, in0=yv,
                                              in1=bc_t(rsd_sb[:, te0:te1]))
                        VE("ln_b").tensor_sub(out=yv, in0=yv,
                                              in1=bc_t(mursd_sb[:, te0:te1]))
                        VE("ln_a").tensor_mul(out=yv, in0=yv,
                                              in1=bc_f(gamma_sb[:]))
                        VE("ln_b").tensor_add(out=yv, in0=yv,
                                              in1=bc_f(beta_sb[:]))
                        getattr(nc, CFG["xqq"]).dma_start(
                            out=y[:, te0 * D_NODE:te1 * D_NODE], in_=yv)
                        ep_done = te1

    nc.compile()
    return nc


# ------------------------------------------------------------------ driver --
def kernel(**inputs) -> np.ndarray:
    per_core, node_lists, meta = _host_prep(**inputs)
    nc = _build_kernel(meta)
    res = run_bass_kernel_spmd(nc, per_core, core_ids=list(range(NCORES)))
    y_full = np.zeros((N, D_NODE), dtype=np.float32)
    for c in range(NCORES):
        yc = res.results[c]["y"].reshape(P, NT, D_NODE).transpose(1, 0, 2)
        yc = yc.reshape(NPC, D_NODE)
        nl = node_lists[c]
        real = nl >= 0
        y_full[nl[real]] = yc[real]
    return y_full


# revision 36
# speedup vs baseline: 1.3040x; 1.0088x over previous
"""NodeAttention (GNN scatter-softmax attention) on 8 Trainium2 NeuronCores.

Strategy (v5 — on-chip KV, paired tiles):
- Host deals nodes to 8 cores round-robin by degree rank, so every core sees a
  near-identical degree profile; one static NEFF serves all cores (SPMD).
- Per core: 49 node-tiles x 128 nodes; node-tile t gets a dense padded slot
  grid [128, D] (D = max degree in the PAIR of tiles across cores; tiles are
  processed two-at-a-time so fixed per-instruction costs amortize over 2x the
  slots; adjacent degree-sorted tiles have near-identical D so pairing adds
  only ~2% padding).
- The host replicates x per SLOT in k-major order, so each KV projection
  matmul's PSUM output lands with the TARGET node on partitions. K is copied
  to SBUF as [P, slots, 64] and V as [P, 64, slots] (feature-major) straight
  from PSUM — no DRAM staging, no gather.
- Per-edge scores: bf16 QK mul in DVE 2x mode, fold-fold-reduce for the
  per-head dot; per-edge bias via block-diagonal matmuls (3 slots x 34
  ef-features on 102 partitions); exp on ACT; softmax normalization AFTER
  aggregation. Feature-major V lets the attn broadcast ride a middle dim so
  the weighted-V mul is also 2x, with a fold+reduce over slots.
- Big muls and one fold run on the otherwise-idle GpSimd (Pool) engine
  (SBUF-only ops; GPSIMD cannot touch PSUM). PSUM->SBUF copies + exp on ACT.
- LayerNorm runs chunked (4 output chunks) so the epilogue and y write-back
  overlap the main loop.
- No max-subtraction in softmax (scores are O(10); identical result).
  Padding slots masked via an extra edge-feature column (weight 1, value -75).
- temp/sqrt(d) folded into Wq; temp folded into We; be via a ones column;
  bo folded into the residual x shipped from host.
"""

import os
import json
import numpy as np
import ml_dtypes

import concourse.bass as bass
import concourse.bacc as bacc
import concourse.tile as tile
from concourse import mybir
from concourse.bass_utils import run_bass_kernel_spmd
from concourse.masks import make_identity

N, E = 50000, 800000
D_NODE, D_EDGE, H = 64, 32, 4
D_H = D_NODE // H
LN_EPS = 1e-5
NCORES = 8
P = 128
NT = 49                # node tiles per core
NPC = NT * P           # padded nodes per core = 6272
GRP = int(os.environ.get('KERNEL_GRP', '1'))   # tiles per iteration
EF_R = 34              # 32 ef features + mask col + ones col (carries be)
EF3 = 3 * EF_R         # 102: three slots stacked on partitions
MASK_VAL = -75.0
F32 = mybir.dt.float32
BF16 = mybir.dt.bfloat16
BF_NP = ml_dtypes.bfloat16


# ---------------------------------------------------------------- host prep --
def _host_prep(node_features, edge_features, edge_index, Wq, bq, Wk, bk, Wv, bv,
               We, be, Wo, bo, ln_gamma, ln_beta, log_temp):
    x = np.ascontiguousarray(np.asarray(node_features, dtype=np.float32))
    ef = np.ascontiguousarray(np.asarray(edge_features, dtype=np.float32))
    src = np.asarray(edge_index[0], dtype=np.int64)
    tgt = np.asarray(edge_index[1], dtype=np.int64)
    temp = np.exp(np.asarray(log_temp, dtype=np.float32))

    deg = np.bincount(tgt, minlength=N)
    order = np.argsort(-deg, kind="stable")
    node_lists = []
    for c in range(NCORES):
        nl = order[c::NCORES]
        nl = np.concatenate([nl, np.full(NPC - len(nl), -1, dtype=np.int64)])
        node_lists.append(nl)

    D_t = np.zeros(NT, dtype=np.int64)
    for c in range(NCORES):
        d = np.where(node_lists[c] >= 0, deg[np.maximum(node_lists[c], 0)], 0)
        D_t = np.maximum(D_t, d.reshape(NT, P).max(axis=1))
    D_t = np.maximum(D_t, 2)
    D_t = D_t + (D_t % 2)          # even D so the aggregation fold halves cleanly
    # iterations: pairs of adjacent tiles sharing one D (last tile solo)
    iters = []                     # (t0, G, D)
    t = 0
    while t < NT:
        G = min(GRP, NT - t)
        D = int(D_t[t:t + G].max())
        D_t[t:t + G] = D
        iters.append((t, G, D))
        t += G
    assert D_t.max() <= 128, f"degree {D_t.max()} exceeds single-bank design"
    SD = int(D_t.sum())
    KC_i = [-(-(G * D) // 3) for (_, G, D) in iters]
    SKC = sum(KC_i)

    eorder = np.argsort(tgt, kind="stable")
    estart = np.zeros(N + 1, dtype=np.int64)
    np.cumsum(deg, out=estart[1:])

    qscale = (np.repeat(temp, D_H) / np.sqrt(D_H)).astype(np.float32)
    Wq_aug = (np.concatenate([np.asarray(Wq).T, np.asarray(bq)[None, :]], 0)
              * qscale[None, :]).astype(BF_NP)                           # [65,64]
    Wkv_aug = np.concatenate(
        [np.concatenate([np.asarray(Wk).T, np.asarray(Wv).T], 1),
         np.concatenate([np.asarray(bk), np.asarray(bv)])[None, :]], 0
    ).astype(BF_NP)                                                      # [65,128]
    We_augT = np.concatenate(
        [np.asarray(We).T * temp[None, :],
         np.ones((1, H), np.float32),
         (np.asarray(be) * temp)[None, :]], 0
    ).astype(np.float32)                                                 # [34,4]
    We_blk = np.zeros((EF3, 3 * H), dtype=np.float32)
    for j3 in range(3):
        We_blk[j3 * EF_R:(j3 + 1) * EF_R, j3 * H:(j3 + 1) * H] = We_augT
    We_blk = We_blk.astype(BF_NP)
    Wo_T = np.ascontiguousarray(np.asarray(Wo).T).astype(BF_NP)          # [64,64]
    gb = np.stack([np.asarray(ln_gamma), np.asarray(ln_beta)]).astype(np.float32)

    x_aug = np.concatenate(
        [x, np.ones((N, 1), np.float32)], 1).astype(BF_NP)               # [N,65]

    per_core = []
    for c in range(NCORES):
        nl = node_lists[c]
        efT = np.zeros((EF3, SKC * P), dtype=BF_NP)
        xTc = np.zeros((65, SD * P), dtype=BF_NP)
        coff = 0
        koff = 0
        for it, (t0, G, D) in enumerate(iters):
            SL = G * D
            KC = KC_i[it]
            # gather edge ids for each tile in the group: slot j = g*D + d
            nlt = nl[t0 * P:(t0 + G) * P].reshape(G, P)          # [G,P]
            degt = np.where(nlt >= 0, deg[np.maximum(nlt, 0)], 0)
            k = np.arange(D)
            valid = k[None, None, :] < degt[:, :, None]          # [G,P,D]
            pos = estart[np.maximum(nlt, 0)][:, :, None] + k[None, None, :]
            eids = eorder[np.minimum(pos, E - 1)]
            eids = np.where(valid, eids, 0)
            gsrc = np.where(valid, src[eids], -1)                # [G,P,D]
            # slot (p, j=g*D+d) lives at xTc column (coff + j)*128 + p
            cols = ((coff + np.arange(SL).reshape(G, 1, D)) * P
                    + np.arange(P)[None, :, None])               # [G,P,D]
            xv = np.where(valid.reshape(-1)[:, None],
                          x_aug[np.maximum(gsrc.reshape(-1), 0)],
                          0).astype(BF_NP)
            xTc[:, cols.reshape(-1)] = xv.T
            # edge-feature bias blocks over the group's SL slots
            blk = np.zeros((P, KC * 3, EF_R), dtype=np.float32)
            blk[:, :, D_EDGE] = MASK_VAL
            efv = np.where(valid[:, :, :, None], ef[eids], 0.0)  # [G,P,D,Ef]
            efv = efv.transpose(1, 0, 2, 3).reshape(P, SL, D_EDGE)
            vmask = valid.transpose(1, 0, 2).reshape(P, SL)
            blk[:, :SL, :D_EDGE] = efv
            blk[:, :SL, D_EDGE] = np.where(vmask, 0.0, MASK_VAL)
            blk[:, :, D_EDGE + 1] = 1.0
            # [P, KC, 3, EF_R] -> [3, EF_R, KC, P] -> [102, KC*128]
            efT[:, koff * P:(koff + KC) * P] = (
                blk.reshape(P, KC, 3, EF_R).transpose(2, 3, 1, 0)
                .reshape(EF3, KC * P).astype(BF_NP))
            coff += SL
            koff += KC
        xq = np.where(nl[:, None] >= 0, x[np.maximum(nl, 0)], 0.0).astype(np.float32)
        xqT_aug = np.concatenate([xq.T, np.ones((1, NPC), np.float32)],
                                 0).astype(BF_NP)
        xqr = (xq + np.asarray(bo, dtype=np.float32)[None, :])
        xq_g = np.ascontiguousarray(
            xqr.reshape(NT, P, D_NODE).transpose(1, 0, 2).reshape(P, NT * D_NODE))
        per_core.append({
            "efT": efT,
            "xTc": xTc,
            "xqT": np.ascontiguousarray(xqT_aug),
            "xq": xq_g,
            "wq": Wq_aug,
            "wkv": np.ascontiguousarray(Wkv_aug),
            "we": np.ascontiguousarray(We_blk),
            "wo": Wo_T,
            "gb": gb,
        })
    meta = dict(iters=iters)
    return per_core, node_lists, meta


# ------------------------------------------------------------- bass kernel --
def _build_kernel(meta, debug_mode=None):
    if debug_mode is None:
        debug_mode = os.environ.get("KERNEL_DEBUG_MODE", "")
    iters = meta["iters"]
    KC_i = [-(-(G * D) // 3) for (_, G, D) in iters]
    SD = sum(G * D for (_, G, D) in iters)
    SKC = sum(KC_i)
    # eft groups: ~7 DMAs over the run, aligned to iteration KC blocks
    NG = 7
    tgt_sz = -(-SKC // NG)
    gsz = []
    acc = 0
    for kc in KC_i:
        if acc + kc > tgt_sz and acc > 0:
            gsz.append(acc)
            acc = 0
        acc += kc
    gsz.append(acc)
    nc = bacc.Bacc(None, target_bir_lowering=False)

    # engine assignment knobs for the fungible ops ("dve" | "pool")
    ENG = dict(qk="pool", exv="pool", scf1="dve", scf2="dve",
               unnf1="pool", ln_a="pool", ln_b="dve")
    ENG.update(json.loads(os.environ.get("KERNEL_ENG", "{}")))
    CFG = dict(vsplit=0, chunked_ep=1, eftq="sync", wq_q="scalar",
               xqq="gpsimd", xtb=3, smb=4, midb=4, ktb=4, ksplit=0, chunk=8,
               kvb=2, xti=1)
    CFG.update(json.loads(os.environ.get("KERNEL_CFG", "{}")))

    def VE(key):
        return nc.gpsimd if ENG[key] == "pool" else nc.vector

    efT = nc.dram_tensor("efT", [EF3, SKC * P], BF16, kind="ExternalInput")
    xTc = nc.dram_tensor("xTc", [65, SD * P], BF16, kind="ExternalInput")
    xqT = nc.dram_tensor("xqT", [65, NPC], BF16, kind="ExternalInput")
    xq = nc.dram_tensor("xq", [P, NT * D_NODE], F32, kind="ExternalInput")
    wq = nc.dram_tensor("wq", [65, D_NODE], BF16, kind="ExternalInput")
    wkv = nc.dram_tensor("wkv", [65, 2 * D_NODE], BF16, kind="ExternalInput")
    we = nc.dram_tensor("we", [EF3, 3 * H], BF16, kind="ExternalInput")
    wo = nc.dram_tensor("wo", [D_NODE, D_NODE], BF16, kind="ExternalInput")
    gb = nc.dram_tensor("gb", [2, D_NODE], F32, kind="ExternalInput")
    y = nc.dram_tensor("y", [P, NT * D_NODE], F32, kind="ExternalOutput")

    with tile.TileContext(nc) as tc:
        with (
            tc.tile_pool(name="singles", bufs=1) as singles,
        ):
            wq_sb = singles.tile([65, D_NODE], BF16)
            getattr(nc, CFG["wq_q"]).dma_start(out=wq_sb[:], in_=wq[:])
            wkv_sb = singles.tile([65, 2 * D_NODE], BF16)
            getattr(nc, CFG["wq_q"]).dma_start(out=wkv_sb[:], in_=wkv[:])
            we_sb = singles.tile([EF3, 3 * H], BF16)
            getattr(nc, CFG["wq_q"]).dma_start(out=we_sb[:], in_=we[:])
            wo_sb = singles.tile([D_NODE, D_NODE], BF16)
            getattr(nc, CFG["wq_q"]).dma_start(out=wo_sb[:], in_=wo[:])
            gamma_sb = singles.tile([P, D_NODE], F32)
            getattr(nc, CFG["xqq"]).dma_start(
                out=gamma_sb[:],
                in_=bass.AP(tensor=gb[:].tensor, offset=0,
                            ap=[[0, P], [1, D_NODE]]))
            beta_sb = singles.tile([P, D_NODE], F32)
            getattr(nc, CFG["xqq"]).dma_start(
                out=beta_sb[:],
                in_=bass.AP(tensor=gb[:].tensor, offset=D_NODE,
                            ap=[[0, P], [1, D_NODE]]))
            xqT_sb = singles.tile([65, NPC], BF16)
            for qc in range(4):
                c0 = (NPC // 4) * qc
                c1 = (NPC // 4) * (qc + 1) if qc < 3 else NPC
                getattr(nc, CFG["wq_q"]).dma_start(
                    out=xqT_sb[:, c0:c1], in_=xqT[:, c0:c1])
            xq_sb = singles.tile([P, NT, D_NODE], F32)
            getattr(nc, CFG["xqq"]).dma_start(out=xq_sb[:], in_=xq[:])
            ident = singles.tile([P, P], BF16)
            make_identity(nc, ident[:])
            eps_sb = singles.tile([P, 1], F32)
            nc.vector.memset(eps_sb[:], LN_EPS)
            q_all = singles.tile([P, NT, D_NODE], BF16)
            yout_sb = singles.tile([P, NT, D_NODE], F32)
            mv_sb = singles.tile([P, NT, 2], F32)
            sd_sb = singles.tile([P, NT], F32)
            rsd_sb = singles.tile([P, NT], F32)
            mursd_sb = singles.tile([P, NT], F32)

            with (
                tc.tile_pool(name="xtp", bufs=CFG["xtb"]) as xtp,
                tc.tile_pool(name="kvp", bufs=CFG["kvb"], space="PSUM") as kvp,
                tc.tile_pool(name="ktp", bufs=CFG["ktb"]) as ktp,
                tc.tile_pool(name="vtp", bufs=CFG["ktb"]) as vtp,
                tc.tile_pool(name="eft", bufs=2) as eftp,
                tc.tile_pool(name="mid", bufs=CFG["midb"]) as midp,
                tc.tile_pool(name="sml", bufs=CFG["smb"]) as smlp,
                tc.tile_pool(name="pb", bufs=2, space="PSUM") as pb,
                tc.tile_pool(name="pt", bufs=1, space="PSUM") as ptp,
                tc.tile_pool(name="py", bufs=1, space="PSUM") as pyp,
            ):
                # pin the ln+exp+copy ACT table once; the auto-placement pass
                # would otherwise bounce between exp-only and ln tables
                nc.scalar.add_instruction(mybir.InstLoadActFuncSet(
                    name=nc.get_next_instruction_name(), ins=[], outs=[],
                    act_func_set_id=6))

                # ---- all Q' tiles upfront: 49 matmuls, 13 PSUM drains ----
                for g in range(-(-NT // 4)):
                    n4 = min(4, NT - g * 4)
                    qp = pyp.tile([P, 4, D_NODE], F32, name="qp", tag="yp")
                    for j in range(n4):
                        t = g * 4 + j
                        nc.tensor.matmul(
                            out=qp[:, j, :],
                            lhsT=xqT_sb[:, t * P:(t + 1) * P],
                            rhs=wq_sb[:], start=True, stop=True)
                    nc.vector.tensor_copy(out=q_all[:, g * 4:g * 4 + n4, :],
                                          in_=qp[:, 0:n4, :])

                coff = [0]
                koff = [0]
                goff = [0]
                gi = [0]
                gleft = [0]
                gbase = [0]
                eft_sb = [None]
                ep_done = 0
                xt_sb = [None, 0, 0]     # tile, slot offset, iters left
                built = {}               # it -> (kvF, biasp)

                def build_kv(it):
                    t0, G, D = iters[it]
                    SL = G * D
                    KC = KC_i[it]
                    # source features, k-major slots; one DMA covers XTI iters
                    if xt_sb[0] is None:
                        nsl = sum(g * d for (_, g, d)
                                  in iters[it:it + CFG["xti"]])
                        xt_sb[0] = xtp.tile([65, nsl * P], BF16, name="xt_sb",
                                            tag="xt")
                        nc.sync.dma_start(
                            out=xt_sb[0][:],
                            in_=xTc[:, coff[0] * P:(coff[0] + nsl) * P])
                        xt_sb[1] = 0
                        xt_sb[2] = CFG["xti"]
                    if gleft[0] == 0:
                        skc = gsz[gi[0]]
                        eft_sb[0] = eftp.tile([EF3, skc, P], BF16, tag="eft",
                                              name="eft_sb")
                        getattr(nc, CFG["eftq"]).dma_start(
                            out=eft_sb[0][:],
                            in_=efT[:, goff[0] * P:(goff[0] + skc) * P])
                        gi[0] += 1
                        gleft[0] = skc
                        gbase[0] = goff[0]
                        goff[0] += skc
                    # position of this iter's KC blocks within the group
                    kbase = koff[0] - gbase[0]

                    # KV build: target node on partitions, no DRAM staging
                    CH = CFG["chunk"]
                    kvF = vtp.tile([P, 2 * D_NODE, SL], BF16, name="kvF",
                                   tag="vt")
                    xo = xt_sb[1]
                    for c8 in range(-(-SL // CH)):
                        j0 = c8 * CH
                        kk = min(CH, SL - j0)
                        pt = kvp.tile([P, CH, 2 * D_NODE], F32, name="pt",
                                      tag="kv")
                        for j in range(kk):
                            nc.tensor.matmul(
                                out=pt[:, j, :],
                                lhsT=xt_sb[0][:, (xo + j0 + j) * P:
                                              (xo + j0 + j + 1) * P],
                                rhs=wkv_sb[:], start=True, stop=True)
                        if CFG["vsplit"] and (it * 7 + c8) % CFG["vsplit"] == 0:
                            nc.vector.tensor_copy(
                                out=kvF[:, :, j0:j0 + kk],
                                in_=pt[:, 0:kk, :].rearrange("p j f -> p f j"))
                        else:
                            nc.scalar.copy(
                                out=kvF[:, :, j0:j0 + kk],
                                in_=pt[:, 0:kk, :].rearrange("p j f -> p f j"))

                    # per-edge bias: 3 slots per matmul
                    biasp = pb.tile([P, 3 * KC, H], F32, tag="biasp",
                                    name="biasp")
                    for k in range(KC):
                        nc.tensor.matmul(out=biasp[:, 3 * k:3 * (k + 1), :],
                                         lhsT=eft_sb[0][:, kbase + k, :],
                                         rhs=we_sb[:], start=True, stop=True)
                    built[it] = (kvF, biasp)
                    coff[0] += SL
                    koff[0] += KC
                    gleft[0] -= KC
                    xt_sb[1] += SL
                    xt_sb[2] -= 1
                    if xt_sb[2] == 0:
                        xt_sb[0] = None

                build_kv(0)
                for it, (t0, G, D) in enumerate(iters):
                    SL = G * D
                    KC = KC_i[it]
                    if it + 1 < len(iters):
                        build_kv(it + 1)
                    kvF, biasp = built.pop(it)

                    # ---- scores: QK mul then a w-fold chain (slots stay
                    # innermost-packed so every fold runs in DVE 2x mode) ----
                    qkp = midp.tile([P, D_NODE, SL], BF16, tag="qkp",
                                    name="qkp")
                    q_f = bass.AP(tensor=q_all[:].tensor,
                                  offset=q_all[:].offset + t0 * D_NODE,
                                  ap=[q_all[:].ap[0], [1, D_NODE], [0, SL]])
                    VE("qk").tensor_mul(out=qkp[:],
                                        in0=kvF[:, 0:D_NODE, :], in1=q_f)
                    qk4 = qkp[:].rearrange("p (h w) s -> p h w s", h=H)
                    sf1 = smlp.tile([P, H, 8, SL], BF16, tag="sf1", name="sf1")
                    VE("scf1").tensor_add(out=sf1[:], in0=qk4[:, :, 0:8, :],
                                          in1=qk4[:, :, 8:16, :])
                    sf2 = smlp.tile([P, H, 4, SL], BF16, tag="sf2", name="sf2")
                    VE("scf2").tensor_add(out=sf2[:], in0=sf1[:, :, 0:4, :],
                                          in1=sf1[:, :, 4:8, :])
                    sf3 = smlp.tile([P, H, 2, SL], BF16, tag="sf3", name="sf3")
                    nc.vector.tensor_add(out=sf3[:], in0=sf2[:, :, 0:2, :],
                                         in1=sf2[:, :, 2:4, :])
                    sc2 = smlp.tile([P, H, SL], F32, tag="sc2", name="sc2")
                    nc.vector.tensor_add(out=sc2[:], in0=sf3[:, :, 0, :],
                                         in1=sf3[:, :, 1, :])
                    bias_h = biasp[:, 0:SL, :].rearrange("p s h -> p h s")
                    nc.vector.tensor_add(out=sc2[:], in0=sc2[:], in1=bias_h)
                    ex = smlp.tile([P, H, SL], BF16, tag="ex", name="ex")
                    nc.scalar.activation(
                        out=ex[:], in_=sc2[:],
                        func=mybir.ActivationFunctionType.Exp)

                    den = smlp.tile([P, H, G], F32, tag="den", name="den")
                    nc.vector.tensor_reduce(
                        out=den[:],
                        in_=ex[:].rearrange("p h (g d) -> p h g d", g=G),
                        axis=mybir.AxisListType.X, op=mybir.AluOpType.add)
                    rden = smlp.tile([P, H, G], F32, tag="rden", name="rden")
                    nc.gpsimd.tensor_scalar_add(den[:], den[:], 1e-10)
                    nc.vector.reciprocal(out=rden[:], in_=den[:])

                    # ---- weighted V aggregation: mul, fold, reduce ----
                    exv = midp.tile([P, D_NODE, SL], BF16, tag="exv",
                                    name="exv")
                    ex_b = bass.AP(tensor=ex[:].tensor, offset=ex[:].offset,
                                   ap=[ex[:].ap[0], [SL, H], [0, D_H],
                                       [1, SL]])
                    VE("exv").tensor_mul(out=exv[:],
                                         in0=kvF[:, D_NODE:2 * D_NODE, :],
                                         in1=ex_b)
                    hD = D // 2
                    exv4 = exv[:].rearrange("p w (g d) -> p w g d", g=G)
                    uf1 = midp.tile([P, D_NODE, G, hD], BF16, tag="uf1",
                                    name="uf1")
                    VE("unnf1").tensor_add(out=uf1[:], in0=exv4[:, :, :, 0:hD],
                                           in1=exv4[:, :, :, hD:D])
                    unn = smlp.tile([P, D_NODE, G], F32, tag="unn", name="unn")
                    nc.vector.tensor_reduce(
                        out=unn[:], in_=uf1[:], axis=mybir.AxisListType.X,
                        op=mybir.AluOpType.add)
                    outn = smlp.tile([P, D_NODE, G], BF16, tag="outn",
                                     name="outn")
                    rden_b = bass.AP(tensor=rden[:].tensor,
                                     offset=rden[:].offset,
                                     ap=[rden[:].ap[0], [G, H], [0, D_H],
                                         [1, G]])
                    nc.gpsimd.tensor_mul(
                        out=outn[:].rearrange("p (h w) g -> p h w g", h=H),
                        in0=unn[:].rearrange("p (h w) g -> p h w g", h=H),
                        in1=rden_b)

                    # ---- projection: y = outn @ Wo.T + (x + bo) ----
                    tp = ptp.tile([D_NODE, G, P], BF16, tag="tp", name="tp")
                    for g in range(G):
                        nc.tensor.transpose(out=tp[:, g, :],
                                            in_=outn[:, :, g],
                                            identity=ident[:])
                    tps = smlp.tile([D_NODE, G, P], BF16, tag="tps",
                                    name="tps")
                    nc.vector.tensor_copy(out=tps[:], in_=tp[:])
                    yp = pyp.tile([P, G, D_NODE], F32, tag="yp", name="yp")
                    for g in range(G):
                        nc.tensor.matmul(out=yp[:, g, :], lhsT=tps[:, g, :],
                                         rhs=wo_sb[:], start=True, stop=True)
                    nc.vector.tensor_add(out=yout_sb[:, t0:t0 + G, :],
                                         in0=yp[:],
                                         in1=xq_sb[:, t0:t0 + G, :])
                    for g in range(G):
                        stats = smlp.tile([P, 6], F32, tag="stats",
                                          name="stats")
                        nc.vector.bn_stats(out=stats[:],
                                           in_=yout_sb[:, t0 + g, :])
                        nc.vector.bn_aggr(out=mv_sb[:, t0 + g, :],
                                          in_=stats[:])

                    # ---- chunked layernorm epilogue: overlap with loop ----
                    tend = t0 + G
                    ep_bounds = (12, 24, 36, NT) if CFG["chunked_ep"] else (NT,)
                    hit = [b for b in ep_bounds if ep_done < b <= tend]
                    if not debug_mode and hit:
                        te0, te1 = ep_done, tend
                        nch = te1 - te0
                        mvs = mv_sb[:, te0:te1, :]
                        mu = bass.AP(tensor=mvs.tensor, offset=mvs.offset,
                                     ap=[mvs.ap[0], [2, nch]])
                        var = bass.AP(tensor=mvs.tensor, offset=mvs.offset + 1,
                                      ap=[mvs.ap[0], [2, nch]])
                        # rsd = exp(-0.5*ln(var+eps)); ln+exp share one ACT
                        # function table (sqrt would force a table swap)
                        nc.scalar.activation(
                            out=sd_sb[:, te0:te1], in_=var,
                            func=mybir.ActivationFunctionType.Ln,
                            bias=eps_sb[:])
                        nc.scalar.activation(
                            out=rsd_sb[:, te0:te1], in_=sd_sb[:, te0:te1],
                            func=mybir.ActivationFunctionType.Exp,
                            scale=-0.5)
                        nc.vector.tensor_mul(out=mursd_sb[:, te0:te1], in0=mu,
                                             in1=rsd_sb[:, te0:te1])

                        def bc_t(a):   # [P, nch] -> [P, nch, 64]
                            return bass.AP(tensor=a.tensor, offset=a.offset,
                                           ap=list(a.ap) + [[0, D_NODE]])

                        def bc_f(a):   # [P, 64] -> [P, nch, 64]
                            return bass.AP(tensor=a.tensor, offset=a.offset,
                                           ap=[a.ap[0], [0, nch], a.ap[1]])

                        yv = yout_sb[:, te0:te1, :]
                        VE("ln_a").tensor_mul(out=yv, in0=yv,
                                              in1=bc_t(rsd_sb[:, te0:te1]))
                        VE("ln_b").tensor_sub(out=yv, in0=yv,
                                              in1=bc_t(mursd_sb[:, te0:te1]))
                        VE("ln_a").tensor_mul(out=yv, in0=yv,
                                              in1=bc_f(gamma_sb[:]))
                        VE("ln_b").tensor_add(out=yv, in0=yv,
                                              in1=bc_f(beta_sb[:]))
                        getattr(nc, CFG["xqq"]).dma_start(
                            out=y[:, te0 * D_NODE:te1 * D_NODE], in_=yv)
                        ep_done = te1

    nc.compile()
    return nc


# ------------------------------------------------------------------ driver --
def kernel(**inputs) -> np.ndarray:
    per_core, node_lists, meta = _host_prep(**inputs)
    nc = _build_kernel(meta)
    res = run_bass_kernel_spmd(nc, per_core, core_ids=list(range(NCORES)))
    y_full = np.zeros((N, D_NODE), dtype=np.float32)
    for c in range(NCORES):
        yc = res.results[c]["y"].reshape(P, NT, D_NODE).transpose(1, 0, 2)
        yc = yc.reshape(NPC, D_NODE)
        nl = node_lists[c]
        real = nl >= 0
        y_full[nl[real]] = yc[real]
    return y_full


# revision 40
# speedup vs baseline: 1.3197x; 1.0120x over previous
"""NodeAttention (GNN scatter-softmax attention) on 8 Trainium2 NeuronCores.

Strategy (v5 — on-chip KV, paired tiles):
- Host deals nodes to 8 cores round-robin by degree rank, so every core sees a
  near-identical degree profile; one static NEFF serves all cores (SPMD).
- Per core: 49 node-tiles x 128 nodes; node-tile t gets a dense padded slot
  grid [128, D] (D = max degree in the PAIR of tiles across cores; tiles are
  processed two-at-a-time so fixed per-instruction costs amortize over 2x the
  slots; adjacent degree-sorted tiles have near-identical D so pairing adds
  only ~2% padding).
- The host replicates x per SLOT in k-major order, so each KV projection
  matmul's PSUM output lands with the TARGET node on partitions. K is copied
  to SBUF as [P, slots, 64] and V as [P, 64, slots] (feature-major) straight
  from PSUM — no DRAM staging, no gather.
- Per-edge scores: bf16 QK mul in DVE 2x mode, fold-fold-reduce for the
  per-head dot; per-edge bias via block-diagonal matmuls (3 slots x 34
  ef-features on 102 partitions); exp on ACT; softmax normalization AFTER
  aggregation. Feature-major V lets the attn broadcast ride a middle dim so
  the weighted-V mul is also 2x, with a fold+reduce over slots.
- Big muls and one fold run on the otherwise-idle GpSimd (Pool) engine
  (SBUF-only ops; GPSIMD cannot touch PSUM). PSUM->SBUF copies + exp on ACT.
- LayerNorm runs chunked (4 output chunks) so the epilogue and y write-back
  overlap the main loop.
- No max-subtraction in softmax (scores are O(10); identical result).
  Padding slots masked via an extra edge-feature column (weight 1, value -75).
- temp/sqrt(d) folded into Wq; temp folded into We; be via a ones column;
  bo folded into the residual x shipped from host.
"""

import os
import json
import numpy as np
import ml_dtypes

import concourse.bass as bass
import concourse.bacc as bacc
import concourse.tile as tile
from concourse import mybir
from concourse.bass_utils import run_bass_kernel_spmd
from concourse.masks import make_identity

N, E = 50000, 800000
D_NODE, D_EDGE, H = 64, 32, 4
D_H = D_NODE // H
LN_EPS = 1e-5
NCORES = 8
P = 128
NT = 49                # node tiles per core
NPC = NT * P           # padded nodes per core = 6272
GRP = int(os.environ.get('KERNEL_GRP', '1'))   # tiles per iteration
EF_R = 34              # 32 ef features + mask col + ones col (carries be)
EF3 = 3 * EF_R         # 102: three slots stacked on partitions
MASK_VAL = -75.0
F32 = mybir.dt.float32
BF16 = mybir.dt.bfloat16
BF_NP = ml_dtypes.bfloat16


# ---------------------------------------------------------------- host prep --
def _host_prep(node_features, edge_features, edge_index, Wq, bq, Wk, bk, Wv, bv,
               We, be, Wo, bo, ln_gamma, ln_beta, log_temp):
    x = np.ascontiguousarray(np.asarray(node_features, dtype=np.float32))
    ef = np.ascontiguousarray(np.asarray(edge_features, dtype=np.float32))
    src = np.asarray(edge_index[0], dtype=np.int64)
    tgt = np.asarray(edge_index[1], dtype=np.int64)
    temp = np.exp(np.asarray(log_temp, dtype=np.float32))

    deg = np.bincount(tgt, minlength=N)
    order = np.argsort(-deg, kind="stable")
    node_lists = []
    for c in range(NCORES):
        nl = order[c::NCORES]
        nl = np.concatenate([nl, np.full(NPC - len(nl), -1, dtype=np.int64)])
        node_lists.append(nl)

    D_t = np.zeros(NT, dtype=np.int64)
    for c in range(NCORES):
        d = np.where(node_lists[c] >= 0, deg[np.maximum(node_lists[c], 0)], 0)
        D_t = np.maximum(D_t, d.reshape(NT, P).max(axis=1))
    D_t = np.maximum(D_t, 2)
    D_t = D_t + (D_t % 2)          # even D so the aggregation fold halves cleanly
    # iterations: pairs of adjacent tiles sharing one D (last tile solo)
    iters = []                     # (t0, G, D)
    t = 0
    while t < NT:
        G = min(GRP, NT - t)
        D = int(D_t[t:t + G].max())
        D_t[t:t + G] = D
        iters.append((t, G, D))
        t += G
    if os.environ.get("KERNEL_REV", "0") == "1":
        iters = iters[::-1]
    assert D_t.max() <= 128, f"degree {D_t.max()} exceeds single-bank design"
    SD = int(D_t.sum())
    KC_i = [-(-(G * D) // 3) for (_, G, D) in iters]
    SKC = sum(KC_i)

    eorder = np.argsort(tgt, kind="stable")
    estart = np.zeros(N + 1, dtype=np.int64)
    np.cumsum(deg, out=estart[1:])

    qscale = (np.repeat(temp, D_H) / np.sqrt(D_H)).astype(np.float32)
    Wq_aug = (np.concatenate([np.asarray(Wq).T, np.asarray(bq)[None, :]], 0)
              * qscale[None, :]).astype(BF_NP)                           # [65,64]
    Wkv_aug = np.concatenate(
        [np.concatenate([np.asarray(Wk).T, np.asarray(Wv).T], 1),
         np.concatenate([np.asarray(bk), np.asarray(bv)])[None, :]], 0
    ).astype(BF_NP)                                                      # [65,128]
    We_augT = np.concatenate(
        [np.asarray(We).T * temp[None, :],
         np.ones((1, H), np.float32),
         (np.asarray(be) * temp)[None, :]], 0
    ).astype(np.float32)                                                 # [34,4]
    We_blk = np.zeros((EF3, 3 * H), dtype=np.float32)
    for j3 in range(3):
        We_blk[j3 * EF_R:(j3 + 1) * EF_R, j3 * H:(j3 + 1) * H] = We_augT
    We_blk = We_blk.astype(BF_NP)
    Wo_T = np.ascontiguousarray(np.asarray(Wo).T).astype(BF_NP)          # [64,64]
    gb = np.stack([np.asarray(ln_gamma), np.asarray(ln_beta)]).astype(np.float32)

    x_aug = np.concatenate(
        [x, np.ones((N, 1), np.float32)], 1).astype(BF_NP)               # [N,65]

    per_core = []
    for c in range(NCORES):
        nl = node_lists[c]
        efT = np.zeros((EF3, SKC * P), dtype=BF_NP)
        xTc = np.zeros((65, SD * P), dtype=BF_NP)
        coff = 0
        koff = 0
        for it, (t0, G, D) in enumerate(iters):
            SL = G * D
            KC = KC_i[it]
            # gather edge ids for each tile in the group: slot j = g*D + d
            nlt = nl[t0 * P:(t0 + G) * P].reshape(G, P)          # [G,P]
            degt = np.where(nlt >= 0, deg[np.maximum(nlt, 0)], 0)
            k = np.arange(D)
            valid = k[None, None, :] < degt[:, :, None]          # [G,P,D]
            pos = estart[np.maximum(nlt, 0)][:, :, None] + k[None, None, :]
            eids = eorder[np.minimum(pos, E - 1)]
            eids = np.where(valid, eids, 0)
            gsrc = np.where(valid, src[eids], -1)                # [G,P,D]
            # slot (p, j=g*D+d) lives at xTc column (coff + j)*128 + p
            cols = ((coff + np.arange(SL).reshape(G, 1, D)) * P
                    + np.arange(P)[None, :, None])               # [G,P,D]
            xv = np.where(valid.reshape(-1)[:, None],
                          x_aug[np.maximum(gsrc.reshape(-1), 0)],
                          0).astype(BF_NP)
            xTc[:, cols.reshape(-1)] = xv.T
            # edge-feature bias blocks over the group's SL slots
            blk = np.zeros((P, KC * 3, EF_R), dtype=np.float32)
            blk[:, :, D_EDGE] = MASK_VAL
            efv = np.where(valid[:, :, :, None], ef[eids], 0.0)  # [G,P,D,Ef]
            efv = efv.transpose(1, 0, 2, 3).reshape(P, SL, D_EDGE)
            vmask = valid.transpose(1, 0, 2).reshape(P, SL)
            blk[:, :SL, :D_EDGE] = efv
            blk[:, :SL, D_EDGE] = np.where(vmask, 0.0, MASK_VAL)
            blk[:, :, D_EDGE + 1] = 1.0
            # [P, KC, 3, EF_R] -> [3, EF_R, KC, P] -> [102, KC*128]
            efT[:, koff * P:(koff + KC) * P] = (
                blk.reshape(P, KC, 3, EF_R).transpose(2, 3, 1, 0)
                .reshape(EF3, KC * P).astype(BF_NP))
            coff += SL
            koff += KC
        xq = np.where(nl[:, None] >= 0, x[np.maximum(nl, 0)], 0.0).astype(np.float32)
        xqT_aug = np.concatenate([xq.T, np.ones((1, NPC), np.float32)],
                                 0).astype(BF_NP)
        xqr = (xq + np.asarray(bo, dtype=np.float32)[None, :])
        xq_g = np.ascontiguousarray(
            xqr.reshape(NT, P, D_NODE).transpose(1, 0, 2).reshape(P, NT * D_NODE))
        per_core.append({
            "efT": efT,
            "xTc": xTc,
            "xqT": np.ascontiguousarray(xqT_aug),
            "xq": xq_g,
            "wq": Wq_aug,
            "wkv": np.ascontiguousarray(Wkv_aug),
            "we": np.ascontiguousarray(We_blk),
            "wo": Wo_T,
            "gb": gb,
        })
    meta = dict(iters=iters)
    return per_core, node_lists, meta


# ------------------------------------------------------------- bass kernel --
def _build_kernel(meta, debug_mode=None):
    if debug_mode is None:
        debug_mode = os.environ.get("KERNEL_DEBUG_MODE", "")
    iters = meta["iters"]
    KC_i = [-(-(G * D) // 3) for (_, G, D) in iters]
    SD = sum(G * D for (_, G, D) in iters)
    SKC = sum(KC_i)
    # eft groups, aligned to iteration KC blocks
    NG = int(os.environ.get("KERNEL_NG", "7"))
    tgt_sz = -(-SKC // NG)
    gsz = []
    acc = 0
    for kc in KC_i:
        if acc + kc > tgt_sz and acc > 0:
            gsz.append(acc)
            acc = 0
        acc += kc
    gsz.append(acc)
    nc = bacc.Bacc(None, target_bir_lowering=False)

    # engine assignment knobs for the fungible ops ("dve" | "pool")
    ENG = dict(qk="pool", exv="pool", scf1="dve", scf2="dve",
               unnf1="pool", ln_a="pool", ln_b="dve")
    ENG.update(json.loads(os.environ.get("KERNEL_ENG", "{}")))
    CFG = dict(vsplit=0, chunked_ep=1, eftq="sync", wq_q="scalar",
               xqq="gpsimd", xtb=3, smb=4, midb=4, ktb=4, ksplit=0, chunk=8,
               kvb=2, xti=1, epb=[8, 16, 24, 32, 40, 46, 49], eftb=2,
               vtail=999)
    CFG.update(json.loads(os.environ.get("KERNEL_CFG", "{}")))

    def VE(key):
        return nc.gpsimd if ENG[key] == "pool" else nc.vector

    efT = nc.dram_tensor("efT", [EF3, SKC * P], BF16, kind="ExternalInput")
    xTc = nc.dram_tensor("xTc", [65, SD * P], BF16, kind="ExternalInput")
    xqT = nc.dram_tensor("xqT", [65, NPC], BF16, kind="ExternalInput")
    xq = nc.dram_tensor("xq", [P, NT * D_NODE], F32, kind="ExternalInput")
    wq = nc.dram_tensor("wq", [65, D_NODE], BF16, kind="ExternalInput")
    wkv = nc.dram_tensor("wkv", [65, 2 * D_NODE], BF16, kind="ExternalInput")
    we = nc.dram_tensor("we", [EF3, 3 * H], BF16, kind="ExternalInput")
    wo = nc.dram_tensor("wo", [D_NODE, D_NODE], BF16, kind="ExternalInput")
    gb = nc.dram_tensor("gb", [2, D_NODE], F32, kind="ExternalInput")
    y = nc.dram_tensor("y", [P, NT * D_NODE], F32, kind="ExternalOutput")

    with tile.TileContext(nc) as tc:
        with (
            tc.tile_pool(name="singles", bufs=1) as singles,
        ):
            wq_sb = singles.tile([65, D_NODE], BF16)
            getattr(nc, CFG["wq_q"]).dma_start(out=wq_sb[:], in_=wq[:])
            wkv_sb = singles.tile([65, 2 * D_NODE], BF16)
            getattr(nc, CFG["wq_q"]).dma_start(out=wkv_sb[:], in_=wkv[:])
            we_sb = singles.tile([EF3, 3 * H], BF16)
            getattr(nc, CFG["wq_q"]).dma_start(out=we_sb[:], in_=we[:])
            wo_sb = singles.tile([D_NODE, D_NODE], BF16)
            getattr(nc, CFG["wq_q"]).dma_start(out=wo_sb[:], in_=wo[:])
            gamma_sb = singles.tile([P, D_NODE], F32)
            getattr(nc, CFG["xqq"]).dma_start(
                out=gamma_sb[:],
                in_=bass.AP(tensor=gb[:].tensor, offset=0,
                            ap=[[0, P], [1, D_NODE]]))
            beta_sb = singles.tile([P, D_NODE], F32)
            getattr(nc, CFG["xqq"]).dma_start(
                out=beta_sb[:],
                in_=bass.AP(tensor=gb[:].tensor, offset=D_NODE,
                            ap=[[0, P], [1, D_NODE]]))
            xqT_sb = singles.tile([65, NPC], BF16)
            for qc in range(4):
                c0 = (NPC // 4) * qc
                c1 = (NPC // 4) * (qc + 1) if qc < 3 else NPC
                getattr(nc, CFG["wq_q"]).dma_start(
                    out=xqT_sb[:, c0:c1], in_=xqT[:, c0:c1])
            xq_sb = singles.tile([P, NT, D_NODE], F32)
            getattr(nc, CFG["xqq"]).dma_start(out=xq_sb[:], in_=xq[:])
            ident = singles.tile([P, P], BF16)
            make_identity(nc, ident[:])
            eps_sb = singles.tile([P, 1], F32)
            nc.vector.memset(eps_sb[:], LN_EPS)
            q_all = singles.tile([P, NT, D_NODE], BF16)
            yout_sb = singles.tile([P, NT, D_NODE], F32)
            mv_sb = singles.tile([P, NT, 2], F32)
            sd_sb = singles.tile([P, NT], F32)
            rsd_sb = singles.tile([P, NT], F32)
            mursd_sb = singles.tile([P, NT], F32)

            with (
                tc.tile_pool(name="xtp", bufs=CFG["xtb"]) as xtp,
                tc.tile_pool(name="kvp", bufs=CFG["kvb"], space="PSUM") as kvp,
                tc.tile_pool(name="ktp", bufs=CFG["ktb"]) as ktp,
                tc.tile_pool(name="vtp", bufs=CFG["ktb"]) as vtp,
                tc.tile_pool(name="eft", bufs=CFG["eftb"]) as eftp,
                tc.tile_pool(name="mid", bufs=CFG["midb"]) as midp,
                tc.tile_pool(name="sml", bufs=CFG["smb"]) as smlp,
                tc.tile_pool(name="pb", bufs=2, space="PSUM") as pb,
                tc.tile_pool(name="pt", bufs=1, space="PSUM") as ptp,
                tc.tile_pool(name="py", bufs=1, space="PSUM") as pyp,
            ):
                # pin the ln+exp+copy ACT table once; the auto-placement pass
                # would otherwise bounce between exp-only and ln tables
                nc.scalar.add_instruction(mybir.InstLoadActFuncSet(
                    name=nc.get_next_instruction_name(), ins=[], outs=[],
                    act_func_set_id=6))

                # ---- all Q' tiles upfront: 49 matmuls, 13 PSUM drains ----
                for g in range(-(-NT // 4)):
                    n4 = min(4, NT - g * 4)
                    qp = pyp.tile([P, 4, D_NODE], F32, name="qp", tag="yp")
                    for j in range(n4):
                        t = g * 4 + j
                        nc.tensor.matmul(
                            out=qp[:, j, :],
                            lhsT=xqT_sb[:, t * P:(t + 1) * P],
                            rhs=wq_sb[:], start=True, stop=True)
                    nc.vector.tensor_copy(out=q_all[:, g * 4:g * 4 + n4, :],
                                          in_=qp[:, 0:n4, :])

                coff = [0]
                koff = [0]
                goff = [0]
                gi = [0]
                gleft = [0]
                gbase = [0]
                eft_sb = [None]
                ep_done = 0
                ep_cnt = 0
                ep_mark = [10 ** 9, -1]
                xt_sb = [None, 0, 0]     # tile, slot offset, iters left
                built = {}               # it -> (kvF, biasp)

                def build_kv(it):
                    t0, G, D = iters[it]
                    SL = G * D
                    KC = KC_i[it]
                    # source features, k-major slots; one DMA covers XTI iters
                    if xt_sb[0] is None:
                        nsl = sum(g * d for (_, g, d)
                                  in iters[it:it + CFG["xti"]])
                        xt_sb[0] = xtp.tile([65, nsl * P], BF16, name="xt_sb",
                                            tag="xt")
                        nc.sync.dma_start(
                            out=xt_sb[0][:],
                            in_=xTc[:, coff[0] * P:(coff[0] + nsl) * P])
                        xt_sb[1] = 0
                        xt_sb[2] = CFG["xti"]
                    if gleft[0] == 0:
                        skc = gsz[gi[0]]
                        eft_sb[0] = eftp.tile([EF3, skc, P], BF16, tag="eft",
                                              name="eft_sb")
                        getattr(nc, CFG["eftq"]).dma_start(
                            out=eft_sb[0][:],
                            in_=efT[:, goff[0] * P:(goff[0] + skc) * P])
                        gi[0] += 1
                        gleft[0] = skc
                        gbase[0] = goff[0]
                        goff[0] += skc
                    # position of this iter's KC blocks within the group
                    kbase = koff[0] - gbase[0]

                    # KV build: target node on partitions, no DRAM staging
                    CH = CFG["chunk"]
                    kvF = vtp.tile([P, 2 * D_NODE, SL], BF16, name="kvF",
                                   tag="vt")
                    xo = xt_sb[1]
                    for c8 in range(-(-SL // CH)):
                        j0 = c8 * CH
                        kk = min(CH, SL - j0)
                        pt = kvp.tile([P, CH, 2 * D_NODE], F32, name="pt",
                                      tag="kv")
                        for j in range(kk):
                            nc.tensor.matmul(
                                out=pt[:, j, :],
                                lhsT=xt_sb[0][:, (xo + j0 + j) * P:
                                              (xo + j0 + j + 1) * P],
                                rhs=wkv_sb[:], start=True, stop=True)
                        on_dve = ((CFG["vsplit"]
                                   and (it * 7 + c8) % CFG["vsplit"] == 0)
                                  or it >= CFG["vtail"])
                        if on_dve:
                            nc.vector.tensor_copy(
                                out=kvF[:, :, j0:j0 + kk],
                                in_=pt[:, 0:kk, :].rearrange("p j f -> p f j"))
                        else:
                            nc.scalar.copy(
                                out=kvF[:, :, j0:j0 + kk],
                                in_=pt[:, 0:kk, :].rearrange("p j f -> p f j"))

                    # per-edge bias: 3 slots per matmul
                    biasp = pb.tile([P, 3 * KC, H], F32, tag="biasp",
                                    name="biasp")
                    for k in range(KC):
                        nc.tensor.matmul(out=biasp[:, 3 * k:3 * (k + 1), :],
                                         lhsT=eft_sb[0][:, kbase + k, :],
                                         rhs=we_sb[:], start=True, stop=True)
                    built[it] = (kvF, biasp)
                    coff[0] += SL
                    koff[0] += KC
                    gleft[0] -= KC
                    xt_sb[1] += SL
                    xt_sb[2] -= 1
                    if xt_sb[2] == 0:
                        xt_sb[0] = None

                build_kv(0)
                for it, (t0, G, D) in enumerate(iters):
                    SL = G * D
                    KC = KC_i[it]
                    if it + 1 < len(iters):
                        build_kv(it + 1)
                    kvF, biasp = built.pop(it)

                    # ---- scores: QK mul then a w-fold chain (slots stay
                    # innermost-packed so every fold runs in DVE 2x mode) ----
                    qkp = midp.tile([P, D_NODE, SL], BF16, tag="qkp",
                                    name="qkp")
                    q_f = bass.AP(tensor=q_all[:].tensor,
                                  offset=q_all[:].offset + t0 * D_NODE,
                                  ap=[q_all[:].ap[0], [1, D_NODE], [0, SL]])
                    VE("qk").tensor_mul(out=qkp[:],
                                        in0=kvF[:, 0:D_NODE, :], in1=q_f)
                    qk4 = qkp[:].rearrange("p (h w) s -> p h w s", h=H)
                    sf1 = smlp.tile([P, H, 8, SL], BF16, tag="sf1", name="sf1")
                    VE("scf1").tensor_add(out=sf1[:], in0=qk4[:, :, 0:8, :],
                                          in1=qk4[:, :, 8:16, :])
                    sf2 = smlp.tile([P, H, 4, SL], BF16, tag="sf2", name="sf2")
                    VE("scf2").tensor_add(out=sf2[:], in0=sf1[:, :, 0:4, :],
                                          in1=sf1[:, :, 4:8, :])
                    sf3 = smlp.tile([P, H, 2, SL], BF16, tag="sf3", name="sf3")
                    nc.vector.tensor_add(out=sf3[:], in0=sf2[:, :, 0:2, :],
                                         in1=sf2[:, :, 2:4, :])
                    sc2 = smlp.tile([P, H, SL], F32, tag="sc2", name="sc2")
                    nc.vector.tensor_add(out=sc2[:], in0=sf3[:, :, 0, :],
                                         in1=sf3[:, :, 1, :])
                    bias_h = biasp[:, 0:SL, :].rearrange("p s h -> p h s")
                    nc.vector.tensor_add(out=sc2[:], in0=sc2[:], in1=bias_h)
                    ex = smlp.tile([P, H, SL], BF16, tag="ex", name="ex")
                    nc.scalar.activation(
                        out=ex[:], in_=sc2[:],
                        func=mybir.ActivationFunctionType.Exp)

                    den = smlp.tile([P, H, G], F32, tag="den", name="den")
                    nc.vector.tensor_reduce(
                        out=den[:],
                        in_=ex[:].rearrange("p h (g d) -> p h g d", g=G),
                        axis=mybir.AxisListType.X, op=mybir.AluOpType.add)
                    rden = smlp.tile([P, H, G], F32, tag="rden", name="rden")
                    nc.gpsimd.tensor_scalar_add(den[:], den[:], 1e-10)
                    nc.vector.reciprocal(out=rden[:], in_=den[:])

                    # ---- weighted V aggregation: mul, fold, reduce ----
                    exv = midp.tile([P, D_NODE, SL], BF16, tag="exv",
                                    name="exv")
                    ex_b = bass.AP(tensor=ex[:].tensor, offset=ex[:].offset,
                                   ap=[ex[:].ap[0], [SL, H], [0, D_H],
                                       [1, SL]])
                    VE("exv").tensor_mul(out=exv[:],
                                         in0=kvF[:, D_NODE:2 * D_NODE, :],
                                         in1=ex_b)
                    hD = D // 2
                    exv4 = exv[:].rearrange("p w (g d) -> p w g d", g=G)
                    uf1 = midp.tile([P, D_NODE, G, hD], BF16, tag="uf1",
                                    name="uf1")
                    VE("unnf1").tensor_add(out=uf1[:], in0=exv4[:, :, :, 0:hD],
                                           in1=exv4[:, :, :, hD:D])
                    unn = smlp.tile([P, D_NODE, G], F32, tag="unn", name="unn")
                    nc.vector.tensor_reduce(
                        out=unn[:], in_=uf1[:], axis=mybir.AxisListType.X,
                        op=mybir.AluOpType.add)
                    outn = smlp.tile([P, D_NODE, G], BF16, tag="outn",
                                     name="outn")
                    rden_b = bass.AP(tensor=rden[:].tensor,
                                     offset=rden[:].offset,
                                     ap=[rden[:].ap[0], [G, H], [0, D_H],
                                         [1, G]])
                    nc.gpsimd.tensor_mul(
                        out=outn[:].rearrange("p (h w) g -> p h w g", h=H),
                        in0=unn[:].rearrange("p (h w) g -> p h w g", h=H),
                        in1=rden_b)

                    # ---- projection: y = outn @ Wo.T + (x + bo) ----
                    tp = ptp.tile([D_NODE, G, P], BF16, tag="tp", name="tp")
                    for g in range(G):
                        nc.tensor.transpose(out=tp[:, g, :],
                                            in_=outn[:, :, g],
                                            identity=ident[:])
                    tps = smlp.tile([D_NODE, G, P], BF16, tag="tps",
                                    name="tps")
                    nc.vector.tensor_copy(out=tps[:], in_=tp[:])
                    yp = pyp.tile([P, G, D_NODE], F32, tag="yp", name="yp")
                    for g in range(G):
                        nc.tensor.matmul(out=yp[:, g, :], lhsT=tps[:, g, :],
                                         rhs=wo_sb[:], start=True, stop=True)
                    nc.vector.tensor_add(out=yout_sb[:, t0:t0 + G, :],
                                         in0=yp[:],
                                         in1=xq_sb[:, t0:t0 + G, :])
                    for g in range(G):
                        stats = smlp.tile([P, 6], F32, tag="stats",
                                          name="stats")
                        nc.vector.bn_stats(out=stats[:],
                                           in_=yout_sb[:, t0 + g, :])
                        nc.vector.bn_aggr(out=mv_sb[:, t0 + g, :],
                                          in_=stats[:])

                    # ---- chunked layernorm epilogue: overlap with loop ----
                    ep_cnt += G
                    ep_bounds = (tuple(CFG["epb"]) if CFG["chunked_ep"]
                                 else (NT,))
                    hit = [b for b in ep_bounds if ep_done < b <= ep_cnt]
                    if not debug_mode and hit:
                        # completed-and-unnormalized tiles form one contiguous
                        # range in either iteration order
                        lo = min(t0, ep_mark[0])
                        hi = max(t0 + G, ep_mark[1])
                        te0, te1 = lo, hi
                        ep_mark[0], ep_mark[1] = 10 ** 9, -1
                        nch = te1 - te0
                        mvs = mv_sb[:, te0:te1, :]
                        mu = bass.AP(tensor=mvs.tensor, offset=mvs.offset,
                                     ap=[mvs.ap[0], [2, nch]])
                        var = bass.AP(tensor=mvs.tensor, offset=mvs.offset + 1,
                                      ap=[mvs.ap[0], [2, nch]])
                        # rsd = exp(-0.5*ln(var+eps)); ln+exp share one ACT
                        # function table (sqrt would force a table swap)
                        nc.scalar.activation(
                            out=sd_sb[:, te0:te1], in_=var,
                            func=mybir.ActivationFunctionType.Ln,
                            bias=eps_sb[:])
                        nc.scalar.activation(
                            out=rsd_sb[:, te0:te1], in_=sd_sb[:, te0:te1],
                            func=mybir.ActivationFunctionType.Exp,
                            scale=-0.5)
                        nc.vector.tensor_mul(out=mursd_sb[:, te0:te1], in0=mu,
                                             in1=rsd_sb[:, te0:te1])

                        def bc_t(a):   # [P, nch] -> [P, nch, 64]
                            return bass.AP(tensor=a.tensor, offset=a.offset,
                                           ap=list(a.ap) + [[0, D_NODE]])

                        def bc_f(a):   # [P, 64] -> [P, nch, 64]
                            return bass.AP(tensor=a.tensor, offset=a.offset,
                                           ap=[a.ap[0], [0, nch], a.ap[1]])

                        yv = yout_sb[:, te0:te1, :]
                        VE("ln_a").tensor_mul(out=yv, in0=yv,
                                              in1=bc_t(rsd_sb[:, te0:te1]))
                        VE("ln_b").tensor_sub(out=yv, in0=yv,
                                              in1=bc_t(mursd_sb[:, te0:te1]))
                        VE("ln_a").tensor_mul(out=yv, in0=yv,
                                              in1=bc_f(gamma_sb[:]))
                        VE("ln_b").tensor_add(out=yv, in0=yv,
                                              in1=bc_f(beta_sb[:]))
                        getattr(nc, CFG["xqq"]).dma_start(
                            out=y[:, te0 * D_NODE:te1 * D_NODE], in_=yv)
                        ep_done = ep_cnt
                    else:
                        ep_mark[0] = min(ep_mark[0], t0)
                        ep_mark[1] = max(ep_mark[1], t0 + G)

    nc.compile()
    return nc


# ------------------------------------------------------------------ driver --
def kernel(**inputs) -> np.ndarray:
    per_core, node_lists, meta = _host_prep(**inputs)
    nc = _build_kernel(meta)
    res = run_bass_kernel_spmd(nc, per_core, core_ids=list(range(NCORES)))
    y_full = np.zeros((N, D_NODE), dtype=np.float32)
    for c in range(NCORES):
        yc = res.results[c]["y"].reshape(P, NT, D_NODE).transpose(1, 0, 2)
        yc = yc.reshape(NPC, D_NODE)
        nl = node_lists[c]
        real = nl >= 0
        y_full[nl[real]] = yc[real]
    return y_full


# revision 43
# speedup vs baseline: 1.3551x; 1.0268x over previous
"""NodeAttention (GNN scatter-softmax attention) on 8 Trainium2 NeuronCores.

Strategy (v5 — on-chip KV, paired tiles):
- Host deals nodes to 8 cores round-robin by degree rank, so every core sees a
  near-identical degree profile; one static NEFF serves all cores (SPMD).
- Per core: 49 node-tiles x 128 nodes; node-tile t gets a dense padded slot
  grid [128, D] (D = max degree in the PAIR of tiles across cores; tiles are
  processed two-at-a-time so fixed per-instruction costs amortize over 2x the
  slots; adjacent degree-sorted tiles have near-identical D so pairing adds
  only ~2% padding).
- The host replicates x per SLOT in k-major order, so each KV projection
  matmul's PSUM output lands with the TARGET node on partitions. K is copied
  to SBUF as [P, slots, 64] and V as [P, 64, slots] (feature-major) straight
  from PSUM — no DRAM staging, no gather.
- Per-edge scores: bf16 QK mul in DVE 2x mode, fold-fold-reduce for the
  per-head dot; per-edge bias via block-diagonal matmuls (3 slots x 34
  ef-features on 102 partitions); exp on ACT; softmax normalization AFTER
  aggregation. Feature-major V lets the attn broadcast ride a middle dim so
  the weighted-V mul is also 2x, with a fold+reduce over slots.
- Big muls and one fold run on the otherwise-idle GpSimd (Pool) engine
  (SBUF-only ops; GPSIMD cannot touch PSUM). PSUM->SBUF copies + exp on ACT.
- LayerNorm runs chunked (4 output chunks) so the epilogue and y write-back
  overlap the main loop.
- No max-subtraction in softmax (scores are O(10); identical result).
  Padding slots masked via an extra edge-feature column (weight 1, value -75).
- temp/sqrt(d) folded into Wq; temp folded into We; be via a ones column;
  bo folded into the residual x shipped from host.
"""

import os
import json
import numpy as np
import ml_dtypes

import concourse.bass as bass
import concourse.bacc as bacc
import concourse.tile as tile
from concourse import mybir
from concourse.bass_utils import run_bass_kernel_spmd
from concourse.masks import make_identity

N, E = 50000, 800000
D_NODE, D_EDGE, H = 64, 32, 4
D_H = D_NODE // H
LN_EPS = 1e-5
NCORES = 8
P = 128
NT = 49                # node tiles per core
NPC = NT * P           # padded nodes per core = 6272
GRP = int(os.environ.get('KERNEL_GRP', '1'))   # tiles per iteration
EF_R = 34              # 32 ef features + mask col + ones col (carries be)
EF3 = 3 * EF_R         # 102: three slots stacked on partitions
MASK_VAL = -75.0
F32 = mybir.dt.float32
BF16 = mybir.dt.bfloat16
BF_NP = ml_dtypes.bfloat16


# ---------------------------------------------------------------- host prep --
def _host_prep(node_features, edge_features, edge_index, Wq, bq, Wk, bk, Wv, bv,
               We, be, Wo, bo, ln_gamma, ln_beta, log_temp):
    x = np.ascontiguousarray(np.asarray(node_features, dtype=np.float32))
    ef = np.ascontiguousarray(np.asarray(edge_features, dtype=np.float32))
    src = np.asarray(edge_index[0], dtype=np.int64)
    tgt = np.asarray(edge_index[1], dtype=np.int64)
    temp = np.exp(np.asarray(log_temp, dtype=np.float32))

    deg = np.bincount(tgt, minlength=N)
    order = np.argsort(-deg, kind="stable")
    node_lists = []
    for c in range(NCORES):
        nl = order[c::NCORES]
        nl = np.concatenate([nl, np.full(NPC - len(nl), -1, dtype=np.int64)])
        node_lists.append(nl)

    D_t = np.zeros(NT, dtype=np.int64)
    for c in range(NCORES):
        d = np.where(node_lists[c] >= 0, deg[np.maximum(node_lists[c], 0)], 0)
        D_t = np.maximum(D_t, d.reshape(NT, P).max(axis=1))
    D_t = np.maximum(D_t, 2)
    D_t = D_t + (D_t % 2)          # even D so the aggregation fold halves cleanly
    # iterations: greedy-group consecutive tiles to a slot budget so small
    # tiles share one iteration's fixed costs (G*D <= SLMAX, tiles padded to
    # the group max D)
    SLMAX = int(os.environ.get("KERNEL_SLMAX", "32"))
    iters = []                     # (t0, G, D)
    t = 0
    while t < NT:
        if SLMAX:
            G = 1
            while (t + G < NT and (G + 1) * int(D_t[t]) <= SLMAX
                   and G < 8):
                G += 1
        else:
            G = min(GRP, NT - t)
        D = int(D_t[t:t + G].max())
        D_t[t:t + G] = D
        iters.append((t, G, D))
        t += G
    if os.environ.get("KERNEL_REV", "0") == "1":
        iters = iters[::-1]
    assert D_t.max() <= 128, f"degree {D_t.max()} exceeds single-bank design"
    SD = int(D_t.sum())
    KC_i = [-(-(G * D) // 3) for (_, G, D) in iters]
    SKC = sum(KC_i)

    eorder = np.argsort(tgt, kind="stable")
    estart = np.zeros(N + 1, dtype=np.int64)
    np.cumsum(deg, out=estart[1:])

    qscale = (np.repeat(temp, D_H) / np.sqrt(D_H)).astype(np.float32)
    Wq_aug = (np.concatenate([np.asarray(Wq).T, np.asarray(bq)[None, :]], 0)
              * qscale[None, :]).astype(BF_NP)                           # [65,64]
    Wkv_aug = np.concatenate(
        [np.concatenate([np.asarray(Wk).T, np.asarray(Wv).T], 1),
         np.concatenate([np.asarray(bk), np.asarray(bv)])[None, :]], 0
    ).astype(BF_NP)                                                      # [65,128]
    We_augT = np.concatenate(
        [np.asarray(We).T * temp[None, :],
         np.ones((1, H), np.float32),
         (np.asarray(be) * temp)[None, :]], 0
    ).astype(np.float32)                                                 # [34,4]
    We_blk = np.zeros((EF3, 3 * H), dtype=np.float32)
    for j3 in range(3):
        We_blk[j3 * EF_R:(j3 + 1) * EF_R, j3 * H:(j3 + 1) * H] = We_augT
    We_blk = We_blk.astype(BF_NP)
    Wo_T = np.ascontiguousarray(np.asarray(Wo).T).astype(BF_NP)          # [64,64]
    gb = np.stack([np.asarray(ln_gamma), np.asarray(ln_beta)]).astype(np.float32)

    x_aug = np.concatenate(
        [x, np.ones((N, 1), np.float32)], 1).astype(BF_NP)               # [N,65]

    per_core = []
    for c in range(NCORES):
        nl = node_lists[c]
        efT = np.zeros((EF3, SKC * P), dtype=BF_NP)
        xTc = np.zeros((65, SD * P), dtype=BF_NP)
        coff = 0
        koff = 0
        for it, (t0, G, D) in enumerate(iters):
            SL = G * D
            KC = KC_i[it]
            # gather edge ids for each tile in the group: slot j = g*D + d
            nlt = nl[t0 * P:(t0 + G) * P].reshape(G, P)          # [G,P]
            degt = np.where(nlt >= 0, deg[np.maximum(nlt, 0)], 0)
            k = np.arange(D)
            valid = k[None, None, :] < degt[:, :, None]          # [G,P,D]
            pos = estart[np.maximum(nlt, 0)][:, :, None] + k[None, None, :]
            eids = eorder[np.minimum(pos, E - 1)]
            eids = np.where(valid, eids, 0)
            gsrc = np.where(valid, src[eids], -1)                # [G,P,D]
            # slot (p, j=g*D+d) lives at xTc column (coff + j)*128 + p
            cols = ((coff + np.arange(SL).reshape(G, 1, D)) * P
                    + np.arange(P)[None, :, None])               # [G,P,D]
            xv = np.where(valid.reshape(-1)[:, None],
                          x_aug[np.maximum(gsrc.reshape(-1), 0)],
                          0).astype(BF_NP)
            xTc[:, cols.reshape(-1)] = xv.T
            # edge-feature bias blocks over the group's SL slots
            blk = np.zeros((P, KC * 3, EF_R), dtype=np.float32)
            blk[:, :, D_EDGE] = MASK_VAL
            efv = np.where(valid[:, :, :, None], ef[eids], 0.0)  # [G,P,D,Ef]
            efv = efv.transpose(1, 0, 2, 3).reshape(P, SL, D_EDGE)
            vmask = valid.transpose(1, 0, 2).reshape(P, SL)
            blk[:, :SL, :D_EDGE] = efv
            blk[:, :SL, D_EDGE] = np.where(vmask, 0.0, MASK_VAL)
            blk[:, :, D_EDGE + 1] = 1.0
            # [P, KC, 3, EF_R] -> [3, EF_R, KC, P] -> [102, KC*128]
            efT[:, koff * P:(koff + KC) * P] = (
                blk.reshape(P, KC, 3, EF_R).transpose(2, 3, 1, 0)
                .reshape(EF3, KC * P).astype(BF_NP))
            coff += SL
            koff += KC
        xq = np.where(nl[:, None] >= 0, x[np.maximum(nl, 0)], 0.0).astype(np.float32)
        xqT_aug = np.concatenate([xq.T, np.ones((1, NPC), np.float32)],
                                 0).astype(BF_NP)
        xqr = (xq + np.asarray(bo, dtype=np.float32)[None, :])
        xq_g = np.ascontiguousarray(
            xqr.reshape(NT, P, D_NODE).transpose(1, 0, 2).reshape(P, NT * D_NODE))
        per_core.append({
            "efT": efT,
            "xTc": xTc,
            "xqT": np.ascontiguousarray(xqT_aug),
            "xq": xq_g,
            "wq": Wq_aug,
            "wkv": np.ascontiguousarray(Wkv_aug),
            "we": np.ascontiguousarray(We_blk),
            "wo": Wo_T,
            "gb": gb,
        })
    meta = dict(iters=iters)
    return per_core, node_lists, meta


# ------------------------------------------------------------- bass kernel --
def _build_kernel(meta, debug_mode=None):
    if debug_mode is None:
        debug_mode = os.environ.get("KERNEL_DEBUG_MODE", "")
    iters = meta["iters"]
    KC_i = [-(-(G * D) // 3) for (_, G, D) in iters]
    SD = sum(G * D for (_, G, D) in iters)
    SKC = sum(KC_i)
    # eft groups, aligned to iteration KC blocks
    NG = int(os.environ.get("KERNEL_NG", "7"))
    tgt_sz = -(-SKC // NG)
    gsz = []
    acc = 0
    for kc in KC_i:
        if acc + kc > tgt_sz and acc > 0:
            gsz.append(acc)
            acc = 0
        acc += kc
    gsz.append(acc)
    nc = bacc.Bacc(None, target_bir_lowering=False)

    # engine assignment knobs for the fungible ops ("dve" | "pool")
    ENG = dict(qk="pool", exv="pool", scf1="dve", scf2="dve",
               unnf1="pool", ln_a="pool", ln_b="dve")
    ENG.update(json.loads(os.environ.get("KERNEL_ENG", "{}")))
    CFG = dict(vsplit=0, chunked_ep=1, eftq="sync", wq_q="scalar",
               xqq="gpsimd", xtb=3, smb=4, midb=4, ktb=4, ksplit=0, chunk=8,
               kvb=2, xti=1, epb=[8, 16, 24, 32, 40, 46, 49], eftb=2,
               vtail=999)
    CFG.update(json.loads(os.environ.get("KERNEL_CFG", "{}")))

    def VE(key):
        return nc.gpsimd if ENG[key] == "pool" else nc.vector

    efT = nc.dram_tensor("efT", [EF3, SKC * P], BF16, kind="ExternalInput")
    xTc = nc.dram_tensor("xTc", [65, SD * P], BF16, kind="ExternalInput")
    xqT = nc.dram_tensor("xqT", [65, NPC], BF16, kind="ExternalInput")
    xq = nc.dram_tensor("xq", [P, NT * D_NODE], F32, kind="ExternalInput")
    wq = nc.dram_tensor("wq", [65, D_NODE], BF16, kind="ExternalInput")
    wkv = nc.dram_tensor("wkv", [65, 2 * D_NODE], BF16, kind="ExternalInput")
    we = nc.dram_tensor("we", [EF3, 3 * H], BF16, kind="ExternalInput")
    wo = nc.dram_tensor("wo", [D_NODE, D_NODE], BF16, kind="ExternalInput")
    gb = nc.dram_tensor("gb", [2, D_NODE], F32, kind="ExternalInput")
    y = nc.dram_tensor("y", [P, NT * D_NODE], F32, kind="ExternalOutput")

    with tile.TileContext(nc) as tc:
        with (
            tc.tile_pool(name="singles", bufs=1) as singles,
        ):
            wq_sb = singles.tile([65, D_NODE], BF16)
            getattr(nc, CFG["wq_q"]).dma_start(out=wq_sb[:], in_=wq[:])
            wkv_sb = singles.tile([65, 2 * D_NODE], BF16)
            getattr(nc, CFG["wq_q"]).dma_start(out=wkv_sb[:], in_=wkv[:])
            we_sb = singles.tile([EF3, 3 * H], BF16)
            getattr(nc, CFG["wq_q"]).dma_start(out=we_sb[:], in_=we[:])
            wo_sb = singles.tile([D_NODE, D_NODE], BF16)
            getattr(nc, CFG["wq_q"]).dma_start(out=wo_sb[:], in_=wo[:])
            gamma_sb = singles.tile([P, D_NODE], F32)
            getattr(nc, CFG["xqq"]).dma_start(
                out=gamma_sb[:],
                in_=bass.AP(tensor=gb[:].tensor, offset=0,
                            ap=[[0, P], [1, D_NODE]]))
            beta_sb = singles.tile([P, D_NODE], F32)
            getattr(nc, CFG["xqq"]).dma_start(
                out=beta_sb[:],
                in_=bass.AP(tensor=gb[:].tensor, offset=D_NODE,
                            ap=[[0, P], [1, D_NODE]]))
            xqT_sb = singles.tile([65, NPC], BF16)
            for qc in range(4):
                c0 = (NPC // 4) * qc
                c1 = (NPC // 4) * (qc + 1) if qc < 3 else NPC
                getattr(nc, CFG["wq_q"]).dma_start(
                    out=xqT_sb[:, c0:c1], in_=xqT[:, c0:c1])
            xq_sb = singles.tile([P, NT, D_NODE], F32)
            getattr(nc, CFG["xqq"]).dma_start(out=xq_sb[:], in_=xq[:])
            ident = singles.tile([P, P], BF16)
            make_identity(nc, ident[:])
            eps_sb = singles.tile([P, 1], F32)
            nc.vector.memset(eps_sb[:], LN_EPS)
            q_all = singles.tile([P, NT, D_NODE], BF16)
            yout_sb = singles.tile([P, NT, D_NODE], F32)
            mv_sb = singles.tile([P, NT, 2], F32)
            sd_sb = singles.tile([P, NT], F32)
            rsd_sb = singles.tile([P, NT], F32)
            mursd_sb = singles.tile([P, NT], F32)

            with (
                tc.tile_pool(name="xtp", bufs=CFG["xtb"]) as xtp,
                tc.tile_pool(name="kvp", bufs=CFG["kvb"], space="PSUM") as kvp,
                tc.tile_pool(name="ktp", bufs=CFG["ktb"]) as ktp,
                tc.tile_pool(name="vtp", bufs=CFG["ktb"]) as vtp,
                tc.tile_pool(name="eft", bufs=CFG["eftb"]) as eftp,
                tc.tile_pool(name="mid", bufs=CFG["midb"]) as midp,
                tc.tile_pool(name="sml", bufs=CFG["smb"]) as smlp,
                tc.tile_pool(name="pb", bufs=2, space="PSUM") as pb,
                tc.tile_pool(name="pt", bufs=1, space="PSUM") as ptp,
                tc.tile_pool(name="py", bufs=1, space="PSUM") as pyp,
            ):
                # pin the ln+exp+copy ACT table once; the auto-placement pass
                # would otherwise bounce between exp-only and ln tables
                nc.scalar.add_instruction(mybir.InstLoadActFuncSet(
                    name=nc.get_next_instruction_name(), ins=[], outs=[],
                    act_func_set_id=6))

                # ---- all Q' tiles upfront: 49 matmuls, 13 PSUM drains ----
                for g in range(-(-NT // 4)):
                    n4 = min(4, NT - g * 4)
                    qp = pyp.tile([P, 4, D_NODE], F32, name="qp", tag="yp")
                    for j in range(n4):
                        t = g * 4 + j
                        nc.tensor.matmul(
                            out=qp[:, j, :],
                            lhsT=xqT_sb[:, t * P:(t + 1) * P],
                            rhs=wq_sb[:], start=True, stop=True)
                    nc.vector.tensor_copy(out=q_all[:, g * 4:g * 4 + n4, :],
                                          in_=qp[:, 0:n4, :])

                coff = [0]
                koff = [0]
                goff = [0]
                gi = [0]
                gleft = [0]
                gbase = [0]
                eft_sb = [None]
                ep_done = 0
                ep_cnt = 0
                ep_mark = [10 ** 9, -1]
                xt_sb = [None, 0, 0]     # tile, slot offset, iters left
                built = {}               # it -> (kvF, biasp)

                def build_kv(it):
                    t0, G, D = iters[it]
                    SL = G * D
                    KC = KC_i[it]
                    # source features, k-major slots; one DMA covers XTI iters
                    if xt_sb[0] is None:
                        nsl = sum(g * d for (_, g, d)
                                  in iters[it:it + CFG["xti"]])
                        xt_sb[0] = xtp.tile([65, nsl * P], BF16, name="xt_sb",
                                            tag="xt")
                        nc.sync.dma_start(
                            out=xt_sb[0][:],
                            in_=xTc[:, coff[0] * P:(coff[0] + nsl) * P])
                        xt_sb[1] = 0
                        xt_sb[2] = CFG["xti"]
                    if gleft[0] == 0:
                        skc = gsz[gi[0]]
                        eft_sb[0] = eftp.tile([EF3, skc, P], BF16, tag="eft",
                                              name="eft_sb")
                        getattr(nc, CFG["eftq"]).dma_start(
                            out=eft_sb[0][:],
                            in_=efT[:, goff[0] * P:(goff[0] + skc) * P])
                        gi[0] += 1
                        gleft[0] = skc
                        gbase[0] = goff[0]
                        goff[0] += skc
                    # position of this iter's KC blocks within the group
                    kbase = koff[0] - gbase[0]

                    # KV build: target node on partitions, no DRAM staging
                    CH = CFG["chunk"]
                    kvF = vtp.tile([P, 2 * D_NODE, SL], BF16, name="kvF",
                                   tag="vt")
                    xo = xt_sb[1]
                    for c8 in range(-(-SL // CH)):
                        j0 = c8 * CH
                        kk = min(CH, SL - j0)
                        pt = kvp.tile([P, CH, 2 * D_NODE], F32, name="pt",
                                      tag="kv")
                        for j in range(kk):
                            nc.tensor.matmul(
                                out=pt[:, j, :],
                                lhsT=xt_sb[0][:, (xo + j0 + j) * P:
                                              (xo + j0 + j + 1) * P],
                                rhs=wkv_sb[:], start=True, stop=True)
                        on_dve = ((CFG["vsplit"]
                                   and (it * 7 + c8) % CFG["vsplit"] == 0)
                                  or it >= CFG["vtail"])
                        if on_dve:
                            nc.vector.tensor_copy(
                                out=kvF[:, :, j0:j0 + kk],
                                in_=pt[:, 0:kk, :].rearrange("p j f -> p f j"))
                        else:
                            nc.scalar.copy(
                                out=kvF[:, :, j0:j0 + kk],
                                in_=pt[:, 0:kk, :].rearrange("p j f -> p f j"))

                    # per-edge bias: 3 slots per matmul
                    biasp = pb.tile([P, 3 * KC, H], F32, tag="biasp",
                                    name="biasp")
                    for k in range(KC):
                        nc.tensor.matmul(out=biasp[:, 3 * k:3 * (k + 1), :],
                                         lhsT=eft_sb[0][:, kbase + k, :],
                                         rhs=we_sb[:], start=True, stop=True)
                    built[it] = (kvF, biasp)
                    coff[0] += SL
                    koff[0] += KC
                    gleft[0] -= KC
                    xt_sb[1] += SL
                    xt_sb[2] -= 1
                    if xt_sb[2] == 0:
                        xt_sb[0] = None

                build_kv(0)
                for it, (t0, G, D) in enumerate(iters):
                    SL = G * D
                    KC = KC_i[it]
                    if it + 1 < len(iters):
                        build_kv(it + 1)
                    kvF, biasp = built.pop(it)

                    # ---- scores: QK mul then a w-fold chain (slots stay
                    # innermost-packed so every fold runs in DVE 2x mode) ----
                    qkp = midp.tile([P, D_NODE, SL], BF16, tag="qkp",
                                    name="qkp")
                    q_f = bass.AP(tensor=q_all[:].tensor,
                                  offset=q_all[:].offset + t0 * D_NODE,
                                  ap=[q_all[:].ap[0], [1, D_NODE],
                                      [D_NODE, G], [0, D]])
                    VE("qk").tensor_mul(
                        out=qkp[:].rearrange("p f (g d) -> p f g d", g=G),
                        in0=kvF[:, 0:D_NODE, :].rearrange(
                            "p f (g d) -> p f g d", g=G),
                        in1=q_f)
                    qk4 = qkp[:].rearrange("p (h w) s -> p h w s", h=H)
                    sf1 = smlp.tile([P, H, 8, SL], BF16, tag="sf1", name="sf1")
                    VE("scf1").tensor_add(out=sf1[:], in0=qk4[:, :, 0:8, :],
                                          in1=qk4[:, :, 8:16, :])
                    sf2 = smlp.tile([P, H, 4, SL], BF16, tag="sf2", name="sf2")
                    VE("scf2").tensor_add(out=sf2[:], in0=sf1[:, :, 0:4, :],
                                          in1=sf1[:, :, 4:8, :])
                    sf3 = smlp.tile([P, H, 2, SL], BF16, tag="sf3", name="sf3")
                    nc.vector.tensor_add(out=sf3[:], in0=sf2[:, :, 0:2, :],
                                         in1=sf2[:, :, 2:4, :])
                    sc2 = smlp.tile([P, H, SL], F32, tag="sc2", name="sc2")
                    nc.vector.tensor_add(out=sc2[:], in0=sf3[:, :, 0, :],
                                         in1=sf3[:, :, 1, :])
                    bias_h = biasp[:, 0:SL, :].rearrange("p s h -> p h s")
                    nc.vector.tensor_add(out=sc2[:], in0=sc2[:], in1=bias_h)
                    ex = smlp.tile([P, H, SL], BF16, tag="ex", name="ex")
                    nc.scalar.activation(
                        out=ex[:], in_=sc2[:],
                        func=mybir.ActivationFunctionType.Exp)

                    den = smlp.tile([P, H, G], F32, tag="den", name="den")
                    nc.vector.tensor_reduce(
                        out=den[:],
                        in_=ex[:].rearrange("p h (g d) -> p h g d", g=G),
                        axis=mybir.AxisListType.X, op=mybir.AluOpType.add)
                    rden = smlp.tile([P, H, G], F32, tag="rden", name="rden")
                    nc.gpsimd.tensor_scalar_add(den[:], den[:], 1e-10)
                    nc.vector.reciprocal(out=rden[:], in_=den[:])

                    # ---- weighted V aggregation: mul, fold, reduce ----
                    exv = midp.tile([P, D_NODE, SL], BF16, tag="exv",
                                    name="exv")
                    ex_b = bass.AP(tensor=ex[:].tensor, offset=ex[:].offset,
                                   ap=[ex[:].ap[0], [SL, H], [0, D_H],
                                       [1, SL]])
                    VE("exv").tensor_mul(out=exv[:],
                                         in0=kvF[:, D_NODE:2 * D_NODE, :],
                                         in1=ex_b)
                    hD = D // 2
                    exv4 = exv[:].rearrange("p w (g d) -> p w g d", g=G)
                    uf1 = midp.tile([P, D_NODE, G, hD], BF16, tag="uf1",
                                    name="uf1")
                    VE("unnf1").tensor_add(out=uf1[:], in0=exv4[:, :, :, 0:hD],
                                           in1=exv4[:, :, :, hD:D])
                    unn = smlp.tile([P, D_NODE, G], F32, tag="unn", name="unn")
                    nc.vector.tensor_reduce(
                        out=unn[:], in_=uf1[:], axis=mybir.AxisListType.X,
                        op=mybir.AluOpType.add)
                    outn = smlp.tile([P, D_NODE, G], BF16, tag="outn",
                                     name="outn")
                    rden_b = bass.AP(tensor=rden[:].tensor,
                                     offset=rden[:].offset,
                                     ap=[rden[:].ap[0], [G, H], [0, D_H],
                                         [1, G]])
                    nc.gpsimd.tensor_mul(
                        out=outn[:].rearrange("p (h w) g -> p h w g", h=H),
                        in0=unn[:].rearrange("p (h w) g -> p h w g", h=H),
                        in1=rden_b)

                    # ---- projection: y = outn @ Wo.T + (x + bo) ----
                    tp = ptp.tile([D_NODE, G, P], BF16, tag="tp", name="tp")
                    for g in range(G):
                        nc.tensor.transpose(out=tp[:, g, :],
                                            in_=outn[:, :, g],
                                            identity=ident[:])
                    tps = smlp.tile([D_NODE, G, P], BF16, tag="tps",
                                    name="tps")
                    nc.vector.tensor_copy(out=tps[:], in_=tp[:])
                    yp = pyp.tile([P, G, D_NODE], F32, tag="yp", name="yp")
                    for g in range(G):
                        nc.tensor.matmul(out=yp[:, g, :], lhsT=tps[:, g, :],
                                         rhs=wo_sb[:], start=True, stop=True)
                    nc.vector.tensor_add(out=yout_sb[:, t0:t0 + G, :],
                                         in0=yp[:],
                                         in1=xq_sb[:, t0:t0 + G, :])
                    for g in range(G):
                        stats = smlp.tile([P, 6], F32, tag="stats",
                                          name="stats")
                        nc.vector.bn_stats(out=stats[:],
                                           in_=yout_sb[:, t0 + g, :])
                        nc.vector.bn_aggr(out=mv_sb[:, t0 + g, :],
                                          in_=stats[:])

                    # ---- chunked layernorm epilogue: overlap with loop ----
                    ep_cnt += G
                    ep_bounds = (tuple(CFG["epb"]) if CFG["chunked_ep"]
                                 else (NT,))
                    hit = [b for b in ep_bounds if ep_done < b <= ep_cnt]
                    if not debug_mode and hit:
                        # completed-and-unnormalized tiles form one contiguous
                        # range in either iteration order
                        lo = min(t0, ep_mark[0])
                        hi = max(t0 + G, ep_mark[1])
                        te0, te1 = lo, hi
                        ep_mark[0], ep_mark[1] = 10 ** 9, -1
                        nch = te1 - te0
                        mvs = mv_sb[:, te0:te1, :]
                        mu = bass.AP(tensor=mvs.tensor, offset=mvs.offset,
                                     ap=[mvs.ap[0], [2, nch]])
                        var = bass.AP(tensor=mvs.tensor, offset=mvs.offset + 1,
                                      ap=[mvs.ap[0], [2, nch]])
                        # rsd = exp(-0.5*ln(var+eps)); ln+exp share one ACT
                        # function table (sqrt would force a table swap)
                        nc.scalar.activation(
                            out=sd_sb[:, te0:te1], in_=var,
                            func=mybir.ActivationFunctionType.Ln,
                            bias=eps_sb[:])
                        nc.scalar.activation(
                            out=rsd_sb[:, te0:te1], in_=sd_sb[:, te0:te1],
                            func=mybir.ActivationFunctionType.Exp,
                            scale=-0.5)
                        nc.vector.tensor_mul(out=mursd_sb[:, te0:te1], in0=mu,
                                             in1=rsd_sb[:, te0:te1])

                        def bc_t(a):   # [P, nch] -> [P, nch, 64]
                            return bass.AP(tensor=a.tensor, offset=a.offset,
                                           ap=list(a.ap) + [[0, D_NODE]])

                        def bc_f(a):   # [P, 64] -> [P, nch, 64]
                            return bass.AP(tensor=a.tensor, offset=a.offset,
                                           ap=[a.ap[0], [0, nch], a.ap[1]])

                        yv = yout_sb[:, te0:te1, :]
                        VE("ln_a").tensor_mul(out=yv, in0=yv,
                                              in1=bc_t(rsd_sb[:, te0:te1]))
                        VE("ln_b").tensor_sub(out=yv, in0=yv,
                                              in1=bc_t(mursd_sb[:, te0:te1]))
                        VE("ln_a").tensor_mul(out=yv, in0=yv,
                                              in1=bc_f(gamma_sb[:]))
                        VE("ln_b").tensor_add(out=yv, in0=yv,
                                              in1=bc_f(beta_sb[:]))
                        getattr(nc, CFG["xqq"]).dma_start(
                            out=y[:, te0 * D_NODE:te1 * D_NODE], in_=yv)
                        ep_done = ep_cnt
                    else:
                        ep_mark[0] = min(ep_mark[0], t0)
                        ep_mark[1] = max(ep_mark[1], t0 + G)

    nc.compile()
    return nc


# ------------------------------------------------------------------ driver --
def kernel(**inputs) -> np.ndarray:
    per_core, node_lists, meta = _host_prep(**inputs)
    nc = _build_kernel(meta)
    res = run_bass_kernel_spmd(nc, per_core, core_ids=list(range(NCORES)))
    y_full = np.zeros((N, D_NODE), dtype=np.float32)
    for c in range(NCORES):
        yc = res.results[c]["y"].reshape(P, NT, D_NODE).transpose(1, 0, 2)
        yc = yc.reshape(NPC, D_NODE)
        nl = node_lists[c]
        real = nl >= 0
        y_full[nl[real]] = yc[real]
    return y_full


# revision 45
# speedup vs baseline: 1.3629x; 1.0058x over previous
"""NodeAttention (GNN scatter-softmax attention) on 8 Trainium2 NeuronCores.

Strategy (final — on-chip KV build, slot-budget tile groups):
- Host deals nodes to 8 cores round-robin by degree rank, so every core sees a
  near-identical degree profile; one static NEFF serves all cores (SPMD).
- Per core: 49 node-tiles x 128 nodes; a tile holds a dense padded slot grid
  [128, D] (D = max in-tile degree across cores, padded even). Consecutive
  tiles are greedily grouped so each iteration covers G*D <= 32 slots: small
  tiles share one iteration's fixed per-instruction costs.
- The host replicates source-node features per SLOT in k-major order, so each
  KV projection matmul's PSUM output lands with the TARGET node on
  partitions. K|V drain PSUM->SBUF in ONE feature-major copy per 8-slot
  chunk ([128, 128feat, slots]) on the Scalar engine — no DRAM staging, no
  gather, no transpose round-trip.
- Scores: bf16 QK mul on GpSimd (Pool) from the feature-major K, then a
  packed w-fold chain on DVE (slots innermost keeps every fold in the 2x
  perf mode); per-edge bias via block-diagonal matmuls (3 slots x 34
  ef-features on 102 partitions) accumulated in fp32; Exp on ACT; softmax
  normalization AFTER aggregation (denominator constant within a node).
- Aggregation: attn*V mul on Pool (attn broadcast rides a middle dim), one
  slot-fold on Pool, fp32 reduce on DVE; out-projection via PE transpose +
  matmul with Wo in bf16; residual (x + bo) folded host-side.
- GPSIMD cannot touch PSUM, so all PSUM traffic stays on ACT/DVE/PE; one
  pinned ACT function table (ln+exp+copy) serves softmax Exp, the LayerNorm
  rsd = exp(-0.5*ln(var+eps)), and all PSUM copies without table swaps.
- LayerNorm + y write-back run in 9 chunks interleaved with the main loop so
  the epilogue overlaps compute; DMA queues are spread (xt on SP, weights/eft
  on ACT, xq/gamma/beta/y on Pool SWDGE) so transfers overlap.
- No max-subtraction in softmax (scores are O(10); identical result).
  Padding slots masked via an extra edge-feature column (weight 1, value -75).
- temp/sqrt(d_h) folded into Wq; temp folded into We; be via a ones column.
"""

import os
import json
import numpy as np
import ml_dtypes

import concourse.bass as bass
import concourse.bacc as bacc
import concourse.tile as tile
from concourse import mybir
from concourse.bass_utils import run_bass_kernel_spmd
from concourse.masks import make_identity

N, E = 50000, 800000
D_NODE, D_EDGE, H = 64, 32, 4
D_H = D_NODE // H
LN_EPS = 1e-5
NCORES = 8
P = 128
NT = 49                # node tiles per core
NPC = NT * P           # padded nodes per core = 6272
GRP = int(os.environ.get('KERNEL_GRP', '1'))   # tiles per iteration
EF_R = 34              # 32 ef features + mask col + ones col (carries be)
EF3 = 3 * EF_R         # 102: three slots stacked on partitions
MASK_VAL = -75.0
F32 = mybir.dt.float32
BF16 = mybir.dt.bfloat16
BF_NP = ml_dtypes.bfloat16


# ---------------------------------------------------------------- host prep --
def _host_prep(node_features, edge_features, edge_index, Wq, bq, Wk, bk, Wv, bv,
               We, be, Wo, bo, ln_gamma, ln_beta, log_temp):
    x = np.ascontiguousarray(np.asarray(node_features, dtype=np.float32))
    ef = np.ascontiguousarray(np.asarray(edge_features, dtype=np.float32))
    src = np.asarray(edge_index[0], dtype=np.int64)
    tgt = np.asarray(edge_index[1], dtype=np.int64)
    temp = np.exp(np.asarray(log_temp, dtype=np.float32))

    deg = np.bincount(tgt, minlength=N)
    order = np.argsort(-deg, kind="stable")
    node_lists = []
    for c in range(NCORES):
        nl = order[c::NCORES]
        nl = np.concatenate([nl, np.full(NPC - len(nl), -1, dtype=np.int64)])
        node_lists.append(nl)

    D_t = np.zeros(NT, dtype=np.int64)
    for c in range(NCORES):
        d = np.where(node_lists[c] >= 0, deg[np.maximum(node_lists[c], 0)], 0)
        D_t = np.maximum(D_t, d.reshape(NT, P).max(axis=1))
    D_t = np.maximum(D_t, 2)
    D_t = D_t + (D_t % 2)          # even D so the aggregation fold halves cleanly
    # iterations: greedy-group consecutive tiles to a slot budget so small
    # tiles share one iteration's fixed costs (G*D <= SLMAX, tiles padded to
    # the group max D)
    SLMAX = int(os.environ.get("KERNEL_SLMAX", "32"))
    iters = []                     # (t0, G, D)
    t = 0
    while t < NT:
        if SLMAX:
            G = 1
            while (t + G < NT and (G + 1) * int(D_t[t]) <= SLMAX
                   and G < 8):
                G += 1
        else:
            G = min(GRP, NT - t)
        D = int(D_t[t:t + G].max())
        D_t[t:t + G] = D
        iters.append((t, G, D))
        t += G
    if os.environ.get("KERNEL_REV", "0") == "1":
        iters = iters[::-1]
    assert D_t.max() <= 128, f"degree {D_t.max()} exceeds single-bank design"
    SD = int(D_t.sum())
    KC_i = [-(-(G * D) // 3) for (_, G, D) in iters]
    SKC = sum(KC_i)

    eorder = np.argsort(tgt, kind="stable")
    estart = np.zeros(N + 1, dtype=np.int64)
    np.cumsum(deg, out=estart[1:])

    qscale = (np.repeat(temp, D_H) / np.sqrt(D_H)).astype(np.float32)
    Wq_aug = (np.concatenate([np.asarray(Wq).T, np.asarray(bq)[None, :]], 0)
              * qscale[None, :]).astype(BF_NP)                           # [65,64]
    Wkv_aug = np.concatenate(
        [np.concatenate([np.asarray(Wk).T, np.asarray(Wv).T], 1),
         np.concatenate([np.asarray(bk), np.asarray(bv)])[None, :]], 0
    ).astype(BF_NP)                                                      # [65,128]
    We_augT = np.concatenate(
        [np.asarray(We).T * temp[None, :],
         np.ones((1, H), np.float32),
         (np.asarray(be) * temp)[None, :]], 0
    ).astype(np.float32)                                                 # [34,4]
    We_blk = np.zeros((EF3, 3 * H), dtype=np.float32)
    for j3 in range(3):
        We_blk[j3 * EF_R:(j3 + 1) * EF_R, j3 * H:(j3 + 1) * H] = We_augT
    We_blk = We_blk.astype(BF_NP)
    Wo_T = np.ascontiguousarray(np.asarray(Wo).T).astype(BF_NP)          # [64,64]
    gb = np.stack([np.asarray(ln_gamma), np.asarray(ln_beta)]).astype(np.float32)

    x_aug = np.concatenate(
        [x, np.ones((N, 1), np.float32)], 1).astype(BF_NP)               # [N,65]

    per_core = []
    for c in range(NCORES):
        nl = node_lists[c]
        efT = np.zeros((EF3, SKC * P), dtype=BF_NP)
        xTc = np.zeros((65, SD * P), dtype=BF_NP)
        coff = 0
        koff = 0
        for it, (t0, G, D) in enumerate(iters):
            SL = G * D
            KC = KC_i[it]
            # gather edge ids for each tile in the group: slot j = g*D + d
            nlt = nl[t0 * P:(t0 + G) * P].reshape(G, P)          # [G,P]
            degt = np.where(nlt >= 0, deg[np.maximum(nlt, 0)], 0)
            k = np.arange(D)
            valid = k[None, None, :] < degt[:, :, None]          # [G,P,D]
            pos = estart[np.maximum(nlt, 0)][:, :, None] + k[None, None, :]
            eids = eorder[np.minimum(pos, E - 1)]
            eids = np.where(valid, eids, 0)
            gsrc = np.where(valid, src[eids], -1)                # [G,P,D]
            # slot (p, j=g*D+d) lives at xTc column (coff + j)*128 + p
            cols = ((coff + np.arange(SL).reshape(G, 1, D)) * P
                    + np.arange(P)[None, :, None])               # [G,P,D]
            xv = np.where(valid.reshape(-1)[:, None],
                          x_aug[np.maximum(gsrc.reshape(-1), 0)],
                          0).astype(BF_NP)
            xTc[:, cols.reshape(-1)] = xv.T
            # edge-feature bias blocks over the group's SL slots
            blk = np.zeros((P, KC * 3, EF_R), dtype=np.float32)
            blk[:, :, D_EDGE] = MASK_VAL
            efv = np.where(valid[:, :, :, None], ef[eids], 0.0)  # [G,P,D,Ef]
            efv = efv.transpose(1, 0, 2, 3).reshape(P, SL, D_EDGE)
            vmask = valid.transpose(1, 0, 2).reshape(P, SL)
            blk[:, :SL, :D_EDGE] = efv
            blk[:, :SL, D_EDGE] = np.where(vmask, 0.0, MASK_VAL)
            blk[:, :, D_EDGE + 1] = 1.0
            # [P, KC, 3, EF_R] -> [3, EF_R, KC, P] -> [102, KC*128]
            efT[:, koff * P:(koff + KC) * P] = (
                blk.reshape(P, KC, 3, EF_R).transpose(2, 3, 1, 0)
                .reshape(EF3, KC * P).astype(BF_NP))
            coff += SL
            koff += KC
        xq = np.where(nl[:, None] >= 0, x[np.maximum(nl, 0)], 0.0).astype(np.float32)
        xqT_aug = np.concatenate([xq.T, np.ones((1, NPC), np.float32)],
                                 0).astype(BF_NP)
        xqr = (xq + np.asarray(bo, dtype=np.float32)[None, :])
        xq_g = np.ascontiguousarray(
            xqr.reshape(NT, P, D_NODE).transpose(1, 0, 2).reshape(P, NT * D_NODE))
        per_core.append({
            "efT": efT,
            "xTc": xTc,
            "xqT": np.ascontiguousarray(xqT_aug),
            "xq": xq_g,
            "wq": Wq_aug,
            "wkv": np.ascontiguousarray(Wkv_aug),
            "we": np.ascontiguousarray(We_blk),
            "wo": Wo_T,
            "gb": gb,
        })
    meta = dict(iters=iters)
    return per_core, node_lists, meta


# ------------------------------------------------------------- bass kernel --
def _build_kernel(meta, debug_mode=None):
    if debug_mode is None:
        debug_mode = os.environ.get("KERNEL_DEBUG_MODE", "")
    iters = meta["iters"]
    KC_i = [-(-(G * D) // 3) for (_, G, D) in iters]
    SD = sum(G * D for (_, G, D) in iters)
    SKC = sum(KC_i)
    # eft groups, aligned to iteration KC blocks
    NG = int(os.environ.get("KERNEL_NG", "7"))
    tgt_sz = -(-SKC // NG)
    gsz = []
    acc = 0
    for kc in KC_i:
        if acc + kc > tgt_sz and acc > 0:
            gsz.append(acc)
            acc = 0
        acc += kc
    gsz.append(acc)
    nc = bacc.Bacc(None, target_bir_lowering=False)

    # engine assignment knobs for the fungible ops ("dve" | "pool")
    ENG = dict(qk="pool", exv="pool", scf1="dve", scf2="dve",
               unnf1="pool", ln_a="pool", ln_b="dve")
    ENG.update(json.loads(os.environ.get("KERNEL_ENG", "{}")))
    CFG = dict(vsplit=0, chunked_ep=1, eftq="sync", wq_q="scalar",
               xqq="gpsimd", xtb=3, smb=4, midb=4, ktb=4, ksplit=0, chunk=8,
               kvb=2, xti=1, epb=[6, 12, 18, 24, 30, 36, 42, 46, 49],
               eftb=2,
               vtail=999)
    CFG.update(json.loads(os.environ.get("KERNEL_CFG", "{}")))

    def VE(key):
        return nc.gpsimd if ENG[key] == "pool" else nc.vector

    efT = nc.dram_tensor("efT", [EF3, SKC * P], BF16, kind="ExternalInput")
    xTc = nc.dram_tensor("xTc", [65, SD * P], BF16, kind="ExternalInput")
    xqT = nc.dram_tensor("xqT", [65, NPC], BF16, kind="ExternalInput")
    xq = nc.dram_tensor("xq", [P, NT * D_NODE], F32, kind="ExternalInput")
    wq = nc.dram_tensor("wq", [65, D_NODE], BF16, kind="ExternalInput")
    wkv = nc.dram_tensor("wkv", [65, 2 * D_NODE], BF16, kind="ExternalInput")
    we = nc.dram_tensor("we", [EF3, 3 * H], BF16, kind="ExternalInput")
    wo = nc.dram_tensor("wo", [D_NODE, D_NODE], BF16, kind="ExternalInput")
    gb = nc.dram_tensor("gb", [2, D_NODE], F32, kind="ExternalInput")
    y = nc.dram_tensor("y", [P, NT * D_NODE], F32, kind="ExternalOutput")

    with tile.TileContext(nc) as tc:
        with (
            tc.tile_pool(name="singles", bufs=1) as singles,
        ):
            wq_sb = singles.tile([65, D_NODE], BF16)
            getattr(nc, CFG["wq_q"]).dma_start(out=wq_sb[:], in_=wq[:])
            wkv_sb = singles.tile([65, 2 * D_NODE], BF16)
            getattr(nc, CFG["wq_q"]).dma_start(out=wkv_sb[:], in_=wkv[:])
            we_sb = singles.tile([EF3, 3 * H], BF16)
            getattr(nc, CFG["wq_q"]).dma_start(out=we_sb[:], in_=we[:])
            wo_sb = singles.tile([D_NODE, D_NODE], BF16)
            getattr(nc, CFG["wq_q"]).dma_start(out=wo_sb[:], in_=wo[:])
            gamma_sb = singles.tile([P, D_NODE], F32)
            getattr(nc, CFG["xqq"]).dma_start(
                out=gamma_sb[:],
                in_=bass.AP(tensor=gb[:].tensor, offset=0,
                            ap=[[0, P], [1, D_NODE]]))
            beta_sb = singles.tile([P, D_NODE], F32)
            getattr(nc, CFG["xqq"]).dma_start(
                out=beta_sb[:],
                in_=bass.AP(tensor=gb[:].tensor, offset=D_NODE,
                            ap=[[0, P], [1, D_NODE]]))
            xqT_sb = singles.tile([65, NPC], BF16)
            for qc in range(4):
                c0 = (NPC // 4) * qc
                c1 = (NPC // 4) * (qc + 1) if qc < 3 else NPC
                getattr(nc, CFG["wq_q"]).dma_start(
                    out=xqT_sb[:, c0:c1], in_=xqT[:, c0:c1])
            xq_sb = singles.tile([P, NT, D_NODE], F32)
            getattr(nc, CFG["xqq"]).dma_start(out=xq_sb[:], in_=xq[:])
            ident = singles.tile([P, P], BF16)
            make_identity(nc, ident[:])
            eps_sb = singles.tile([P, 1], F32)
            nc.vector.memset(eps_sb[:], LN_EPS)
            q_all = singles.tile([P, NT, D_NODE], BF16)
            yout_sb = singles.tile([P, NT, D_NODE], F32)
            mv_sb = singles.tile([P, NT, 2], F32)
            sd_sb = singles.tile([P, NT], F32)
            rsd_sb = singles.tile([P, NT], F32)
            mursd_sb = singles.tile([P, NT], F32)

            with (
                tc.tile_pool(name="xtp", bufs=CFG["xtb"]) as xtp,
                tc.tile_pool(name="kvp", bufs=CFG["kvb"], space="PSUM") as kvp,
                tc.tile_pool(name="ktp", bufs=CFG["ktb"]) as ktp,
                tc.tile_pool(name="vtp", bufs=CFG["ktb"]) as vtp,
                tc.tile_pool(name="eft", bufs=CFG["eftb"]) as eftp,
                tc.tile_pool(name="mid", bufs=CFG["midb"]) as midp,
                tc.tile_pool(name="sml", bufs=CFG["smb"]) as smlp,
                tc.tile_pool(name="pb", bufs=2, space="PSUM") as pb,
                tc.tile_pool(name="pt", bufs=1, space="PSUM") as ptp,
                tc.tile_pool(name="py", bufs=1, space="PSUM") as pyp,
            ):
                # pin the ln+exp+copy ACT table once; the auto-placement pass
                # would otherwise bounce between exp-only and ln tables
                nc.scalar.add_instruction(mybir.InstLoadActFuncSet(
                    name=nc.get_next_instruction_name(), ins=[], outs=[],
                    act_func_set_id=6))

                # ---- all Q' tiles upfront: 49 matmuls, 13 PSUM drains ----
                for g in range(-(-NT // 4)):
                    n4 = min(4, NT - g * 4)
                    qp = pyp.tile([P, 4, D_NODE], F32, name="qp", tag="yp")
                    for j in range(n4):
                        t = g * 4 + j
                        nc.tensor.matmul(
                            out=qp[:, j, :],
                            lhsT=xqT_sb[:, t * P:(t + 1) * P],
                            rhs=wq_sb[:], start=True, stop=True)
                    nc.vector.tensor_copy(out=q_all[:, g * 4:g * 4 + n4, :],
                                          in_=qp[:, 0:n4, :])

                coff = [0]
                koff = [0]
                goff = [0]
                gi = [0]
                gleft = [0]
                gbase = [0]
                eft_sb = [None]
                ep_done = 0
                ep_cnt = 0
                ep_mark = [10 ** 9, -1]
                xt_sb = [None, 0, 0]     # tile, slot offset, iters left
                built = {}               # it -> (kvF, biasp)

                def build_kv(it):
                    t0, G, D = iters[it]
                    SL = G * D
                    KC = KC_i[it]
                    # source features, k-major slots; one DMA covers XTI iters
                    if xt_sb[0] is None:
                        nsl = sum(g * d for (_, g, d)
                                  in iters[it:it + CFG["xti"]])
                        xt_sb[0] = xtp.tile([65, nsl * P], BF16, name="xt_sb",
                                            tag="xt")
                        nc.sync.dma_start(
                            out=xt_sb[0][:],
                            in_=xTc[:, coff[0] * P:(coff[0] + nsl) * P])
                        xt_sb[1] = 0
                        xt_sb[2] = CFG["xti"]
                    if gleft[0] == 0:
                        skc = gsz[gi[0]]
                        eft_sb[0] = eftp.tile([EF3, skc, P], BF16, tag="eft",
                                              name="eft_sb")
                        getattr(nc, CFG["eftq"]).dma_start(
                            out=eft_sb[0][:],
                            in_=efT[:, goff[0] * P:(goff[0] + skc) * P])
                        gi[0] += 1
                        gleft[0] = skc
                        gbase[0] = goff[0]
                        goff[0] += skc
                    # position of this iter's KC blocks within the group
                    kbase = koff[0] - gbase[0]

                    # KV build: target node on partitions, no DRAM staging
                    CH = CFG["chunk"]
                    kvF = vtp.tile([P, 2 * D_NODE, SL], BF16, name="kvF",
                                   tag="vt")
                    xo = xt_sb[1]
                    for c8 in range(-(-SL // CH)):
                        j0 = c8 * CH
                        kk = min(CH, SL - j0)
                        pt = kvp.tile([P, CH, 2 * D_NODE], F32, name="pt",
                                      tag="kv")
                        for j in range(kk):
                            nc.tensor.matmul(
                                out=pt[:, j, :],
                                lhsT=xt_sb[0][:, (xo + j0 + j) * P:
                                              (xo + j0 + j + 1) * P],
                                rhs=wkv_sb[:], start=True, stop=True)
                        on_dve = ((CFG["vsplit"]
                                   and (it * 7 + c8) % CFG["vsplit"] == 0)
                                  or it >= CFG["vtail"])
                        if on_dve:
                            nc.vector.tensor_copy(
                                out=kvF[:, :, j0:j0 + kk],
                                in_=pt[:, 0:kk, :].rearrange("p j f -> p f j"))
                        else:
                            nc.scalar.copy(
                                out=kvF[:, :, j0:j0 + kk],
                                in_=pt[:, 0:kk, :].rearrange("p j f -> p f j"))

                    # per-edge bias: 3 slots per matmul
                    biasp = pb.tile([P, 3 * KC, H], F32, tag="biasp",
                                    name="biasp")
                    for k in range(KC):
                        nc.tensor.matmul(out=biasp[:, 3 * k:3 * (k + 1), :],
                                         lhsT=eft_sb[0][:, kbase + k, :],
                                         rhs=we_sb[:], start=True, stop=True)
                    built[it] = (kvF, biasp)
                    coff[0] += SL
                    koff[0] += KC
                    gleft[0] -= KC
                    xt_sb[1] += SL
                    xt_sb[2] -= 1
                    if xt_sb[2] == 0:
                        xt_sb[0] = None

                build_kv(0)
                for it, (t0, G, D) in enumerate(iters):
                    SL = G * D
                    KC = KC_i[it]
                    if it + 1 < len(iters):
                        build_kv(it + 1)
                    kvF, biasp = built.pop(it)

                    # ---- scores: QK mul then a w-fold chain (slots stay
                    # innermost-packed so every fold runs in DVE 2x mode) ----
                    qkp = midp.tile([P, D_NODE, SL], BF16, tag="qkp",
                                    name="qkp")
                    q_f = bass.AP(tensor=q_all[:].tensor,
                                  offset=q_all[:].offset + t0 * D_NODE,
                                  ap=[q_all[:].ap[0], [1, D_NODE],
                                      [D_NODE, G], [0, D]])
                    VE("qk").tensor_mul(
                        out=qkp[:].rearrange("p f (g d) -> p f g d", g=G),
                        in0=kvF[:, 0:D_NODE, :].rearrange(
                            "p f (g d) -> p f g d", g=G),
                        in1=q_f)
                    qk4 = qkp[:].rearrange("p (h w) s -> p h w s", h=H)
                    sf1 = smlp.tile([P, H, 8, SL], BF16, tag="sf1", name="sf1")
                    VE("scf1").tensor_add(out=sf1[:], in0=qk4[:, :, 0:8, :],
                                          in1=qk4[:, :, 8:16, :])
                    sf2 = smlp.tile([P, H, 4, SL], BF16, tag="sf2", name="sf2")
                    VE("scf2").tensor_add(out=sf2[:], in0=sf1[:, :, 0:4, :],
                                          in1=sf1[:, :, 4:8, :])
                    sf3 = smlp.tile([P, H, 2, SL], BF16, tag="sf3", name="sf3")
                    nc.vector.tensor_add(out=sf3[:], in0=sf2[:, :, 0:2, :],
                                         in1=sf2[:, :, 2:4, :])
                    sc2 = smlp.tile([P, H, SL], F32, tag="sc2", name="sc2")
                    nc.vector.tensor_add(out=sc2[:], in0=sf3[:, :, 0, :],
                                         in1=sf3[:, :, 1, :])
                    bias_h = biasp[:, 0:SL, :].rearrange("p s h -> p h s")
                    nc.vector.tensor_add(out=sc2[:], in0=sc2[:], in1=bias_h)
                    ex = smlp.tile([P, H, SL], BF16, tag="ex", name="ex")
                    nc.scalar.activation(
                        out=ex[:], in_=sc2[:],
                        func=mybir.ActivationFunctionType.Exp)

                    den = smlp.tile([P, H, G], F32, tag="den", name="den")
                    nc.vector.tensor_reduce(
                        out=den[:],
                        in_=ex[:].rearrange("p h (g d) -> p h g d", g=G),
                        axis=mybir.AxisListType.X, op=mybir.AluOpType.add)
                    rden = smlp.tile([P, H, G], F32, tag="rden", name="rden")
                    nc.gpsimd.tensor_scalar_add(den[:], den[:], 1e-10)
                    nc.vector.reciprocal(out=rden[:], in_=den[:])

                    # ---- weighted V aggregation: mul, fold, reduce ----
                    exv = midp.tile([P, D_NODE, SL], BF16, tag="exv",
                                    name="exv")
                    ex_b = bass.AP(tensor=ex[:].tensor, offset=ex[:].offset,
                                   ap=[ex[:].ap[0], [SL, H], [0, D_H],
                                       [1, SL]])
                    VE("exv").tensor_mul(out=exv[:],
                                         in0=kvF[:, D_NODE:2 * D_NODE, :],
                                         in1=ex_b)
                    hD = D // 2
                    exv4 = exv[:].rearrange("p w (g d) -> p w g d", g=G)
                    uf1 = midp.tile([P, D_NODE, G, hD], BF16, tag="uf1",
                                    name="uf1")
                    VE("unnf1").tensor_add(out=uf1[:], in0=exv4[:, :, :, 0:hD],
                                           in1=exv4[:, :, :, hD:D])
                    unn = smlp.tile([P, D_NODE, G], F32, tag="unn", name="unn")
                    nc.vector.tensor_reduce(
                        out=unn[:], in_=uf1[:], axis=mybir.AxisListType.X,
                        op=mybir.AluOpType.add)
                    outn = smlp.tile([P, D_NODE, G], BF16, tag="outn",
                                     name="outn")
                    rden_b = bass.AP(tensor=rden[:].tensor,
                                     offset=rden[:].offset,
                                     ap=[rden[:].ap[0], [G, H], [0, D_H],
                                         [1, G]])
                    nc.gpsimd.tensor_mul(
                        out=outn[:].rearrange("p (h w) g -> p h w g", h=H),
                        in0=unn[:].rearrange("p (h w) g -> p h w g", h=H),
                        in1=rden_b)

                    # ---- projection: y = outn @ Wo.T + (x + bo) ----
                    tp = ptp.tile([D_NODE, G, P], BF16, tag="tp", name="tp")
                    for g in range(G):
                        nc.tensor.transpose(out=tp[:, g, :],
                                            in_=outn[:, :, g],
                                            identity=ident[:])
                    tps = smlp.tile([D_NODE, G, P], BF16, tag="tps",
                                    name="tps")
                    nc.vector.tensor_copy(out=tps[:], in_=tp[:])
                    yp = pyp.tile([P, G, D_NODE], F32, tag="yp", name="yp")
                    for g in range(G):
                        nc.tensor.matmul(out=yp[:, g, :], lhsT=tps[:, g, :],
                                         rhs=wo_sb[:], start=True, stop=True)
                    nc.vector.tensor_add(out=yout_sb[:, t0:t0 + G, :],
                                         in0=yp[:],
                                         in1=xq_sb[:, t0:t0 + G, :])
                    for g in range(G):
                        stats = smlp.tile([P, 6], F32, tag="stats",
                                          name="stats")
                        nc.vector.bn_stats(out=stats[:],
                                           in_=yout_sb[:, t0 + g, :])
                        nc.vector.bn_aggr(out=mv_sb[:, t0 + g, :],
                                          in_=stats[:])

                    # ---- chunked layernorm epilogue: overlap with loop ----
                    ep_cnt += G
                    ep_bounds = (tuple(CFG["epb"]) if CFG["chunked_ep"]
                                 else (NT,))
                    hit = [b for b in ep_bounds if ep_done < b <= ep_cnt]
                    if not debug_mode and hit:
                        # completed-and-unnormalized tiles form one contiguous
                        # range in either iteration order
                        lo = min(t0, ep_mark[0])
                        hi = max(t0 + G, ep_mark[1])
                        te0, te1 = lo, hi
                        ep_mark[0], ep_mark[1] = 10 ** 9, -1
                        nch = te1 - te0
                        mvs = mv_sb[:, te0:te1, :]
                        mu = bass.AP(tensor=mvs.tensor, offset=mvs.offset,
                                     ap=[mvs.ap[0], [2, nch]])
                        var = bass.AP(tensor=mvs.tensor, offset=mvs.offset + 1,
                                      ap=[mvs.ap[0], [2, nch]])
                        # rsd = exp(-0.5*ln(var+eps)); ln+exp share one ACT
                        # function table (sqrt would force a table swap)
                        nc.scalar.activation(
                            out=sd_sb[:, te0:te1], in_=var,
                            func=mybir.ActivationFunctionType.Ln,
                            bias=eps_sb[:])
                        nc.scalar.activation(
                            out=rsd_sb[:, te0:te1], in_=sd_sb[:, te0:te1],
                            func=mybir.ActivationFunctionType.Exp,
                            scale=-0.5)
                        nc.vector.tensor_mul(out=mursd_sb[:, te0:te1], in0=mu,
                                             in1=rsd_sb[:, te0:te1])

                        def bc_t(a):   # [P, nch] -> [P, nch, 64]
                            return bass.AP(tensor=a.tensor, offset=a.offset,
                                           ap=list(a.ap) + [[0, D_NODE]])

                        def bc_f(a):   # [P, 64] -> [P, nch, 64]
                            return bass.AP(tensor=a.tensor, offset=a.offset,
                                           ap=[a.ap[0], [0, nch], a.ap[1]])

                        yv = yout_sb[:, te0:te1, :]
                        VE("ln_a").tensor_mul(out=yv, in0=yv,
                                              in1=bc_t(rsd_sb[:, te0:te1]))
                        VE("ln_b").tensor_sub(out=yv, in0=yv,
                                              in1=bc_t(mursd_sb[:, te0:te1]))
                        VE("ln_a").tensor_mul(out=yv, in0=yv,
                                              in1=bc_f(gamma_sb[:]))
                        VE("ln_b").tensor_add(out=yv, in0=yv,
                                              in1=bc_f(beta_sb[:]))
                        getattr(nc, CFG["xqq"]).dma_start(
                            out=y[:, te0 * D_NODE:te1 * D_NODE], in_=yv)
                        ep_done = ep_cnt
                    else:
                        ep_mark[0] = min(ep_mark[0], t0)
                        ep_mark[1] = max(ep_mark[1], t0 + G)

    nc.compile()
    return nc


# ------------------------------------------------------------------ driver --
def kernel(**inputs) -> np.ndarray:
    per_core, node_lists, meta = _host_prep(**inputs)
    nc = _build_kernel(meta)
    res = run_bass_kernel_spmd(nc, per_core, core_ids=list(range(NCORES)))
    y_full = np.zeros((N, D_NODE), dtype=np.float32)
    for c in range(NCORES):
        yc = res.results[c]["y"].reshape(P, NT, D_NODE).transpose(1, 0, 2)
        yc = yc.reshape(NPC, D_NODE)
        nl = node_lists[c]
        real = nl >= 0
        y_full[nl[real]] = yc[real]
    return y_full


# revision 47
# speedup vs baseline: 1.4001x; 1.0273x over previous
"""NodeAttention (GNN scatter-softmax attention) on 8 Trainium2 NeuronCores.

Strategy (final — on-chip KV build, slot-budget tile groups):
- Host deals nodes to 8 cores round-robin by degree rank, so every core sees a
  near-identical degree profile; one static NEFF serves all cores (SPMD).
- Per core: 49 node-tiles x 128 nodes; a tile holds a dense padded slot grid
  [128, D] (D = max in-tile degree across cores, padded even). Consecutive
  tiles are greedily grouped so each iteration covers G*D <= 32 slots: small
  tiles share one iteration's fixed per-instruction costs.
- The host replicates source-node features per SLOT in k-major order, so each
  KV projection matmul's PSUM output lands with the TARGET node on
  partitions. K|V drain PSUM->SBUF in ONE feature-major copy per 8-slot
  chunk ([128, 128feat, slots]) on the Scalar engine — no DRAM staging, no
  gather, no transpose round-trip.
- Scores: bf16 QK mul on GpSimd (Pool) from the feature-major K, then a
  packed w-fold chain on DVE (slots innermost keeps every fold in the 2x
  perf mode); per-edge bias via block-diagonal matmuls (3 slots x 34
  ef-features on 102 partitions) accumulated in fp32; Exp on ACT; softmax
  normalization AFTER aggregation (denominator constant within a node).
- Aggregation: attn*V mul on Pool (attn broadcast rides a middle dim), one
  slot-fold on Pool, fp32 reduce on DVE; out-projection via PE transpose +
  matmul with Wo in bf16; residual (x + bo) folded host-side.
- GPSIMD cannot touch PSUM, so all PSUM traffic stays on ACT/DVE/PE; one
  pinned ACT function table (ln+exp+copy) serves softmax Exp, the LayerNorm
  rsd = exp(-0.5*ln(var+eps)), and all PSUM copies without table swaps.
- LayerNorm + y write-back run in 9 chunks interleaved with the main loop so
  the epilogue overlaps compute; DMA queues are spread (xt on SP, weights/eft
  on ACT, xq/gamma/beta/y on Pool SWDGE) so transfers overlap.
- No max-subtraction in softmax (scores are O(10); identical result).
  Padding slots masked via an extra edge-feature column (weight 1, value -75).
- temp/sqrt(d_h) folded into Wq; temp folded into We; be via a ones column.
"""

import os
import json
import numpy as np
import ml_dtypes

import concourse.bass as bass
import concourse.bacc as bacc
import concourse.tile as tile
from concourse import mybir
from concourse.bass_utils import run_bass_kernel_spmd
from concourse.masks import make_identity

N, E = 50000, 800000
D_NODE, D_EDGE, H = 64, 32, 4
D_H = D_NODE // H
LN_EPS = 1e-5
NCORES = 8
P = 128
NT = 49                # node tiles per core
NPC = NT * P           # padded nodes per core = 6272
GRP = int(os.environ.get('KERNEL_GRP', '1'))   # tiles per iteration
EF_R = 34              # 32 ef features + mask col + ones col (carries be)
EF3 = 3 * EF_R         # 102: three slots stacked on partitions
MASK_VAL = -75.0
F32 = mybir.dt.float32
BF16 = mybir.dt.bfloat16
BF_NP = ml_dtypes.bfloat16


# ---------------------------------------------------------------- host prep --
def _host_prep(node_features, edge_features, edge_index, Wq, bq, Wk, bk, Wv, bv,
               We, be, Wo, bo, ln_gamma, ln_beta, log_temp):
    x = np.ascontiguousarray(np.asarray(node_features, dtype=np.float32))
    ef = np.ascontiguousarray(np.asarray(edge_features, dtype=np.float32))
    src = np.asarray(edge_index[0], dtype=np.int64)
    tgt = np.asarray(edge_index[1], dtype=np.int64)
    temp = np.exp(np.asarray(log_temp, dtype=np.float32))

    deg = np.bincount(tgt, minlength=N)
    order = np.argsort(-deg, kind="stable")
    node_lists = []
    for c in range(NCORES):
        nl = order[c::NCORES]
        nl = np.concatenate([nl, np.full(NPC - len(nl), -1, dtype=np.int64)])
        node_lists.append(nl)

    D_t = np.zeros(NT, dtype=np.int64)
    for c in range(NCORES):
        d = np.where(node_lists[c] >= 0, deg[np.maximum(node_lists[c], 0)], 0)
        D_t = np.maximum(D_t, d.reshape(NT, P).max(axis=1))
    D_t = np.maximum(D_t, 2)
    D_t = D_t + (D_t % 2)          # even D so the aggregation fold halves cleanly
    # iterations: greedy-group consecutive tiles to a slot budget so small
    # tiles share one iteration's fixed costs (G*D <= SLMAX, tiles padded to
    # the group max D)
    SLMAX = int(os.environ.get("KERNEL_SLMAX", "32"))
    iters = []                     # (t0, G, D)
    t = 0
    while t < NT:
        if SLMAX:
            G = 1
            while (t + G < NT and (G + 1) * int(D_t[t]) <= SLMAX
                   and G < 8):
                G += 1
        else:
            G = min(GRP, NT - t)
        D = int(D_t[t:t + G].max())
        D_t[t:t + G] = D
        iters.append((t, G, D))
        t += G
    if os.environ.get("KERNEL_REV", "0") == "1":
        iters = iters[::-1]
    assert D_t.max() <= 128, f"degree {D_t.max()} exceeds single-bank design"
    SD = int(D_t.sum())
    KC_i = [-(-(G * D) // 3) for (_, G, D) in iters]
    SKC = sum(KC_i)

    eorder = np.argsort(tgt, kind="stable")
    estart = np.zeros(N + 1, dtype=np.int64)
    np.cumsum(deg, out=estart[1:])

    qscale = (np.repeat(temp, D_H) / np.sqrt(D_H)).astype(np.float32)
    Wq_aug = (np.concatenate([np.asarray(Wq).T, np.asarray(bq)[None, :]], 0)
              * qscale[None, :]).astype(BF_NP)                           # [65,64]
    Wkv_aug = np.concatenate(
        [np.concatenate([np.asarray(Wk).T, np.asarray(Wv).T], 1),
         np.concatenate([np.asarray(bk), np.asarray(bv)])[None, :]], 0
    ).astype(BF_NP)                                                      # [65,128]
    We_augT = np.concatenate(
        [np.asarray(We).T * temp[None, :],
         np.ones((1, H), np.float32),
         (np.asarray(be) * temp)[None, :]], 0
    ).astype(np.float32)                                                 # [34,4]
    We_blk = np.zeros((EF3, 3 * H), dtype=np.float32)
    for j3 in range(3):
        We_blk[j3 * EF_R:(j3 + 1) * EF_R, j3 * H:(j3 + 1) * H] = We_augT
    We_blk = We_blk.astype(BF_NP)
    Wo_T = np.ascontiguousarray(np.asarray(Wo).T).astype(BF_NP)          # [64,64]
    gb = np.stack([np.asarray(ln_gamma), np.asarray(ln_beta)]).astype(np.float32)

    x_aug = np.concatenate(
        [x, np.ones((N, 1), np.float32)], 1).astype(BF_NP)               # [N,65]

    per_core = []
    for c in range(NCORES):
        nl = node_lists[c]
        efT = np.zeros((EF3, SKC * P), dtype=BF_NP)
        xTc = np.zeros((65, SD * P), dtype=BF_NP)
        coff = 0
        koff = 0
        for it, (t0, G, D) in enumerate(iters):
            SL = G * D
            KC = KC_i[it]
            # gather edge ids for each tile in the group: slot j = g*D + d
            nlt = nl[t0 * P:(t0 + G) * P].reshape(G, P)          # [G,P]
            degt = np.where(nlt >= 0, deg[np.maximum(nlt, 0)], 0)
            k = np.arange(D)
            valid = k[None, None, :] < degt[:, :, None]          # [G,P,D]
            pos = estart[np.maximum(nlt, 0)][:, :, None] + k[None, None, :]
            eids = eorder[np.minimum(pos, E - 1)]
            eids = np.where(valid, eids, 0)
            gsrc = np.where(valid, src[eids], -1)                # [G,P,D]
            # slot (p, j=g*D+d) lives at xTc column (coff + j)*128 + p
            cols = ((coff + np.arange(SL).reshape(G, 1, D)) * P
                    + np.arange(P)[None, :, None])               # [G,P,D]
            xv = np.where(valid.reshape(-1)[:, None],
                          x_aug[np.maximum(gsrc.reshape(-1), 0)],
                          0).astype(BF_NP)
            xTc[:, cols.reshape(-1)] = xv.T
            # edge-feature bias blocks over the group's SL slots
            blk = np.zeros((P, KC * 3, EF_R), dtype=np.float32)
            blk[:, :, D_EDGE] = MASK_VAL
            efv = np.where(valid[:, :, :, None], ef[eids], 0.0)  # [G,P,D,Ef]
            efv = efv.transpose(1, 0, 2, 3).reshape(P, SL, D_EDGE)
            vmask = valid.transpose(1, 0, 2).reshape(P, SL)
            blk[:, :SL, :D_EDGE] = efv
            blk[:, :SL, D_EDGE] = np.where(vmask, 0.0, MASK_VAL)
            blk[:, :, D_EDGE + 1] = 1.0
            # [P, KC, 3, EF_R] -> [3, EF_R, KC, P] -> [102, KC*128]
            efT[:, koff * P:(koff + KC) * P] = (
                blk.reshape(P, KC, 3, EF_R).transpose(2, 3, 1, 0)
                .reshape(EF3, KC * P).astype(BF_NP))
            coff += SL
            koff += KC
        xq = np.where(nl[:, None] >= 0, x[np.maximum(nl, 0)], 0.0).astype(np.float32)
        xqT_aug = np.concatenate([xq.T, np.ones((1, NPC), np.float32)],
                                 0).astype(BF_NP)
        xqr = (xq + np.asarray(bo, dtype=np.float32)[None, :])
        xq_g = np.ascontiguousarray(
            xqr.reshape(NT, P, D_NODE).transpose(1, 0, 2).reshape(P, NT * D_NODE))
        per_core.append({
            "efT": efT,
            "xTc": xTc,
            "xqT": np.ascontiguousarray(xqT_aug),
            "xq": xq_g,
            "wq": Wq_aug,
            "wkv": np.ascontiguousarray(Wkv_aug),
            "we": np.ascontiguousarray(We_blk),
            "wo": Wo_T,
            "gb": gb,
        })
    meta = dict(iters=iters)
    return per_core, node_lists, meta


# ------------------------------------------------------------- bass kernel --
def _build_kernel(meta, debug_mode=None):
    if debug_mode is None:
        debug_mode = os.environ.get("KERNEL_DEBUG_MODE", "")
    iters = meta["iters"]
    KC_i = [-(-(G * D) // 3) for (_, G, D) in iters]
    SD = sum(G * D for (_, G, D) in iters)
    SKC = sum(KC_i)
    # eft groups, aligned to iteration KC blocks
    NG = int(os.environ.get("KERNEL_NG", "7"))
    tgt_sz = -(-SKC // NG)
    gsz = []
    acc = 0
    for kc in KC_i:
        if acc + kc > tgt_sz and acc > 0:
            gsz.append(acc)
            acc = 0
        acc += kc
    gsz.append(acc)
    nc = bacc.Bacc(None, target_bir_lowering=False)

    # engine assignment knobs for the fungible ops ("dve" | "pool")
    ENG = dict(qk="pool", exv="pool", scf1="dve", scf2="dve",
               unnf1="pool", ln_a="pool", ln_b="dve")
    ENG.update(json.loads(os.environ.get("KERNEL_ENG", "{}")))
    CFG = dict(vsplit=0, chunked_ep=1, eftq="sync", wq_q="scalar",
               xqq="gpsimd", xtb=3, smb=4, midb=4, ktb=4, ksplit=0, chunk=8,
               kvb=2, xti=1, epb=[6, 12, 18, 24, 30, 36, 42, 46, 49],
               eftb=2,
               vtail=999, tail_it=9999, yq="sync")
    CFG.update(json.loads(os.environ.get("KERNEL_CFG", "{}")))

    def VE(key, it=None):
        if (it is not None and key in ("exv", "unnf1")
                and it >= CFG["tail_it"]):
            return nc.vector
        return nc.gpsimd if ENG[key] == "pool" else nc.vector

    efT = nc.dram_tensor("efT", [EF3, SKC * P], BF16, kind="ExternalInput")
    xTc = nc.dram_tensor("xTc", [65, SD * P], BF16, kind="ExternalInput")
    xqT = nc.dram_tensor("xqT", [65, NPC], BF16, kind="ExternalInput")
    xq = nc.dram_tensor("xq", [P, NT * D_NODE], F32, kind="ExternalInput")
    wq = nc.dram_tensor("wq", [65, D_NODE], BF16, kind="ExternalInput")
    wkv = nc.dram_tensor("wkv", [65, 2 * D_NODE], BF16, kind="ExternalInput")
    we = nc.dram_tensor("we", [EF3, 3 * H], BF16, kind="ExternalInput")
    wo = nc.dram_tensor("wo", [D_NODE, D_NODE], BF16, kind="ExternalInput")
    gb = nc.dram_tensor("gb", [2, D_NODE], F32, kind="ExternalInput")
    y = nc.dram_tensor("y", [P, NT * D_NODE], F32, kind="ExternalOutput")

    with tile.TileContext(nc) as tc:
        with (
            tc.tile_pool(name="singles", bufs=1) as singles,
        ):
            wq_sb = singles.tile([65, D_NODE], BF16)
            getattr(nc, CFG["wq_q"]).dma_start(out=wq_sb[:], in_=wq[:])
            wkv_sb = singles.tile([65, 2 * D_NODE], BF16)
            getattr(nc, CFG["wq_q"]).dma_start(out=wkv_sb[:], in_=wkv[:])
            we_sb = singles.tile([EF3, 3 * H], BF16)
            getattr(nc, CFG["wq_q"]).dma_start(out=we_sb[:], in_=we[:])
            wo_sb = singles.tile([D_NODE, D_NODE], BF16)
            getattr(nc, CFG["wq_q"]).dma_start(out=wo_sb[:], in_=wo[:])
            gamma_sb = singles.tile([P, D_NODE], F32)
            getattr(nc, CFG["xqq"]).dma_start(
                out=gamma_sb[:],
                in_=bass.AP(tensor=gb[:].tensor, offset=0,
                            ap=[[0, P], [1, D_NODE]]))
            beta_sb = singles.tile([P, D_NODE], F32)
            getattr(nc, CFG["xqq"]).dma_start(
                out=beta_sb[:],
                in_=bass.AP(tensor=gb[:].tensor, offset=D_NODE,
                            ap=[[0, P], [1, D_NODE]]))
            xqT_sb = singles.tile([65, NPC], BF16)
            for qc in range(4):
                c0 = (NPC // 4) * qc
                c1 = (NPC // 4) * (qc + 1) if qc < 3 else NPC
                getattr(nc, CFG["wq_q"]).dma_start(
                    out=xqT_sb[:, c0:c1], in_=xqT[:, c0:c1])
            xq_sb = singles.tile([P, NT, D_NODE], F32)
            getattr(nc, CFG["xqq"]).dma_start(out=xq_sb[:], in_=xq[:])
            ident = singles.tile([P, P], BF16)
            make_identity(nc, ident[:])
            eps_sb = singles.tile([P, 1], F32)
            nc.vector.memset(eps_sb[:], LN_EPS)
            q_all = singles.tile([P, NT, D_NODE], BF16)
            yout_sb = singles.tile([P, NT, D_NODE], F32)
            mv_sb = singles.tile([P, NT, 2], F32)
            sd_sb = singles.tile([P, NT], F32)
            rsd_sb = singles.tile([P, NT], F32)
            mursd_sb = singles.tile([P, NT], F32)

            with (
                tc.tile_pool(name="xtp", bufs=CFG["xtb"]) as xtp,
                tc.tile_pool(name="kvp", bufs=CFG["kvb"], space="PSUM") as kvp,
                tc.tile_pool(name="ktp", bufs=CFG["ktb"]) as ktp,
                tc.tile_pool(name="vtp", bufs=CFG["ktb"]) as vtp,
                tc.tile_pool(name="eft", bufs=CFG["eftb"]) as eftp,
                tc.tile_pool(name="mid", bufs=CFG["midb"]) as midp,
                tc.tile_pool(name="sml", bufs=CFG["smb"]) as smlp,
                tc.tile_pool(name="pb", bufs=2, space="PSUM") as pb,
                tc.tile_pool(name="pt", bufs=1, space="PSUM") as ptp,
                tc.tile_pool(name="py", bufs=1, space="PSUM") as pyp,
            ):
                # pin the ln+exp+copy ACT table once; the auto-placement pass
                # would otherwise bounce between exp-only and ln tables
                nc.scalar.add_instruction(mybir.InstLoadActFuncSet(
                    name=nc.get_next_instruction_name(), ins=[], outs=[],
                    act_func_set_id=6))

                # ---- all Q' tiles upfront: 49 matmuls, 13 PSUM drains ----
                for g in range(-(-NT // 4)):
                    n4 = min(4, NT - g * 4)
                    qp = pyp.tile([P, 4, D_NODE], F32, name="qp", tag="yp")
                    for j in range(n4):
                        t = g * 4 + j
                        nc.tensor.matmul(
                            out=qp[:, j, :],
                            lhsT=xqT_sb[:, t * P:(t + 1) * P],
                            rhs=wq_sb[:], start=True, stop=True)
                    nc.vector.tensor_copy(out=q_all[:, g * 4:g * 4 + n4, :],
                                          in_=qp[:, 0:n4, :])

                coff = [0]
                koff = [0]
                goff = [0]
                gi = [0]
                gleft = [0]
                gbase = [0]
                eft_sb = [None]
                ep_done = 0
                ep_cnt = 0
                ep_mark = [10 ** 9, -1]
                xt_sb = [None, 0, 0]     # tile, slot offset, iters left
                built = {}               # it -> (kvF, biasp)

                def build_kv(it):
                    t0, G, D = iters[it]
                    SL = G * D
                    KC = KC_i[it]
                    # source features, k-major slots; one DMA covers XTI iters
                    if xt_sb[0] is None:
                        nsl = sum(g * d for (_, g, d)
                                  in iters[it:it + CFG["xti"]])
                        xt_sb[0] = xtp.tile([65, nsl * P], BF16, name="xt_sb",
                                            tag="xt")
                        nc.sync.dma_start(
                            out=xt_sb[0][:],
                            in_=xTc[:, coff[0] * P:(coff[0] + nsl) * P])
                        xt_sb[1] = 0
                        xt_sb[2] = CFG["xti"]
                    if gleft[0] == 0:
                        skc = gsz[gi[0]]
                        eft_sb[0] = eftp.tile([EF3, skc, P], BF16, tag="eft",
                                              name="eft_sb")
                        getattr(nc, CFG["eftq"]).dma_start(
                            out=eft_sb[0][:],
                            in_=efT[:, goff[0] * P:(goff[0] + skc) * P])
                        gi[0] += 1
                        gleft[0] = skc
                        gbase[0] = goff[0]
                        goff[0] += skc
                    # position of this iter's KC blocks within the group
                    kbase = koff[0] - gbase[0]

                    # KV build: target node on partitions, no DRAM staging
                    CH = CFG["chunk"]
                    kvF = vtp.tile([P, 2 * D_NODE, SL], BF16, name="kvF",
                                   tag="vt")
                    xo = xt_sb[1]
                    for c8 in range(-(-SL // CH)):
                        j0 = c8 * CH
                        kk = min(CH, SL - j0)
                        pt = kvp.tile([P, CH, 2 * D_NODE], F32, name="pt",
                                      tag="kv")
                        for j in range(kk):
                            nc.tensor.matmul(
                                out=pt[:, j, :],
                                lhsT=xt_sb[0][:, (xo + j0 + j) * P:
                                              (xo + j0 + j + 1) * P],
                                rhs=wkv_sb[:], start=True, stop=True)
                        on_dve = ((CFG["vsplit"]
                                   and (it * 7 + c8) % CFG["vsplit"] == 0)
                                  or it >= CFG["vtail"])
                        if on_dve:
                            nc.vector.tensor_copy(
                                out=kvF[:, :, j0:j0 + kk],
                                in_=pt[:, 0:kk, :].rearrange("p j f -> p f j"))
                        else:
                            nc.scalar.copy(
                                out=kvF[:, :, j0:j0 + kk],
                                in_=pt[:, 0:kk, :].rearrange("p j f -> p f j"))

                    # per-edge bias: 3 slots per matmul
                    biasp = pb.tile([P, 3 * KC, H], F32, tag="biasp",
                                    name="biasp")
                    for k in range(KC):
                        nc.tensor.matmul(out=biasp[:, 3 * k:3 * (k + 1), :],
                                         lhsT=eft_sb[0][:, kbase + k, :],
                                         rhs=we_sb[:], start=True, stop=True)
                    built[it] = (kvF, biasp)
                    coff[0] += SL
                    koff[0] += KC
                    gleft[0] -= KC
                    xt_sb[1] += SL
                    xt_sb[2] -= 1
                    if xt_sb[2] == 0:
                        xt_sb[0] = None

                build_kv(0)
                for it, (t0, G, D) in enumerate(iters):
                    SL = G * D
                    KC = KC_i[it]
                    if it + 1 < len(iters):
                        build_kv(it + 1)
                    kvF, biasp = built.pop(it)

                    # ---- scores: QK mul then a w-fold chain (slots stay
                    # innermost-packed so every fold runs in DVE 2x mode) ----
                    qkp = midp.tile([P, D_NODE, SL], BF16, tag="qkp",
                                    name="qkp")
                    q_f = bass.AP(tensor=q_all[:].tensor,
                                  offset=q_all[:].offset + t0 * D_NODE,
                                  ap=[q_all[:].ap[0], [1, D_NODE],
                                      [D_NODE, G], [0, D]])
                    VE("qk").tensor_mul(
                        out=qkp[:].rearrange("p f (g d) -> p f g d", g=G),
                        in0=kvF[:, 0:D_NODE, :].rearrange(
                            "p f (g d) -> p f g d", g=G),
                        in1=q_f)
                    qk4 = qkp[:].rearrange("p (h w) s -> p h w s", h=H)
                    sf1 = smlp.tile([P, H, 8, SL], BF16, tag="sf1", name="sf1")
                    VE("scf1").tensor_add(out=sf1[:], in0=qk4[:, :, 0:8, :],
                                          in1=qk4[:, :, 8:16, :])
                    sf2 = smlp.tile([P, H, 4, SL], BF16, tag="sf2", name="sf2")
                    VE("scf2").tensor_add(out=sf2[:], in0=sf1[:, :, 0:4, :],
                                          in1=sf1[:, :, 4:8, :])
                    sf3 = smlp.tile([P, H, 2, SL], BF16, tag="sf3", name="sf3")
                    nc.vector.tensor_add(out=sf3[:], in0=sf2[:, :, 0:2, :],
                                         in1=sf2[:, :, 2:4, :])
                    sc2 = smlp.tile([P, H, SL], F32, tag="sc2", name="sc2")
                    nc.vector.tensor_add(out=sc2[:], in0=sf3[:, :, 0, :],
                                         in1=sf3[:, :, 1, :])
                    bias_h = biasp[:, 0:SL, :].rearrange("p s h -> p h s")
                    nc.vector.tensor_add(out=sc2[:], in0=sc2[:], in1=bias_h)
                    ex = smlp.tile([P, H, SL], BF16, tag="ex", name="ex")
                    nc.scalar.activation(
                        out=ex[:], in_=sc2[:],
                        func=mybir.ActivationFunctionType.Exp)

                    den = smlp.tile([P, H, G], F32, tag="den", name="den")
                    nc.vector.tensor_reduce(
                        out=den[:],
                        in_=ex[:].rearrange("p h (g d) -> p h g d", g=G),
                        axis=mybir.AxisListType.X, op=mybir.AluOpType.add)
                    rden = smlp.tile([P, H, G], F32, tag="rden", name="rden")
                    nc.gpsimd.tensor_scalar_add(den[:], den[:], 1e-10)
                    nc.vector.reciprocal(out=rden[:], in_=den[:])

                    # ---- weighted V aggregation: mul, fold, reduce ----
                    exv = midp.tile([P, D_NODE, SL], BF16, tag="exv",
                                    name="exv")
                    ex_b = bass.AP(tensor=ex[:].tensor, offset=ex[:].offset,
                                   ap=[ex[:].ap[0], [SL, H], [0, D_H],
                                       [1, SL]])
                    VE("exv", it).tensor_mul(out=exv[:],
                                         in0=kvF[:, D_NODE:2 * D_NODE, :],
                                         in1=ex_b)
                    hD = D // 2
                    exv4 = exv[:].rearrange("p w (g d) -> p w g d", g=G)
                    uf1 = midp.tile([P, D_NODE, G, hD], BF16, tag="uf1",
                                    name="uf1")
                    VE("unnf1", it).tensor_add(out=uf1[:], in0=exv4[:, :, :, 0:hD],
                                           in1=exv4[:, :, :, hD:D])
                    unn = smlp.tile([P, D_NODE, G], F32, tag="unn", name="unn")
                    nc.vector.tensor_reduce(
                        out=unn[:], in_=uf1[:], axis=mybir.AxisListType.X,
                        op=mybir.AluOpType.add)
                    outn = smlp.tile([P, D_NODE, G], BF16, tag="outn",
                                     name="outn")
                    rden_b = bass.AP(tensor=rden[:].tensor,
                                     offset=rden[:].offset,
                                     ap=[rden[:].ap[0], [G, H], [0, D_H],
                                         [1, G]])
                    nc.gpsimd.tensor_mul(
                        out=outn[:].rearrange("p (h w) g -> p h w g", h=H),
                        in0=unn[:].rearrange("p (h w) g -> p h w g", h=H),
                        in1=rden_b)

                    # ---- projection: y = outn @ Wo.T + (x + bo) ----
                    tp = ptp.tile([D_NODE, G, P], BF16, tag="tp", name="tp")
                    for g in range(G):
                        nc.tensor.transpose(out=tp[:, g, :],
                                            in_=outn[:, :, g],
                                            identity=ident[:])
                    tps = smlp.tile([D_NODE, G, P], BF16, tag="tps",
                                    name="tps")
                    nc.vector.tensor_copy(out=tps[:], in_=tp[:])
                    yp = pyp.tile([P, G, D_NODE], F32, tag="yp", name="yp")
                    for g in range(G):
                        nc.tensor.matmul(out=yp[:, g, :], lhsT=tps[:, g, :],
                                         rhs=wo_sb[:], start=True, stop=True)
                    nc.vector.tensor_add(out=yout_sb[:, t0:t0 + G, :],
                                         in0=yp[:],
                                         in1=xq_sb[:, t0:t0 + G, :])
                    for g in range(G):
                        stats = smlp.tile([P, 6], F32, tag="stats",
                                          name="stats")
                        nc.vector.bn_stats(out=stats[:],
                                           in_=yout_sb[:, t0 + g, :])
                        nc.vector.bn_aggr(out=mv_sb[:, t0 + g, :],
                                          in_=stats[:])

                    # ---- chunked layernorm epilogue: overlap with loop ----
                    ep_cnt += G
                    ep_bounds = (tuple(CFG["epb"]) if CFG["chunked_ep"]
                                 else (NT,))
                    hit = [b for b in ep_bounds if ep_done < b <= ep_cnt]
                    if not debug_mode and hit:
                        # completed-and-unnormalized tiles form one contiguous
                        # range in either iteration order
                        lo = min(t0, ep_mark[0])
                        hi = max(t0 + G, ep_mark[1])
                        te0, te1 = lo, hi
                        ep_mark[0], ep_mark[1] = 10 ** 9, -1
                        nch = te1 - te0
                        mvs = mv_sb[:, te0:te1, :]
                        mu = bass.AP(tensor=mvs.tensor, offset=mvs.offset,
                                     ap=[mvs.ap[0], [2, nch]])
                        var = bass.AP(tensor=mvs.tensor, offset=mvs.offset + 1,
                                      ap=[mvs.ap[0], [2, nch]])
                        # rsd = exp(-0.5*ln(var+eps)); ln+exp share one ACT
                        # function table (sqrt would force a table swap)
                        nc.scalar.activation(
                            out=sd_sb[:, te0:te1], in_=var,
                            func=mybir.ActivationFunctionType.Ln,
                            bias=eps_sb[:])
                        nc.scalar.activation(
                            out=rsd_sb[:, te0:te1], in_=sd_sb[:, te0:te1],
                            func=mybir.ActivationFunctionType.Exp,
                            scale=-0.5)
                        nc.vector.tensor_mul(out=mursd_sb[:, te0:te1], in0=mu,
                                             in1=rsd_sb[:, te0:te1])

                        def bc_t(a):   # [P, nch] -> [P, nch, 64]
                            return bass.AP(tensor=a.tensor, offset=a.offset,
                                           ap=list(a.ap) + [[0, D_NODE]])

                        def bc_f(a):   # [P, 64] -> [P, nch, 64]
                            return bass.AP(tensor=a.tensor, offset=a.offset,
                                           ap=[a.ap[0], [0, nch], a.ap[1]])

                        yv = yout_sb[:, te0:te1, :]
                        VE("ln_a").tensor_mul(out=yv, in0=yv,
                                              in1=bc_t(rsd_sb[:, te0:te1]))
                        VE("ln_b").tensor_sub(out=yv, in0=yv,
                                              in1=bc_t(mursd_sb[:, te0:te1]))
                        VE("ln_a").tensor_mul(out=yv, in0=yv,
                                              in1=bc_f(gamma_sb[:]))
                        VE("ln_b").tensor_add(out=yv, in0=yv,
                                              in1=bc_f(beta_sb[:]))
                        getattr(nc, CFG["yq"]).dma_start(
                            out=y[:, te0 * D_NODE:te1 * D_NODE], in_=yv)
                        ep_done = ep_cnt
                    else:
                        ep_mark[0] = min(ep_mark[0], t0)
                        ep_mark[1] = max(ep_mark[1], t0 + G)

    nc.compile()
    return nc


# ------------------------------------------------------------------ driver --
def kernel(**inputs) -> np.ndarray:
    per_core, node_lists, meta = _host_prep(**inputs)
    nc = _build_kernel(meta)
    res = run_bass_kernel_spmd(nc, per_core, core_ids=list(range(NCORES)))
    y_full = np.zeros((N, D_NODE), dtype=np.float32)
    for c in range(NCORES):
        yc = res.results[c]["y"].reshape(P, NT, D_NODE).transpose(1, 0, 2)
        yc = yc.reshape(NPC, D_NODE)
        nl = node_lists[c]
        real = nl >= 0
        y_full[nl[real]] = yc[real]
    return y_full


# revision 50
# speedup vs baseline: 1.4270x; 1.0192x over previous
"""NodeAttention (GNN scatter-softmax attention) on 8 Trainium2 NeuronCores.

Strategy (final — on-chip KV build, slot-budget tile groups):
- Host deals nodes to 8 cores round-robin by degree rank, so every core sees a
  near-identical degree profile; one static NEFF serves all cores (SPMD).
- Per core: 49 node-tiles x 128 nodes; a tile holds a dense padded slot grid
  [128, D] (D = max in-tile degree across cores, padded even). Consecutive
  tiles are greedily grouped so each iteration covers G*D <= 32 slots: small
  tiles share one iteration's fixed per-instruction costs.
- The host replicates source-node features per SLOT in k-major order, so each
  KV projection matmul's PSUM output lands with the TARGET node on
  partitions. K|V drain PSUM->SBUF in ONE feature-major copy per 8-slot
  chunk ([128, 128feat, slots]) on the Scalar engine — no DRAM staging, no
  gather, no transpose round-trip.
- Scores: bf16 QK mul on GpSimd (Pool) from the feature-major K, then a
  packed w-fold chain on DVE (slots innermost keeps every fold in the 2x
  perf mode); per-edge bias via block-diagonal matmuls (3 slots x 34
  ef-features on 102 partitions) accumulated in fp32; Exp on ACT; softmax
  normalization AFTER aggregation (denominator constant within a node).
- Aggregation: attn*V mul on Pool (attn broadcast rides a middle dim), one
  slot-fold on Pool, fp32 reduce on DVE; out-projection via PE transpose +
  matmul with Wo in bf16; residual (x + bo) folded host-side.
- GPSIMD cannot touch PSUM, so all PSUM traffic stays on ACT/DVE/PE; one
  pinned ACT function table (ln+exp+copy) serves softmax Exp, the LayerNorm
  rsd = exp(-0.5*ln(var+eps)), and all PSUM copies without table swaps.
- LayerNorm + y write-back run in 9 chunks interleaved with the main loop so
  the epilogue overlaps compute; DMA queues are spread (xt on SP, weights/eft
  on ACT, xq/gamma/beta/y on Pool SWDGE) so transfers overlap.
- No max-subtraction in softmax (scores are O(10); identical result).
  Padding slots masked via an extra edge-feature column (weight 1, value -75).
- temp/sqrt(d_h) folded into Wq; temp folded into We; be via a ones column.
"""

import os
import json
import numpy as np
import ml_dtypes

import concourse.bass as bass
import concourse.bacc as bacc
import concourse.tile as tile
from concourse import mybir
from concourse.bass_utils import run_bass_kernel_spmd
from concourse.masks import make_identity

N, E = 50000, 800000
D_NODE, D_EDGE, H = 64, 32, 4
D_H = D_NODE // H
LN_EPS = 1e-5
NCORES = 8
P = 128
NT = 49                # node tiles per core
NPC = NT * P           # padded nodes per core = 6272
GRP = int(os.environ.get('KERNEL_GRP', '1'))   # tiles per iteration
EF_R = 34              # 32 ef features + mask col + ones col (carries be)
EF3 = 3 * EF_R         # 102: three slots stacked on partitions
MASK_VAL = -75.0
F32 = mybir.dt.float32
BF16 = mybir.dt.bfloat16
BF_NP = ml_dtypes.bfloat16


# ---------------------------------------------------------------- host prep --
def _host_prep(node_features, edge_features, edge_index, Wq, bq, Wk, bk, Wv, bv,
               We, be, Wo, bo, ln_gamma, ln_beta, log_temp):
    x = np.ascontiguousarray(np.asarray(node_features, dtype=np.float32))
    ef = np.ascontiguousarray(np.asarray(edge_features, dtype=np.float32))
    src = np.asarray(edge_index[0], dtype=np.int64)
    tgt = np.asarray(edge_index[1], dtype=np.int64)
    temp = np.exp(np.asarray(log_temp, dtype=np.float32))

    deg = np.bincount(tgt, minlength=N)
    order = np.argsort(-deg, kind="stable")
    node_lists = []
    for c in range(NCORES):
        nl = order[c::NCORES]
        nl = np.concatenate([nl, np.full(NPC - len(nl), -1, dtype=np.int64)])
        node_lists.append(nl)

    D_t = np.zeros(NT, dtype=np.int64)
    for c in range(NCORES):
        d = np.where(node_lists[c] >= 0, deg[np.maximum(node_lists[c], 0)], 0)
        D_t = np.maximum(D_t, d.reshape(NT, P).max(axis=1))
    D_t = np.maximum(D_t, 2)
    D_t = D_t + (D_t % 2)          # even D so the aggregation fold halves cleanly
    # iterations: greedy-group consecutive tiles to a slot budget so small
    # tiles share one iteration's fixed costs (G*D <= SLMAX, tiles padded to
    # the group max D)
    SLMAX = int(os.environ.get("KERNEL_SLMAX", "32"))
    iters = []                     # (t0, G, D)
    t = 0
    while t < NT:
        if SLMAX:
            lim = (int(os.environ.get("KERNEL_SLTAIL", "0")) or SLMAX) \
                if t >= int(os.environ.get("KERNEL_TAILT", "40")) else SLMAX
            G = 1
            while (t + G < NT and (G + 1) * int(D_t[t]) <= lim
                   and G < 8):
                G += 1
        else:
            G = min(GRP, NT - t)
        D = int(D_t[t:t + G].max())
        D_t[t:t + G] = D
        iters.append((t, G, D))
        t += G
    if os.environ.get("KERNEL_REV", "0") == "1":
        iters = iters[::-1]
    assert D_t.max() <= 128, f"degree {D_t.max()} exceeds single-bank design"
    SD = int(D_t.sum())
    KC_i = [-(-(G * D) // 3) for (_, G, D) in iters]
    SKC = sum(KC_i)

    eorder = np.argsort(tgt, kind="stable")
    estart = np.zeros(N + 1, dtype=np.int64)
    np.cumsum(deg, out=estart[1:])

    qscale = (np.repeat(temp, D_H) / np.sqrt(D_H)).astype(np.float32)
    Wq_aug = (np.concatenate([np.asarray(Wq).T, np.asarray(bq)[None, :]], 0)
              * qscale[None, :]).astype(BF_NP)                           # [65,64]
    Wkv_aug = np.concatenate(
        [np.concatenate([np.asarray(Wk).T, np.asarray(Wv).T], 1),
         np.concatenate([np.asarray(bk), np.asarray(bv)])[None, :]], 0
    ).astype(BF_NP)                                                      # [65,128]
    We_augT = np.concatenate(
        [np.asarray(We).T * temp[None, :],
         np.ones((1, H), np.float32),
         (np.asarray(be) * temp)[None, :]], 0
    ).astype(np.float32)                                                 # [34,4]
    We_blk = np.zeros((EF3, 3 * H), dtype=np.float32)
    for j3 in range(3):
        We_blk[j3 * EF_R:(j3 + 1) * EF_R, j3 * H:(j3 + 1) * H] = We_augT
    We_blk = We_blk.astype(BF_NP)
    Wo_T = np.ascontiguousarray(np.asarray(Wo).T).astype(BF_NP)          # [64,64]
    gb = np.stack([np.asarray(ln_gamma), np.asarray(ln_beta)]).astype(np.float32)

    x_aug = np.concatenate(
        [x, np.ones((N, 1), np.float32)], 1).astype(BF_NP)               # [N,65]

    per_core = []
    for c in range(NCORES):
        nl = node_lists[c]
        efT = np.zeros((EF3, SKC * P), dtype=BF_NP)
        xTc = np.zeros((65, SD * P), dtype=BF_NP)
        coff = 0
        koff = 0
        for it, (t0, G, D) in enumerate(iters):
            SL = G * D
            KC = KC_i[it]
            # gather edge ids for each tile in the group: slot j = g*D + d
            nlt = nl[t0 * P:(t0 + G) * P].reshape(G, P)          # [G,P]
            degt = np.where(nlt >= 0, deg[np.maximum(nlt, 0)], 0)
            k = np.arange(D)
            valid = k[None, None, :] < degt[:, :, None]          # [G,P,D]
            pos = estart[np.maximum(nlt, 0)][:, :, None] + k[None, None, :]
            eids = eorder[np.minimum(pos, E - 1)]
            eids = np.where(valid, eids, 0)
            gsrc = np.where(valid, src[eids], -1)                # [G,P,D]
            # slot (p, j=g*D+d) lives at xTc column (coff + j)*128 + p
            cols = ((coff + np.arange(SL).reshape(G, 1, D)) * P
                    + np.arange(P)[None, :, None])               # [G,P,D]
            xv = np.where(valid.reshape(-1)[:, None],
                          x_aug[np.maximum(gsrc.reshape(-1), 0)],
                          0).astype(BF_NP)
            xTc[:, cols.reshape(-1)] = xv.T
            # edge-feature bias blocks over the group's SL slots
            blk = np.zeros((P, KC * 3, EF_R), dtype=np.float32)
            blk[:, :, D_EDGE] = MASK_VAL
            efv = np.where(valid[:, :, :, None], ef[eids], 0.0)  # [G,P,D,Ef]
            efv = efv.transpose(1, 0, 2, 3).reshape(P, SL, D_EDGE)
            vmask = valid.transpose(1, 0, 2).reshape(P, SL)
            blk[:, :SL, :D_EDGE] = efv
            blk[:, :SL, D_EDGE] = np.where(vmask, 0.0, MASK_VAL)
            blk[:, :, D_EDGE + 1] = 1.0
            # [P, KC, 3, EF_R] -> [3, EF_R, KC, P] -> [102, KC*128]
            efT[:, koff * P:(koff + KC) * P] = (
                blk.reshape(P, KC, 3, EF_R).transpose(2, 3, 1, 0)
                .reshape(EF3, KC * P).astype(BF_NP))
            coff += SL
            koff += KC
        xq = np.where(nl[:, None] >= 0, x[np.maximum(nl, 0)], 0.0).astype(np.float32)
        xqT_aug = np.concatenate([xq.T, np.ones((1, NPC), np.float32)],
                                 0).astype(BF_NP)
        xqr = (xq + np.asarray(bo, dtype=np.float32)[None, :])
        xq_g = np.ascontiguousarray(
            xqr.reshape(NT, P, D_NODE).transpose(1, 0, 2).reshape(P, NT * D_NODE))
        per_core.append({
            "efT": efT,
            "xTc": xTc,
            "xqT": np.ascontiguousarray(xqT_aug),
            "xq": xq_g,
            "wq": Wq_aug,
            "wkv": np.ascontiguousarray(Wkv_aug),
            "we": np.ascontiguousarray(We_blk),
            "wo": Wo_T,
            "gb": gb,
        })
    meta = dict(iters=iters)
    return per_core, node_lists, meta


# ------------------------------------------------------------- bass kernel --
def _build_kernel(meta, debug_mode=None):
    if debug_mode is None:
        debug_mode = os.environ.get("KERNEL_DEBUG_MODE", "")
    iters = meta["iters"]
    KC_i = [-(-(G * D) // 3) for (_, G, D) in iters]
    SD = sum(G * D for (_, G, D) in iters)
    SKC = sum(KC_i)
    # eft groups, aligned to iteration KC blocks
    NG = int(os.environ.get("KERNEL_NG", "7"))
    tgt_sz = -(-SKC // NG)
    gsz = []
    acc = 0
    for kc in KC_i:
        if acc + kc > tgt_sz and acc > 0:
            gsz.append(acc)
            acc = 0
        acc += kc
    gsz.append(acc)
    nc = bacc.Bacc(None, target_bir_lowering=False)

    # engine assignment knobs for the fungible ops ("dve" | "pool")
    ENG = dict(qk="pool", exv="pool", scf1="dve", scf2="dve",
               unnf1="pool", ln_a="pool", ln_b="dve")
    ENG.update(json.loads(os.environ.get("KERNEL_ENG", "{}")))
    CFG = dict(vsplit=0, chunked_ep=1, eftq="sync", wq_q="scalar",
               xqq="gpsimd", xtb=3, smb=4, midb=4, ktb=4, ksplit=0, chunk=8,
               kvb=2, xti=1, epb=[6, 12, 18, 24, 30, 36, 42, 46, 49],
               eftb=2,
               vtail=999, tail_it=9999, yq="sync", qksplit=8)
    CFG.update(json.loads(os.environ.get("KERNEL_CFG", "{}")))

    def VE(key, it=None):
        if (it is not None and key in ("exv", "unnf1")
                and it >= CFG["tail_it"]):
            return nc.vector
        return nc.gpsimd if ENG[key] == "pool" else nc.vector

    efT = nc.dram_tensor("efT", [EF3, SKC * P], BF16, kind="ExternalInput")
    xTc = nc.dram_tensor("xTc", [65, SD * P], BF16, kind="ExternalInput")
    xqT = nc.dram_tensor("xqT", [65, NPC], BF16, kind="ExternalInput")
    xq = nc.dram_tensor("xq", [P, NT * D_NODE], F32, kind="ExternalInput")
    wq = nc.dram_tensor("wq", [65, D_NODE], BF16, kind="ExternalInput")
    wkv = nc.dram_tensor("wkv", [65, 2 * D_NODE], BF16, kind="ExternalInput")
    we = nc.dram_tensor("we", [EF3, 3 * H], BF16, kind="ExternalInput")
    wo = nc.dram_tensor("wo", [D_NODE, D_NODE], BF16, kind="ExternalInput")
    gb = nc.dram_tensor("gb", [2, D_NODE], F32, kind="ExternalInput")
    y = nc.dram_tensor("y", [P, NT * D_NODE], F32, kind="ExternalOutput")

    with tile.TileContext(nc) as tc:
        with (
            tc.tile_pool(name="singles", bufs=1) as singles,
        ):
            wq_sb = singles.tile([65, D_NODE], BF16)
            getattr(nc, CFG["wq_q"]).dma_start(out=wq_sb[:], in_=wq[:])
            wkv_sb = singles.tile([65, 2 * D_NODE], BF16)
            getattr(nc, CFG["wq_q"]).dma_start(out=wkv_sb[:], in_=wkv[:])
            we_sb = singles.tile([EF3, 3 * H], BF16)
            getattr(nc, CFG["wq_q"]).dma_start(out=we_sb[:], in_=we[:])
            wo_sb = singles.tile([D_NODE, D_NODE], BF16)
            getattr(nc, CFG["wq_q"]).dma_start(out=wo_sb[:], in_=wo[:])
            gamma_sb = singles.tile([P, D_NODE], F32)
            getattr(nc, CFG["xqq"]).dma_start(
                out=gamma_sb[:],
                in_=bass.AP(tensor=gb[:].tensor, offset=0,
                            ap=[[0, P], [1, D_NODE]]))
            beta_sb = singles.tile([P, D_NODE], F32)
            getattr(nc, CFG["xqq"]).dma_start(
                out=beta_sb[:],
                in_=bass.AP(tensor=gb[:].tensor, offset=D_NODE,
                            ap=[[0, P], [1, D_NODE]]))
            xqT_sb = singles.tile([65, NPC], BF16)
            for qc in range(4):
                c0 = (NPC // 4) * qc
                c1 = (NPC // 4) * (qc + 1) if qc < 3 else NPC
                getattr(nc, CFG["wq_q"]).dma_start(
                    out=xqT_sb[:, c0:c1], in_=xqT[:, c0:c1])
            xq_sb = singles.tile([P, NT, D_NODE], F32)
            getattr(nc, CFG["xqq"]).dma_start(out=xq_sb[:], in_=xq[:])
            ident = singles.tile([P, P], BF16)
            make_identity(nc, ident[:])
            eps_sb = singles.tile([P, 1], F32)
            nc.vector.memset(eps_sb[:], LN_EPS)
            q_all = singles.tile([P, NT, D_NODE], BF16)
            yout_sb = singles.tile([P, NT, D_NODE], F32)
            mv_sb = singles.tile([P, NT, 2], F32)
            sd_sb = singles.tile([P, NT], F32)
            rsd_sb = singles.tile([P, NT], F32)
            mursd_sb = singles.tile([P, NT], F32)

            with (
                tc.tile_pool(name="xtp", bufs=CFG["xtb"]) as xtp,
                tc.tile_pool(name="kvp", bufs=CFG["kvb"], space="PSUM") as kvp,
                tc.tile_pool(name="ktp", bufs=CFG["ktb"]) as ktp,
                tc.tile_pool(name="vtp", bufs=CFG["ktb"]) as vtp,
                tc.tile_pool(name="eft", bufs=CFG["eftb"]) as eftp,
                tc.tile_pool(name="mid", bufs=CFG["midb"]) as midp,
                tc.tile_pool(name="sml", bufs=CFG["smb"]) as smlp,
                tc.tile_pool(name="pb", bufs=2, space="PSUM") as pb,
                tc.tile_pool(name="pt", bufs=1, space="PSUM") as ptp,
                tc.tile_pool(name="py", bufs=1, space="PSUM") as pyp,
            ):
                # pin the ln+exp+copy ACT table once; the auto-placement pass
                # would otherwise bounce between exp-only and ln tables
                nc.scalar.add_instruction(mybir.InstLoadActFuncSet(
                    name=nc.get_next_instruction_name(), ins=[], outs=[],
                    act_func_set_id=6))

                # ---- all Q' tiles upfront: 49 matmuls, 13 PSUM drains ----
                for g in range(-(-NT // 4)):
                    n4 = min(4, NT - g * 4)
                    qp = pyp.tile([P, 4, D_NODE], F32, name="qp", tag="yp")
                    for j in range(n4):
                        t = g * 4 + j
                        nc.tensor.matmul(
                            out=qp[:, j, :],
                            lhsT=xqT_sb[:, t * P:(t + 1) * P],
                            rhs=wq_sb[:], start=True, stop=True)
                    nc.vector.tensor_copy(out=q_all[:, g * 4:g * 4 + n4, :],
                                          in_=qp[:, 0:n4, :])

                coff = [0]
                koff = [0]
                goff = [0]
                gi = [0]
                gleft = [0]
                gbase = [0]
                eft_sb = [None]
                ep_done = 0
                ep_cnt = 0
                ep_mark = [10 ** 9, -1]
                xt_sb = [None, 0, 0]     # tile, slot offset, iters left
                built = {}               # it -> (kvF, biasp)

                def build_kv(it):
                    t0, G, D = iters[it]
                    SL = G * D
                    KC = KC_i[it]
                    # source features, k-major slots; one DMA covers XTI iters
                    if xt_sb[0] is None:
                        nsl = sum(g * d for (_, g, d)
                                  in iters[it:it + CFG["xti"]])
                        xt_sb[0] = xtp.tile([65, nsl * P], BF16, name="xt_sb",
                                            tag="xt")
                        nc.sync.dma_start(
                            out=xt_sb[0][:],
                            in_=xTc[:, coff[0] * P:(coff[0] + nsl) * P])
                        xt_sb[1] = 0
                        xt_sb[2] = CFG["xti"]
                    if gleft[0] == 0:
                        skc = gsz[gi[0]]
                        eft_sb[0] = eftp.tile([EF3, skc, P], BF16, tag="eft",
                                              name="eft_sb")
                        getattr(nc, CFG["eftq"]).dma_start(
                            out=eft_sb[0][:],
                            in_=efT[:, goff[0] * P:(goff[0] + skc) * P])
                        gi[0] += 1
                        gleft[0] = skc
                        gbase[0] = goff[0]
                        goff[0] += skc
                    # position of this iter's KC blocks within the group
                    kbase = koff[0] - gbase[0]

                    # KV build: target node on partitions, no DRAM staging
                    CH = CFG["chunk"]
                    kvF = vtp.tile([P, 2 * D_NODE, SL], BF16, name="kvF",
                                   tag="vt")
                    qkp_e = midp.tile([P, D_NODE, SL], BF16, tag="qkp",
                                      name="qkp_e")
                    xo = xt_sb[1]
                    for c8 in range(-(-SL // CH)):
                        j0 = c8 * CH
                        kk = min(CH, SL - j0)
                        pt = kvp.tile([P, CH, 2 * D_NODE], F32, name="pt",
                                      tag="kv")
                        for j in range(kk):
                            nc.tensor.matmul(
                                out=pt[:, j, :],
                                lhsT=xt_sb[0][:, (xo + j0 + j) * P:
                                              (xo + j0 + j + 1) * P],
                                rhs=wkv_sb[:], start=True, stop=True)
                        if it < CFG["qksplit"] and G == 1:
                            # early iterations: score each chunk as it lands
                            # so the first fold isn't gated on the full tile
                            q_c = bass.AP(
                                tensor=q_all[:].tensor,
                                offset=q_all[:].offset + t0 * D_NODE,
                                ap=[q_all[:].ap[0], [1, D_NODE], [0, kk]])
                            # (iteration 0 is a single tile, so one Q row)
                        on_dve = ((CFG["vsplit"]
                                   and (it * 7 + c8) % CFG["vsplit"] == 0)
                                  or it >= CFG["vtail"])
                        if on_dve:
                            nc.vector.tensor_copy(
                                out=kvF[:, :, j0:j0 + kk],
                                in_=pt[:, 0:kk, :].rearrange("p j f -> p f j"))
                        else:
                            nc.scalar.copy(
                                out=kvF[:, :, j0:j0 + kk],
                                in_=pt[:, 0:kk, :].rearrange("p j f -> p f j"))
                        if it < CFG["qksplit"] and G == 1:
                            nc.gpsimd.tensor_mul(
                                out=qkp_e[:, :, j0:j0 + kk],
                                in0=kvF[:, 0:D_NODE, j0:j0 + kk], in1=q_c)

                    # per-edge bias: 3 slots per matmul
                    biasp = pb.tile([P, 3 * KC, H], F32, tag="biasp",
                                    name="biasp")
                    for k in range(KC):
                        nc.tensor.matmul(out=biasp[:, 3 * k:3 * (k + 1), :],
                                         lhsT=eft_sb[0][:, kbase + k, :],
                                         rhs=we_sb[:], start=True, stop=True)
                    built[it] = (kvF, biasp, qkp_e)
                    coff[0] += SL
                    koff[0] += KC
                    gleft[0] -= KC
                    xt_sb[1] += SL
                    xt_sb[2] -= 1
                    if xt_sb[2] == 0:
                        xt_sb[0] = None

                build_kv(0)
                for it, (t0, G, D) in enumerate(iters):
                    SL = G * D
                    KC = KC_i[it]
                    if it + 1 < len(iters):
                        build_kv(it + 1)
                    kvF, biasp, qkp_e = built.pop(it)

                    # ---- scores: QK mul then a w-fold chain (slots stay
                    # innermost-packed so every fold runs in DVE 2x mode) ----
                    qkp = qkp_e
                    q_f = bass.AP(tensor=q_all[:].tensor,
                                  offset=q_all[:].offset + t0 * D_NODE,
                                  ap=[q_all[:].ap[0], [1, D_NODE],
                                      [D_NODE, G], [0, D]])
                    if it >= CFG["qksplit"] or G > 1:
                        VE("qk").tensor_mul(
                            out=qkp[:].rearrange("p f (g d) -> p f g d", g=G),
                            in0=kvF[:, 0:D_NODE, :].rearrange(
                                "p f (g d) -> p f g d", g=G),
                            in1=q_f)
                    qk4 = qkp[:].rearrange("p (h w) s -> p h w s", h=H)
                    sf1 = smlp.tile([P, H, 8, SL], BF16, tag="sf1", name="sf1")
                    VE("scf1").tensor_add(out=sf1[:], in0=qk4[:, :, 0:8, :],
                                          in1=qk4[:, :, 8:16, :])
                    sf2 = smlp.tile([P, H, 4, SL], BF16, tag="sf2", name="sf2")
                    VE("scf2").tensor_add(out=sf2[:], in0=sf1[:, :, 0:4, :],
                                          in1=sf1[:, :, 4:8, :])
                    sf3 = smlp.tile([P, H, 2, SL], BF16, tag="sf3", name="sf3")
                    nc.vector.tensor_add(out=sf3[:], in0=sf2[:, :, 0:2, :],
                                         in1=sf2[:, :, 2:4, :])
                    sc2 = smlp.tile([P, H, SL], F32, tag="sc2", name="sc2")
                    nc.vector.tensor_add(out=sc2[:], in0=sf3[:, :, 0, :],
                                         in1=sf3[:, :, 1, :])
                    bias_h = biasp[:, 0:SL, :].rearrange("p s h -> p h s")
                    nc.vector.tensor_add(out=sc2[:], in0=sc2[:], in1=bias_h)
                    ex = smlp.tile([P, H, SL], BF16, tag="ex", name="ex")
                    nc.scalar.activation(
                        out=ex[:], in_=sc2[:],
                        func=mybir.ActivationFunctionType.Exp)

                    den = smlp.tile([P, H, G], F32, tag="den", name="den")
                    nc.vector.tensor_reduce(
                        out=den[:],
                        in_=ex[:].rearrange("p h (g d) -> p h g d", g=G),
                        axis=mybir.AxisListType.X, op=mybir.AluOpType.add)
                    rden = smlp.tile([P, H, G], F32, tag="rden", name="rden")
                    nc.gpsimd.tensor_scalar_add(den[:], den[:], 1e-10)
                    nc.vector.reciprocal(out=rden[:], in_=den[:])

                    # ---- weighted V aggregation: mul, fold, reduce ----
                    exv = midp.tile([P, D_NODE, SL], BF16, tag="exv",
                                    name="exv")
                    ex_b = bass.AP(tensor=ex[:].tensor, offset=ex[:].offset,
                                   ap=[ex[:].ap[0], [SL, H], [0, D_H],
                                       [1, SL]])
                    VE("exv", it).tensor_mul(out=exv[:],
                                         in0=kvF[:, D_NODE:2 * D_NODE, :],
                                         in1=ex_b)
                    hD = D // 2
                    exv4 = exv[:].rearrange("p w (g d) -> p w g d", g=G)
                    uf1 = midp.tile([P, D_NODE, G, hD], BF16, tag="uf1",
                                    name="uf1")
                    VE("unnf1", it).tensor_add(out=uf1[:], in0=exv4[:, :, :, 0:hD],
                                           in1=exv4[:, :, :, hD:D])
                    unn = smlp.tile([P, D_NODE, G], F32, tag="unn", name="unn")
                    nc.vector.tensor_reduce(
                        out=unn[:], in_=uf1[:], axis=mybir.AxisListType.X,
                        op=mybir.AluOpType.add)
                    outn = smlp.tile([P, D_NODE, G], BF16, tag="outn",
                                     name="outn")
                    rden_b = bass.AP(tensor=rden[:].tensor,
                                     offset=rden[:].offset,
                                     ap=[rden[:].ap[0], [G, H], [0, D_H],
                                         [1, G]])
                    nc.gpsimd.tensor_mul(
                        out=outn[:].rearrange("p (h w) g -> p h w g", h=H),
                        in0=unn[:].rearrange("p (h w) g -> p h w g", h=H),
                        in1=rden_b)

                    # ---- projection: y = outn @ Wo.T + (x + bo) ----
                    tp = ptp.tile([D_NODE, G, P], BF16, tag="tp", name="tp")
                    for g in range(G):
                        nc.tensor.transpose(out=tp[:, g, :],
                                            in_=outn[:, :, g],
                                            identity=ident[:])
                    tps = smlp.tile([D_NODE, G, P], BF16, tag="tps",
                                    name="tps")
                    nc.vector.tensor_copy(out=tps[:], in_=tp[:])
                    yp = pyp.tile([P, G, D_NODE], F32, tag="yp", name="yp")
                    for g in range(G):
                        nc.tensor.matmul(out=yp[:, g, :], lhsT=tps[:, g, :],
                                         rhs=wo_sb[:], start=True, stop=True)
                    nc.vector.tensor_add(out=yout_sb[:, t0:t0 + G, :],
                                         in0=yp[:],
                                         in1=xq_sb[:, t0:t0 + G, :])
                    for g in range(G):
                        stats = smlp.tile([P, 6], F32, tag="stats",
                                          name="stats")
                        nc.vector.bn_stats(out=stats[:],
                                           in_=yout_sb[:, t0 + g, :])
                        nc.vector.bn_aggr(out=mv_sb[:, t0 + g, :],
                                          in_=stats[:])

                    # ---- chunked layernorm epilogue: overlap with loop ----
                    ep_cnt += G
                    ep_bounds = (tuple(CFG["epb"]) if CFG["chunked_ep"]
                                 else (NT,))
                    hit = [b for b in ep_bounds if ep_done < b <= ep_cnt]
                    if not debug_mode and hit:
                        # completed-and-unnormalized tiles form one contiguous
                        # range in either iteration order
                        lo = min(t0, ep_mark[0])
                        hi = max(t0 + G, ep_mark[1])
                        te0, te1 = lo, hi
                        ep_mark[0], ep_mark[1] = 10 ** 9, -1
                        nch = te1 - te0
                        mvs = mv_sb[:, te0:te1, :]
                        mu = bass.AP(tensor=mvs.tensor, offset=mvs.offset,
                                     ap=[mvs.ap[0], [2, nch]])
                        var = bass.AP(tensor=mvs.tensor, offset=mvs.offset + 1,
                                      ap=[mvs.ap[0], [2, nch]])
                        # rsd = exp(-0.5*ln(var+eps)); ln+exp share one ACT
                        # function table (sqrt would force a table swap)
                        nc.scalar.activation(
                            out=sd_sb[:, te0:te1], in_=var,
                            func=mybir.ActivationFunctionType.Ln,
                            bias=eps_sb[:])
                        nc.scalar.activation(
                            out=rsd_sb[:, te0:te1], in_=sd_sb[:, te0:te1],
                            func=mybir.ActivationFunctionType.Exp,
                            scale=-0.5)
                        nc.vector.tensor_mul(out=mursd_sb[:, te0:te1], in0=mu,
                                             in1=rsd_sb[:, te0:te1])

                        def bc_t(a):   # [P, nch] -> [P, nch, 64]
                            return bass.AP(tensor=a.tensor, offset=a.offset,
                                           ap=list(a.ap) + [[0, D_NODE]])

                        def bc_f(a):   # [P, 64] -> [P, nch, 64]
                            return bass.AP(tensor=a.tensor, offset=a.offset,
                                           ap=[a.ap[0], [0, nch], a.ap[1]])

                        yv = yout_sb[:, te0:te1, :]
                        VE("ln_a").tensor_mul(out=yv, in0=yv,
                                              in1=bc_t(rsd_sb[:, te0:te1]))
                        VE("ln_b").tensor_sub(out=yv, in0=yv,
                                              in1=bc_t(mursd_sb[:, te0:te1]))
                        VE("ln_a").tensor_mul(out=yv, in0=yv,
                                              in1=bc_f(gamma_sb[:]))
                        VE("ln_b").tensor_add(out=yv, in0=yv,
                                              in1=bc_f(beta_sb[:]))
                        getattr(nc, CFG["yq"]).dma_start(
                            out=y[:, te0 * D_NODE:te1 * D_NODE], in_=yv)
                        ep_done = ep_cnt
                    else:
                        ep_mark[0] = min(ep_mark[0], t0)
                        ep_mark[1] = max(ep_mark[1], t0 + G)

    nc.compile()
    return nc


# ------------------------------------------------------------------ driver --
def kernel(**inputs) -> np.ndarray:
    per_core, node_lists, meta = _host_prep(**inputs)
    nc = _build_kernel(meta)
    res = run_bass_kernel_spmd(nc, per_core, core_ids=list(range(NCORES)))
    y_full = np.zeros((N, D_NODE), dtype=np.float32)
    for c in range(NCORES):
        yc = res.results[c]["y"].reshape(P, NT, D_NODE).transpose(1, 0, 2)
        yc = yc.reshape(NPC, D_NODE)
        nl = node_lists[c]
        real = nl >= 0
        y_full[nl[real]] = yc[real]
    return y_full
